# revision 39
# baseline (speedup 1.0000x reference)
"""Trainium2 Bass kernel for nn_DecoderBlock (BitNet-style decoder block with
self-attention, cross-attention and BitFeedForward), data-parallel over
(batch x sequence) tokens across 8 NeuronCores.

Sharding: 4096 tokens (B=2 x N=2048) split into 8 shards of 512 tokens.
Cores 0-3 hold batch 0, cores 4-7 batch 1.  Self-attention K/V are
computed on local tokens and exchanged within each 4-core batch group by
FOUR pipelined AllGathers (one per local 128-token tile), and attention
consumes the gathered key tiles in four availability batches, carrying the
softmax numerator and denominator across batches in per-head SBUF
accumulators.  The first quarter lands while the input projections are
still finishing, so the exchange is almost entirely off the critical path.

Weights are ternary-quantized on the host (exact same math as the
reference's _weight_quant) and shipped as bf16 {-1,0,1} in transposed
[in, out] layout, plus one packed row of fp32 scales/LN params.
Activations are fake-quantized on device; integer-valued operands are
exact in bf16, so the bf16 matmul path is exact for the quantized matmuls
(fp32 PSUM accumulation).

Quant statistics run on the Scalar engine (Square with accum_out gives
sum(x^2) per token in one pass) plus one DVE absmax reduce, so the Vector
engine stays off the critical path.  When the LayerNorm params are g=1,b=0
(true for this problem; checked on the host), LayerNorm + the following
BitLinear RMSNorm collapse into one affine normalize:
    rmsnorm(LN(x)) = (x - mean) * rsqrt(var*(1+1e-6) + 1e-11)
with absmax(x - mean) = max(max(x)-mean, mean-min(x)); the LN apply pass
and its stats pass disappear.  The per-head attention-output normalize ops
carry accum_out, so sum(x) per token is free.

Attention: q heads are host-permuted into pairs (0,2),(1,3),(4,6)... so a
q-pair shares one K tile pair; the two 64-contraction score matmuls run
CONCURRENTLY in the PE array as row-tiles (base partitions 0 and 64),
writing one 2-bank PSUM pair that a single Exp activation consumes.
Softmax denominators come free via a ones-column appended to V.
"""

import numpy as np
import ml_dtypes
from contextlib import ExitStack

import concourse.bacc as bacc
import concourse.mybir as mybir
import concourse.tile as tile
from concourse.bass_utils import run_bass_kernel_spmd
from concourse.masks import make_identity

F32 = mybir.dt.float32
BF16 = mybir.dt.bfloat16
AX = mybir.AxisListType
OP = mybir.AluOpType
ACT = mybir.ActivationFunctionType

# model dims
B, N, S, D = 2, 2048, 256, 768
HQ, HK, HEAD = 12, 6, 64
DKV = HEAD * HK          # 384
H4 = 4 * D               # 3072
NCORES = 8
GROUPS = [[0, 1, 2, 3], [4, 5, 6, 7]]
GSZ = 4                  # cores per batch group
T = (B * N) // NCORES    # 512 tokens per core
NT = T // 128            # 4 token tiles per core
ST = S // 128            # 2 condition token tiles
KT = D // 128            # 6 feature tiles of D
KTH = H4 // 128          # 24 feature tiles of 4D
KP = DKV // 128          # 3 kv-head-pair tiles

# q heads permuted so psum pair tile mt holds (QPERM[2mt], QPERM[2mt+1]),
# and both heads of a pair read the same gathered K pair tile.
QPERM = [0, 2, 1, 3, 4, 6, 5, 7, 8, 10, 9, 11]

# (out_features, in_features); device gets ternary bf16 f"{name}_q" [I, O].
WSPECS = {
    'sa_wq': (D, D), 'sa_wk': (DKV, D), 'sa_wv': (DKV, D), 'sa_wo': (D, D),
    'ca_wq': (D, D), 'ca_wk': (DKV, D), 'ca_wv': (DKV, D), 'ca_wo': (D, D),
    'w_cond': (D, D), 'w1': (H4, D), 'w2': (D, H4),
}
SCALE_SLOTS = list(WSPECS)          # order of m scales in the combo row
NSLOT = 16                          # padded scale slots
COMBO_W = NSLOT + 4 * D             # + sa_g, sa_b, ca_g, ca_b

_PROGRAM_CACHE = {}

MAGIC = 12582912.0   # 1.5 * 2^23: fp32 add/sub forces round-half-even to int

# exchange-quarter sizes (elements, bf16)
KSLICE = KP * 128 * 128   # K columns for one 128-token tile, all kp rows
VSLICE = 128 * DKV        # V for one 128-token tile
QSZ = KSLICE + VSLICE


class Ctx:
    pass


# ---------------------------------------------------------------------------
# quant statistics + per-token scale chains
# ---------------------------------------------------------------------------

def _rms_stats_tile(g, X, F, S2, amax, j):
    """Per-tile stats: S2[:, j] = sum(X^2) (Scalar engine Square with
    accum_out), amax[:, j] = max|X| (one DVE reduce).
    (tensor_tensor_reduce would do the square on the DVE, but it crashes
    this runtime -- verified with a minimal repro.)"""
    nc = g.nc
    sq = g.sq_scratch(F)
    nc.scalar.activation(sq, X, ACT.Square, accum_out=S2[:, j:j + 1])
    nc.vector.tensor_reduce(amax[:, j:j + 1], X, axis=AX.X, op=OP.max,
                            apply_absolute_value=True)


def _rms_chain(g, S2, amax, F, nj, uid):
    """al = absmax_n/127 (dequant alpha), srnd = 127*r/absmax_n where
    r = rsqrt(sum(x^2)/F + 1e-6), absmax_n = clip(absmax*r, 1e-5)."""
    nc, qpool = g.nc, g.qpool
    sd = qpool.tile([128, nj], F32, tag=f"qs_{uid}", name=f"qs_{uid}")
    nc.scalar.activation(sd, S2, ACT.Sqrt, bias=g.eps6, scale=1.0 / F)
    r = qpool.tile([128, nj], F32, tag=f"qr_{uid}", name=f"qr_{uid}")
    nc.vector.reciprocal(r, sd)
    amn = qpool.tile([128, nj], F32, tag=f"qm_{uid}", name=f"qm_{uid}")
    nc.vector.tensor_mul(amn, amax, r)
    nc.vector.tensor_scalar_max(amn, amn, 1e-5)
    al = qpool.tile([128, nj], F32, tag=f"al_{uid}", name=f"al_{uid}")
    nc.vector.tensor_scalar_mul(al, amn, 1.0 / 127.0)
    ra = qpool.tile([128, nj], F32, tag=f"qi_{uid}", name=f"qi_{uid}")
    nc.vector.reciprocal(ra, amn)
    srnd = qpool.tile([128, nj], F32, tag=f"qn_{uid}", name=f"qn_{uid}")
    nc.vector.tensor_mul(srnd, ra, r)
    nc.vector.tensor_scalar_mul(srnd, srnd, 127.0)
    return al, srnd


def _quant_tile(g, X, F, srnd_col, dst3, wk, dma_eng):
    """Quantize one token tile: round(x*srnd) via the fp32 magic-number
    trick (DVE mul+add, Act sub) -- integer-exact in bf16; then the
    feature-major transpose into dst3 [128, F//128, 128]."""
    nc = g.nc
    tmp = wk.tile([128, F], F32, tag=f"qt_{F}", name="qt",
                  bufs=(2 if F <= 1024 else 1))
    nc.vector.tensor_scalar(tmp, X, srnd_col, MAGIC, OP.mult, OP.add)
    xq = wk.tile([128, F], BF16, tag=f"xq_{F}", name="xq", bufs=2)
    nc.scalar.activation(xq, tmp, ACT.Copy, bias=-MAGIC)
    dma_eng.dma_start(dst3, xq, transpose=True)
    return xq


def _fused_ln_chain(g, S1, S2, mx, mn, nj, uid):
    """g=1,b=0 fast path: rmsnorm(LN(a)) == (a - m) * R with m = S1/D,
    var = S2/D - m^2, R = rsqrt(var*(1+1e-6) + 1e-11).
    absmax = max(mx - m, m - mn) * R.  Quantized int = (a*cq - dq) - MAGIC
    with cq = R*127/clip(absmax,1e-5), dq = m*cq - MAGIC.
    Returns (al, cq, dq); al is the dequant alpha."""
    nc, qpool = g.nc, g.qpool

    def t(nm):
        return qpool.tile([128, nj], F32, tag=f"{nm}_{uid}",
                          name=f"{nm}_{uid}")
    m = t("lm")
    nc.vector.tensor_scalar_mul(m, S1, 1.0 / D)
    t1 = t("lt")
    nc.vector.tensor_scalar_mul(t1, S2, 1.0 / D)
    msq = t("lq")
    nc.vector.tensor_mul(msq, m, m)
    var = t("lv")
    nc.vector.tensor_sub(var, t1, msq)
    dd = t("ld")
    nc.vector.tensor_scalar(dd, var, 1.0 + 1e-6, 1e-11, OP.mult, OP.add)
    sd = t("ls")
    nc.scalar.activation(sd, dd, ACT.Sqrt)
    R = t("lr")
    nc.vector.reciprocal(R, sd)
    t3 = t("l3")
    nc.vector.tensor_sub(t3, mx, m)
    t4 = t("l4")
    nc.vector.tensor_sub(t4, m, mn)
    am = t("la")
    nc.vector.tensor_tensor(am, t3, t4, op=OP.max)
    amn = t("ln")
    nc.vector.tensor_mul(amn, am, R)
    nc.vector.tensor_scalar_max(amn, amn, 1e-5)
    al = t("al")
    nc.vector.tensor_scalar_mul(al, amn, 1.0 / 127.0)
    ra = t("li")
    nc.vector.reciprocal(ra, amn)
    cq = t("lc")
    nc.vector.tensor_mul(cq, ra, R)
    nc.vector.tensor_scalar_mul(cq, cq, 127.0)
    dq = t("lz")
    nc.vector.tensor_mul(dq, m, cq)
    nc.vector.tensor_scalar(dq, dq, MAGIC, None, OP.subtract)
    return al, cq, dq


def _fused_quant_tile(g, X, cq_col, dq_col, dst3, wk, dma_eng):
    """Quantize one fused-LN tile: (X*cq - dq) - MAGIC, then transpose."""
    nc = g.nc
    tmp = wk.tile([128, D], F32, tag="qt_768", name="qt", bufs=2)
    nc.vector.tensor_scalar(tmp, X, cq_col, dq_col, OP.mult, OP.subtract)
    xq = wk.tile([128, D], BF16, tag="xq_768", name="xq", bufs=2)
    nc.scalar.activation(xq, tmp, ACT.Copy, bias=-MAGIC)
    dma_eng.dma_start(dst3, xq, transpose=True)
    return xq


def _layernorm(g, a_tiles, g_bc, b_bc, out_tiles, uid):
    """General-g/b LayerNorm (fallback path)."""
    nc, qpool = g.nc, g.qpool
    nj = len(a_tiles)
    s1 = qpool.tile([128, nj], F32, tag=f"ls1_{uid}", name=f"ls1_{uid}")
    s2 = qpool.tile([128, nj], F32, tag=f"ls2_{uid}", name=f"ls2_{uid}")
    for j, A in enumerate(a_tiles):
        sq = g.sq_scratch(D)
        nc.scalar.activation(sq, A, ACT.Square, accum_out=s2[:, j:j + 1])
        nc.vector.tensor_reduce(s1[:, j:j + 1], A, axis=AX.X, op=OP.add)
    m = qpool.tile([128, nj], F32, tag=f"lmu_{uid}", name=f"lmu_{uid}")
    nc.vector.tensor_scalar_mul(m, s1, 1.0 / D)
    t1 = qpool.tile([128, nj], F32, tag=f"lt1_{uid}", name=f"lt1_{uid}")
    nc.vector.tensor_scalar_mul(t1, s2, 1.0 / D)
    msq = qpool.tile([128, nj], F32, tag=f"lms_{uid}", name=f"lms_{uid}")
    nc.vector.tensor_mul(msq, m, m)
    var = qpool.tile([128, nj], F32, tag=f"lva_{uid}", name=f"lva_{uid}")
    nc.vector.tensor_sub(var, t1, msq)
    sd = qpool.tile([128, nj], F32, tag=f"lsd_{uid}", name=f"lsd_{uid}")
    nc.scalar.activation(sd, var, ACT.Sqrt, bias=g.eps5)
    rs = qpool.tile([128, nj], F32, tag=f"lrs_{uid}", name=f"lrs_{uid}")
    nc.vector.reciprocal(rs, sd)
    for j, A in enumerate(a_tiles):
        X = out_tiles[j]
        nc.vector.tensor_scalar(X, A, m[:, j:j + 1], rs[:, j:j + 1],
                                OP.subtract, OP.mult)
        nc.vector.tensor_mul(X, X, g_bc)
        nc.vector.tensor_add(X, X, b_bc)


def _make_abc(g, al_mat, nj, Ttot, pool, uid):
    """Row-broadcast of per-token alpha: [128, nj] -> [128, Ttot], done
    entirely on the PE (transpose, then a rank-1 ones matmul per 128-token
    block) so it never queues behind gpsimd weight-prefetch DMAs."""
    nc = g.nc
    abc = pool.tile([128, Ttot], F32, tag=f"abc_{uid}", name=f"abc_{uid}")
    with g.tc.tile_pool(name=f"psabc_{uid}", bufs=1, space="PSUM") as pp:
        pst = pp.tile([nj, 128], F32, tag="ps_abc", name="pst")
        nc.tensor.transpose(pst, al_mat, g.ident)
        at = g.stat.tile([nj, 128], F32, tag="at", name="at", bufs=1)
        nc.scalar.copy(at, pst)
        arow = g.stat.tile([1, Ttot], F32, tag="arow", name="arow", bufs=1)
        for j in range(nj):
            nc.sync.dma_start(arow[0:1, j * 128:(j + 1) * 128],
                              at[j:j + 1, :])
        psb = pp.tile([128, Ttot], F32, tag="ps_abc2", name="psb")
        nc.tensor.matmul(psb, g.ones1, arow[0:1, :], start=True, stop=True)
        nc.vector.tensor_copy(abc, psb)
    return abc


# ---------------------------------------------------------------------------
# attention
# ---------------------------------------------------------------------------

def _attn_norm(g, h, o_sb, a_out, s1cols, psum_t):
    """Per-head transpose + softmax normalize; accum_out gives the
    per-token feature sum of the normalized head chunk for free."""
    nc = g.nc
    for j in range(NT):
        ps_t = psum_t.tile([128, 65], F32, tag="pst", name="ps_t")
        nc.tensor.transpose(ps_t, o_sb[:, j * 128:(j + 1) * 128],
                            g.ident[0:65, 0:65])
        rec = g.stat.tile([128, 1], F32, tag="rec", name="rec")
        nc.vector.reciprocal(rec, ps_t[:, 64:65])
        acc = s1cols[j][:, h:h + 1] if s1cols is not None else None
        nc.vector.tensor_scalar(a_out[j][:, h * 64:(h + 1) * 64],
                                ps_t[:, 0:64], rec, 0.0, OP.mult, OP.add,
                                accum_out=acc)


def _attention(g, batches, k_views, v_views, q_lo, q_hi, a_out, s1cols,
               psum_s, psum_o, psum_t, awork, acc_pool):
    """Batched paired GQA attention.  batches: list of lists of s-tile
    indices in availability order.  The first batch seeds per-head SBUF
    accumulators, middle batches add into them, the last merges and emits
    transposes + normalize.  Single-batch callers skip the accumulators.

    k_views[kp][s]: [128, 128] bf16 (k-heads 2kp/2kp+1 row-tiled);
    v_views[s]: [128, HK, HEAD+1] bf16 (ones column -> denominator)."""
    nc = g.nc
    nb = len(batches)
    accA = accB = None
    if nb > 1:
        accA = acc_pool.tile([65, HQ // 2, 512], F32, name="accA")
        accB = acc_pool.tile([65, HQ // 2, 512], F32, name="accB")
    for b, batch in enumerate(batches):
        first, last = b == 0, b == nb - 1
        for qp in range(HQ // 2):
            hA, hB = QPERM[2 * qp], QPERM[2 * qp + 1]
            khA, khB = hA // 2, hB // 2
            kp = khA // 2
            ps_oA = psum_o.tile([65, 512], F32, tag="pvA", name="pvA")
            ps_oB = psum_o.tile([65, 512], F32, tag="pvB", name="pvB")
            for i, s in enumerate(batch):
                ps_pair = psum_s.tile([128, 1024], F32, tag="pss",
                                      name="pss")
                ps_A, ps_B = ps_pair[:, 0:512], ps_pair[:, 512:1024]
                nc.tensor.matmul(ps_A, k_views[kp][s][0:64, :], q_lo[qp],
                                 start=True, stop=True)
                nc.tensor.matmul(ps_B, k_views[kp][s][64:128, :], q_hi[qp],
                                 start=True, stop=True)
                pT = awork.tile([128, 1024], BF16, tag="pT", name="pT",
                                bufs=2)
                nc.scalar.activation(pT, ps_pair, ACT.Exp)
                nc.tensor.matmul(ps_oA, v_views[s][:, khA, :],
                                 pT[:, 0:512], start=(i == 0),
                                 stop=(i == len(batch) - 1))
                nc.tensor.matmul(ps_oB, v_views[s][:, khB, :],
                                 pT[:, 512:1024], start=(i == 0),
                                 stop=(i == len(batch) - 1))
            if nb == 1:
                for h, ps_o in ((hA, ps_oA), (hB, ps_oB)):
                    o_sb = awork.tile([65, 512], F32, tag="osb",
                                      name="osb", bufs=2)
                    nc.vector.tensor_copy(o_sb, ps_o)
                    _attn_norm(g, h, o_sb, a_out, s1cols, psum_t)
            elif first:
                nc.vector.tensor_copy(accA[:, qp, :], ps_oA)
                nc.vector.tensor_copy(accB[:, qp, :], ps_oB)
            elif not last:
                nc.vector.tensor_add(accA[:, qp, :], accA[:, qp, :], ps_oA)
                nc.vector.tensor_add(accB[:, qp, :], accB[:, qp, :], ps_oB)
            else:
                for h, ps_o, acc in ((hA, ps_oA, accA), (hB, ps_oB, accB)):
                    o_sb = awork.tile([65, 512], F32, tag="osb",
                                      name="osb", bufs=2)
                    nc.vector.tensor_add(o_sb, acc[:, qp, :], ps_o)
                    _attn_norm(g, h, o_sb, a_out, s1cols, psum_t)


# ---------------------------------------------------------------------------
# program builder
# ---------------------------------------------------------------------------

def build_program(groups=None, fused_ln=True):
    if groups is None:
        groups = GROUPS
    gsz = len(groups[0])
    nc = bacc.Bacc()

    x_in = nc.declare_dram_parameter("x_sh", [T, D], F32, isOutput=False)
    y_in = nc.declare_dram_parameter("y_b", [S, D], F32, isOutput=False)
    wt_in = {}
    for name, (O, I) in WSPECS.items():
        wt_in[name] = nc.declare_dram_parameter(f"{name}_q", [I, O], BF16,
                                                isOutput=False)
    combo_in = nc.declare_dram_parameter("combo", [1, COMBO_W], F32,
                                         isOutput=False)
    out_sh = nc.declare_dram_parameter("out_sh", [T, D], F32, isOutput=True)

    g = Ctx()
    g.nc = nc

    with tile.TileContext(nc) as tc, ExitStack() as ctx:
        g.tc = tc
        g.const = ctx.enter_context(tc.tile_pool(name="const", bufs=1))
        g.stat = ctx.enter_context(tc.tile_pool(name="stat", bufs=4))
        g.work = ctx.enter_context(tc.tile_pool(name="work", bufs=2))
        g.qpool = ctx.enter_context(tc.tile_pool(name="qpool", bufs=1))
        sqpool = ctx.enter_context(tc.tile_pool(name="sqpool", bufs=1))
        dram = ctx.enter_context(tc.tile_pool(name="dram", bufs=1,
                                              space="DRAM"))

        def sq_scratch(F):
            return sqpool.tile([128, F], BF16, tag=f"sq_{F}", name="sq")
        g.sq_scratch = sq_scratch

        # four quarter-exchanges, one per local 128-token tile
        cc_in = [dram.tile([QSZ], BF16, name=f"cc_in{i}") for i in range(NT)]
        cc_out = [dram.tile([gsz, QSZ], BF16, name=f"cc_out{i}")
                  for i in range(NT)]

        g.eps6 = g.const.tile([128, 1], F32, name="eps6")
        nc.vector.memset(g.eps6, 1e-6)
        g.eps5 = g.const.tile([128, 1], F32, name="eps5")
        nc.vector.memset(g.eps5, 1e-5)
        g.ident = g.const.tile([128, 128], F32, name="ident")
        make_identity(nc, g.ident)
        identb = g.const.tile([128, 128], BF16, name="identb")
        nc.vector.tensor_copy(identb, g.ident)
        g.ones1 = g.const.tile([1, 128], F32, name="ones1")
        nc.vector.memset(g.ones1, 1.0)

        # one DMA + partition broadcasts for the scales (+ LN params only
        # in the general-g/b fallback path)
        cbw = NSLOT if fused_ln else COMBO_W
        cb = g.const.tile([128, cbw], F32, name="cb")
        with tc.tile_pool(name="crowp", bufs=1) as crowp:
            crow = crowp.tile([1, COMBO_W], F32, name="crow")
            nc.scalar.dma_start(crow, combo_in[:, :])
            nc.gpsimd.partition_broadcast(cb[:, 0:NSLOT], crow[0:1, 0:NSLOT])
            if not fused_ln:
                for i in range(4):
                    sl = slice(NSLOT + i * D, NSLOT + (i + 1) * D)
                    nc.gpsimd.partition_broadcast(cb[:, sl], crow[0:1, sl])
        msc = {name: cb[:, i:i + 1] for i, name in enumerate(SCALE_SLOTS)}
        ln_bc = ({} if fused_ln else
                 {name: cb[:, NSLOT + i * D:NSLOT + (i + 1) * D]
                  for i, name in enumerate(('sa_g', 'sa_b',
                                            'ca_g', 'ca_b'))})

        g.ka_pool = None

        def keepalive(ap, n):
            """Dummy matmuls reading `ap` (bf16, <=512 cols) to hold the PE
            p-state up through otherwise PE-idle stretches."""
            if g.ka_pool is None:
                return
            for _ in range(n):
                ps = g.ka_pool.tile([128, 512], F32, tag="ka", name="ka")
                nc.tensor.matmul(ps, identb, ap, start=True, stop=True)
        g.keepalive = keepalive

        def load_weight(pool, name, eng):
            O, I = WSPECS[name]
            rows = I // 128
            wt = pool.tile([128, rows, O], BF16, tag=f"w_{name}",
                           name=f"w_{name}")
            for r in range(rows):
                eng.dma_start(wt[:, r, :],
                              wt_in[name][r * 128:(r + 1) * 128, :])
            return wt

        def proj_fm(wsb, xqT_all, mscale, abc, O, Ttot, pool, tag, ps_pool):
            """feature-major projection: O//128 tiles [128, Ttot] bf16."""
            nk = xqT_all.shape[1]
            outs = []
            for mt in range(O // 128):
                ps = ps_pool.tile([128, Ttot], F32, tag="ps", name="ps_pf")
                for k in range(nk):
                    nc.tensor.matmul(ps, wsb[:, k, mt * 128:(mt + 1) * 128],
                                     xqT_all[:, k, :], start=(k == 0),
                                     stop=(k == nk - 1))
                o = pool.tile([128, Ttot], BF16, tag=f"{tag}{mt}",
                              name=f"{tag}{mt}")
                nc.vector.scalar_tensor_tensor(o, ps, mscale, abc,
                                               OP.mult, OP.mult)
                outs.append(o)
            return outs

        def proj_tok_resid(xq_j, wsb, al_mat, mscale, resid_tiles,
                           out_tiles, ps_pool, nk=KT, pre=None, post=None):
            """token-major projection + dequant + residual add, with
            per-tile pre (quantize just-in-time) and post (stats of the
            produced residual tile) hooks so everything pipelines."""
            for j in range(NT):
                if pre is not None:
                    pre(j)
                xqj = xq_j(j)
                ao = g.stat.tile([128, 1], F32, tag="ao", name="ao")
                nc.vector.tensor_mul(ao, al_mat[:, j:j + 1], mscale)
                for c in range(2):
                    ps = ps_pool.tile([128, 384], F32, tag="ps",
                                      name="ps_pt")
                    for k in range(nk):
                        nc.tensor.matmul(
                            ps, xqj[:, k, :],
                            wsb[:, k, c * 384:(c + 1) * 384],
                            start=(k == 0), stop=(k == nk - 1))
                    nc.vector.scalar_tensor_tensor(
                        out_tiles[j][:, c * 384:(c + 1) * 384], ps, ao,
                        resid_tiles[j][:, c * 384:(c + 1) * 384],
                        OP.mult, OP.add)
                if post is not None:
                    post(j)

        # ------------------------------------------------------------------
        # scoped pools
        # ------------------------------------------------------------------
        es_wsa = ExitStack()
        es_wca = ExitStack()
        es_x = ExitStack()
        es_x2 = ExitStack()
        es_sa = ExitStack()
        es_cond = ExitStack()
        es_ffnw = ExitStack()
        es_saq = ExitStack()

        resid3 = ctx.enter_context(tc.tile_pool(name="resid3", bufs=1))
        x3_all = resid3.tile([128, NT, D], F32, name="x3_all")
        x3 = [x3_all[:, j, :] for j in range(NT)]
        x2pool = es_x2.enter_context(tc.tile_pool(name="x2pool", bufs=1,
                                                  side="right"))
        x2_all = x2pool.tile([128, NT, D], F32, name="x2_all")
        x2 = [x2_all[:, j, :] for j in range(NT)]

        # x first on the sync queue, then its stats/quant compute ops are
        # emitted BEFORE any weight-row DMA lands on a compute-engine
        # queue: HBM bandwidth is saturated during startup, so a weight
        # DMA ahead of the stats ops would stall them ~20us.
        with_wsa = es_wsa.enter_context(tc.tile_pool(name="w_sa", bufs=1))
        with_wca = es_wca.enter_context(tc.tile_pool(name="w_ca", bufs=1,
                                                     side="right"))
        xpool = es_x.enter_context(tc.tile_pool(name="xpool", bufs=1))
        x_all = xpool.tile([128, NT, D], F32, name="x_all")
        for j in range(NT):
            nc.sync.dma_start(x_all[:, j, :], x_in[j * 128:(j + 1) * 128, :])
        x_tiles = [x_all[:, j, :] for j in range(NT)]

        # K/V/Q weight rows immediately behind x on the sync queue: the
        # bandwidth window while the stats run is otherwise free (DMA
        # transposes of the quant tiles only start ~30us in).
        g.w = {}
        g.w['sa_wk'] = load_weight(with_wsa, 'sa_wk', nc.sync)
        g.w['sa_wv'] = load_weight(with_wsa, 'sa_wv', nc.sync)
        g.w['sa_wq'] = load_weight(with_wsa, 'sa_wq', nc.sync)

        sa_act = es_sa.enter_context(tc.tile_pool(name="sa_act", bufs=1))
        sa_xq = es_saq.enter_context(tc.tile_pool(name="sa_xq", bufs=1))

        # ---- SA input quant ----
        x1qT = sa_xq.tile([128, KT, T], BF16, name="x1qT")
        S2x = g.qpool.tile([128, NT], F32, tag="S2x1", name="S2x1")
        amx1 = g.qpool.tile([128, NT], F32, tag="amx1", name="amx1")
        for j in range(NT):
            _rms_stats_tile(g, x_tiles[j], D, S2x, amx1, j)
        al_x, srnd_x = _rms_chain(g, S2x, amx1, D, NT, "x1")
        first_xq = None
        for j in range(NT):
            xq = _quant_tile(g, x_tiles[j], D, srnd_x[:, j:j + 1],
                             x1qT[:, :, j * 128:(j + 1) * 128], g.work,
                             nc.sync)
            if first_xq is None:
                first_xq = xq
        abc_x = _make_abc(g, al_x, NT, T, sa_xq, "x1")

        # HAM warm-up: dense burst reading the first quant tile ramps the
        # PE clock while the remaining quant tiles stream.
        with tc.tile_pool(name="ps_warm0", bufs=1, space="PSUM") as psw:
            wps = psw.tile([128, 512], F32, tag="warm0", name="warm0")
            for _ in range(16):
                nc.tensor.matmul(wps, identb, first_xq[:, 0:512],
                                 start=True, stop=True)

        # ---- K, V projections; fire the four quarter-gathers; then Q ----
        with tc.tile_pool(name="ps_proj", bufs=2, space="PSUM") as psp:
            kf = proj_fm(g.w['sa_wk'], x1qT, msc['sa_wk'], abc_x, DKV, T,
                         sa_xq, "kf", psp)
            for j in range(NT):
                for t in range(KP):
                    dst = cc_in[j][t * 128 * 128:(t + 1) * 128 * 128]
                    nc.sync.dma_start(
                        dst.rearrange("(p t) -> p t", p=128),
                        kf[t][:, j * 128:(j + 1) * 128])
            for j in range(NT):
                ps = psp.tile([128, DKV], F32, tag="psv", name="ps_v")
                for k in range(KT):
                    nc.tensor.matmul(ps, x1qT[:, k, j * 128:(j + 1) * 128],
                                     g.w['sa_wv'][:, k, :], start=(k == 0),
                                     stop=(k == KT - 1))
                av = g.stat.tile([128, 1], F32, tag="av", name="av")
                nc.vector.tensor_mul(av, al_x[:, j:j + 1], msc['sa_wv'])
                vtok = g.work.tile([128, DKV], BF16, tag="vtok",
                                   name="vtok")
                nc.vector.tensor_scalar_mul(vtok, ps, av)
                nc.sync.dma_start(
                    cc_in[j][KSLICE:KSLICE + VSLICE].rearrange(
                        "(p f) -> p f", p=128), vtok)
                nc.gpsimd.collective_compute(
                    "AllGather", OP.bypass, replica_groups=groups,
                    ins=[cc_in[j][:].opt()],
                    outs=[cc_out[j][:, :].opt()])

            # deferred weight prefetch: the gpsimd SWDGE queue is blocked
            # by the gather triggers above until the K/V writes land, so
            # these streams start only once the startup crunch is over.
            for k in ('w_cond', 'ca_wk', 'ca_wv'):
                g.w[k] = load_weight(with_wca, k, nc.gpsimd)
            g.w['sa_wo'] = load_weight(with_wsa, 'sa_wo', nc.gpsimd)

            qpairs = proj_fm(g.w['sa_wq'], x1qT, msc['sa_wq'], abc_x, D, T,
                             sa_act, "qp", psp)
            es_saq.close()

            # ---- CA condition-side work (independent of x; overlaps the
            # gathers).  All its DMAs go on the scalar queue so they can
            # never sit behind a gather-dependent wait. ----
            ca_cond = es_cond.enter_context(tc.tile_pool(name="ca_cond",
                                                         bufs=1,
                                                         side="right"))
            with tc.tile_pool(name="ysc", bufs=1) as ysc:
                y_all = ysc.tile([128, ST, D], F32, name="y_all")
                for j in range(ST):
                    nc.scalar.dma_start(y_all[:, j, :],
                                        y_in[j * 128:(j + 1) * 128, :])
                y_tiles = [y_all[:, j, :] for j in range(ST)]
                yqT = ysc.tile([128, KT, S], BF16, name="yqT")
                S2y = g.qpool.tile([128, ST], F32, tag="S2y", name="S2y")
                amy = g.qpool.tile([128, ST], F32, tag="amy", name="amy")
                for j in range(ST):
                    _rms_stats_tile(g, y_tiles[j], D, S2y, amy, j)
                al_y, srnd_y = _rms_chain(g, S2y, amy, D, ST, "y")
                for j in range(ST):
                    _quant_tile(g, y_tiles[j], D, srnd_y[:, j:j + 1],
                                yqT[:, :, j * 128:(j + 1) * 128], g.work,
                                nc.scalar)
                yc_all = ysc.tile([128, ST, D], F32, name="yc_all")
                yc = [yc_all[:, j, :] for j in range(ST)]
                for j in range(ST):
                    am = g.stat.tile([128, 1], F32, tag="am", name="am")
                    nc.vector.tensor_mul(am, al_y[:, j:j + 1],
                                         msc['w_cond'])
                    for c in range(2):
                        ps = psp.tile([128, 384], F32, tag="psy",
                                      name="ps_yc")
                        for k in range(KT):
                            nc.tensor.matmul(
                                ps, yqT[:, k, j * 128:(j + 1) * 128],
                                g.w['w_cond'][:, k, c * 384:(c + 1) * 384],
                                start=(k == 0), stop=(k == KT - 1))
                        nc.vector.tensor_scalar_mul(
                            yc[j][:, c * 384:(c + 1) * 384], ps, am)

                ycqT = ysc.tile([128, KT, S], BF16, name="ycqT")
                S2c = g.qpool.tile([128, ST], F32, tag="S2c", name="S2c")
                amc = g.qpool.tile([128, ST], F32, tag="amc", name="amc")
                for j in range(ST):
                    _rms_stats_tile(g, yc[j], D, S2c, amc, j)
                al_yc, srnd_yc = _rms_chain(g, S2c, amc, D, ST, "yc")
                for j in range(ST):
                    _quant_tile(g, yc[j], D, srnd_yc[:, j:j + 1],
                                ycqT[:, :, j * 128:(j + 1) * 128], g.work,
                                nc.scalar)
                abc_yc = _make_abc(g, al_yc, ST, S, ysc, "yc")

                ca_kpairs = proj_fm(g.w['ca_wk'], ycqT, msc['ca_wk'],
                                    abc_yc, DKV, S, ca_cond, "ck", psp)
                v_ca = []
                for j in range(ST):
                    ps = psp.tile([128, DKV], F32, tag="psv", name="ps_vc")
                    for k in range(KT):
                        nc.tensor.matmul(
                            ps, ycqT[:, k, j * 128:(j + 1) * 128],
                            g.w['ca_wv'][:, k, :], start=(k == 0),
                            stop=(k == KT - 1))
                    av = g.stat.tile([128, 1], F32, tag="av", name="avc")
                    nc.vector.tensor_mul(av, al_yc[:, j:j + 1],
                                         msc['ca_wv'])
                    va = ca_cond.tile([128, HK, HEAD + 1], BF16,
                                      tag=f"vc{j}", name=f"vc{j}")
                    nc.vector.tensor_scalar_mul(
                        va[:, :, 0:HEAD],
                        ps.rearrange("p (h e) -> p h e", e=HEAD), av)
                    nc.vector.memset(va[:, :, HEAD:HEAD + 1], 1.0)
                    v_ca.append(va)

        # ---- gathered K/V tiles; s-tile index = quarter j * gsz + slot ----
        # (pool opened only now, after ysc closed, so the cond-side scratch
        # and the gathered K/V never coexist in SBUF)
        sa_kv = es_sa.enter_context(tc.tile_pool(name="sa_kv", bufs=1))
        kt_g = []
        for kp in range(KP):
            kt = sa_kv.tile([128, NT * gsz, 128], BF16, tag=f"kT{kp}",
                            name=f"kT{kp}")
            kt_g.append(kt)
        v_aug = []
        for s in range(NT * gsz):
            va = sa_kv.tile([128, HK, HEAD + 1], BF16, tag=f"va{s}",
                            name=f"va{s}")
            nc.vector.memset(va[:, :, HEAD:HEAD + 1], 1.0)
            v_aug.append(va)
        for j in range(NT):
            for kp in range(KP):
                src = cc_out[j][:, kp * 128 * 128:(kp + 1) * 128 * 128]
                nc.sync.dma_start(
                    kt_g[kp][:, j * gsz:(j + 1) * gsz, :],
                    src.rearrange("r (p t) -> p r t", p=128))
            for r in range(gsz):
                s = j * gsz + r
                src = cc_out[j][r, KSLICE:KSLICE + VSLICE]
                nc.sync.dma_start(
                    v_aug[s][:, :, 0:HEAD],
                    src.rearrange("(p h e) -> p h e", p=128, e=HEAD))
        k_views = [[kt_g[kp][:, s, :] for s in range(NT * gsz)]
                   for kp in range(KP)]

        # a second HAM warm-up right before attention
        with tc.tile_pool(name="ps_warm1", bufs=1, space="PSUM") as psw:
            wps = psw.tile([128, 512], F32, tag="warm1", name="warm1")
            for _ in range(12):
                nc.tensor.matmul(wps, identb, qpairs[0][:, 0:512],
                                 start=True, stop=True)

        q_lo = [qt[0:64, :] for qt in qpairs]
        q_hi = [qt[64:128, :] for qt in qpairs]

        a_pool = es_sa.enter_context(tc.tile_pool(name="a_pool", bufs=1))
        a_all = a_pool.tile([128, NT, D], F32, name="a_all")
        a_tok = [a_all[:, j, :] for j in range(NT)]
        s1c = None
        if fused_ln:
            s1c = [a_pool.tile([128, HQ], F32, tag=f"s1c{j}",
                               name=f"s1c{j}") for j in range(NT)]
        batches = [[j * gsz + r for r in range(gsz)] for j in range(NT)]
        with tc.tile_pool(name="awork", bufs=1) as awork, \
             tc.tile_pool(name="ps_s", bufs=2, space="PSUM") as psum_s, \
             tc.tile_pool(name="ps_o", bufs=1, space="PSUM") as psum_o, \
             tc.tile_pool(name="ps_t", bufs=2, space="PSUM") as psum_t:
            _attention(g, batches, k_views, v_aug, q_lo, q_hi, a_tok, s1c,
                       psum_s, psum_o, psum_t, awork, a_pool)

        # ---- LN1 (+fused rms) + quant + wo projection + residual ----
        def wo_block(a_tok, s1cols, gname, bname, woname, resid, out_tiles,
                     aqT, post, uid):
            es_ka = ExitStack()
            g.ka_pool = es_ka.enter_context(
                tc.tile_pool(name=f"ka_{uid}", bufs=1, space="PSUM"))
            S2m = g.qpool.tile([128, NT], F32, tag=f"wS2_{uid}",
                               name=f"wS2_{uid}")
            if fused_ln:
                S1m = g.qpool.tile([128, NT], F32, tag=f"wS1_{uid}",
                                   name=f"wS1_{uid}")
                mx = g.qpool.tile([128, NT], F32, tag=f"wmx_{uid}",
                                  name=f"wmx_{uid}")
                mn = g.qpool.tile([128, NT], F32, tag=f"wmn_{uid}",
                                  name=f"wmn_{uid}")
                for j in range(NT):
                    sq = g.sq_scratch(D)
                    nc.scalar.activation(sq, a_tok[j], ACT.Square,
                                         accum_out=S2m[:, j:j + 1])
                    nc.vector.tensor_reduce(S1m[:, j:j + 1], s1cols[j],
                                            axis=AX.X, op=OP.add)
                    nc.vector.tensor_reduce(mx[:, j:j + 1], a_tok[j],
                                            axis=AX.X, op=OP.max)
                    nc.vector.tensor_reduce(mn[:, j:j + 1], a_tok[j],
                                            axis=AX.X, op=OP.min)
                    g.keepalive(sq[:, 0:512], 2)
                al, cq, dq = _fused_ln_chain(g, S1m, S2m, mx, mn, NT, uid)

                def pre(j):
                    _fused_quant_tile(
                        g, a_tok[j], cq[:, j:j + 1], dq[:, j:j + 1],
                        aqT[:, :, j * 128:(j + 1) * 128], g.work, nc.sync)
            else:
                ln_t = a_tok
                _layernorm(g, a_tok, ln_bc[gname], ln_bc[bname], ln_t, uid)
                amax = g.qpool.tile([128, NT], F32, tag=f"wam_{uid}",
                                    name=f"wam_{uid}")
                for j in range(NT):
                    _rms_stats_tile(g, ln_t[j], D, S2m, amax, j)
                al, srnd = _rms_chain(g, S2m, amax, D, NT, uid)

                def pre(j):
                    _quant_tile(g, ln_t[j], D, srnd[:, j:j + 1],
                                aqT[:, :, j * 128:(j + 1) * 128],
                                g.work, nc.sync)
            with tc.tile_pool(name=f"pswo_{uid}", bufs=3,
                              space="PSUM") as pswo:
                proj_tok_resid(
                    lambda j: aqT[:, :, j * 128:(j + 1) * 128],
                    g.w[woname], al, msc[woname], resid, out_tiles,
                    pswo, pre=pre, post=post)
            g.ka_pool = None
            es_ka.close()

        # x2 quant stats pipeline into the wo projection
        S2x2 = g.qpool.tile([128, NT], F32, tag="S2x2", name="S2x2")
        amx2 = g.qpool.tile([128, NT], F32, tag="amx2", name="amx2")

        def post_x2(j):
            _rms_stats_tile(g, x2[j], D, S2x2, amx2, j)

        a1qT = es_sa.enter_context(
            tc.tile_pool(name="a1qT", bufs=1)).tile(
                [128, KT, T], BF16, name="a1qT")
        wo_block(a_tok, s1c, 'sa_g', 'sa_b', 'sa_wo', x_tiles, x2, a1qT,
                 post_x2, "l1")
        es_sa.close()
        es_x.close()
        es_wsa.close()

        ffn_w = es_ffnw.enter_context(tc.tile_pool(name="ffn_w", bufs=1))

        # ---- CA ----
        with tc.tile_pool(name="ca_act", bufs=1) as ca_act, \
             tc.tile_pool(name="awork2", bufs=1) as awork:
            x2qT = ca_act.tile([128, KT, T], BF16, name="x2qT")
            al_x2, srnd_x2 = _rms_chain(g, S2x2, amx2, D, NT, "x2")
            for j in range(NT):
                _quant_tile(g, x2[j], D, srnd_x2[:, j:j + 1],
                            x2qT[:, :, j * 128:(j + 1) * 128], g.work,
                            nc.sync)
            abc_x2 = _make_abc(g, al_x2, NT, T, ca_act, "x2")
            # weight prefetch emitted only now, so the DMA burst overlaps
            # the q2 projection + CA attention instead of the x2 transposes
            g.w['ca_wq'] = load_weight(with_wca, 'ca_wq', nc.sync)
            g.w['ca_wo'] = load_weight(with_wca, 'ca_wo', nc.gpsimd)
            g.w['w1'] = load_weight(ffn_w, 'w1', nc.gpsimd)
            with tc.tile_pool(name="ps_q2", bufs=3, space="PSUM") as psq:
                q2pairs = proj_fm(g.w['ca_wq'], x2qT, msc['ca_wq'], abc_x2,
                                  D, T, ca_act, "q2", psq)

            q2_lo = [qt[0:64, :] for qt in q2pairs]
            q2_hi = [qt[64:128, :] for qt in q2pairs]
            ck_views = [[ca_kpairs[kp][:, s * 128:(s + 1) * 128]
                         for s in range(ST)] for kp in range(KP)]

            # keep the PE warm into the short CA attention phase
            with tc.tile_pool(name="ps_warm2", bufs=1, space="PSUM") as psw:
                wps = psw.tile([128, 512], F32, tag="warm2", name="warm2")
                for _ in range(10):
                    nc.tensor.matmul(wps, identb, q2pairs[0][:, 0:512],
                                     start=True, stop=True)

            a2_all = ca_act.tile([128, NT, D], F32, name="a2_all")
            a2_tok = [a2_all[:, j, :] for j in range(NT)]
            s2c = None
            if fused_ln:
                s2c = [ca_act.tile([128, HQ], F32, tag=f"s2c{j}",
                                   name=f"s2c{j}") for j in range(NT)]
            with tc.tile_pool(name="ps_s2", bufs=2, space="PSUM") as psum_s, \
                 tc.tile_pool(name="ps_o2", bufs=1, space="PSUM") as psum_o, \
                 tc.tile_pool(name="ps_t2", bufs=2, space="PSUM") as psum_t:
                _attention(g, [list(range(ST))], ck_views, v_ca, q2_lo,
                           q2_hi, a2_tok, s2c, psum_s, psum_o, psum_t,
                           awork, ca_act)

            # x3 quant stats pipeline into the wo2 projection
            S2x3 = g.qpool.tile([128, NT], F32, tag="S2x3", name="S2x3")
            amx3 = g.qpool.tile([128, NT], F32, tag="amx3", name="amx3")

            def post_x3(j):
                _rms_stats_tile(g, x3[j], D, S2x3, amx3, j)

            a2qT = x2qT        # x2qT is dead after the q2 projection
            wo_block(a2_tok, s2c, 'ca_g', 'ca_b', 'ca_wo', x2, x3, a2qT,
                     post_x3, "l2")
        es_cond.close()
        es_wca.close()
        es_x2.close()

        g.w['w2'] = load_weight(ffn_w, 'w2', nc.gpsimd)

        # ---- FFN ----
        with tc.tile_pool(name="ffn_act", bufs=1) as ffn_act, \
             tc.tile_pool(name="ffn_wk", bufs=1) as ffn_wk, \
             tc.tile_pool(name="outp", bufs=2) as outp:
            x3qT = ffn_act.tile([128, KT, T], BF16, name="x3qT")
            al_3, srnd_3 = _rms_chain(g, S2x3, amx3, D, NT, "x3")
            for j in range(NT):
                _quant_tile(g, x3[j], D, srnd_3[:, j:j + 1],
                            x3qT[:, :, j * 128:(j + 1) * 128], g.work,
                            nc.sync)

            # fully per-tile pipeline: w1 -> gelu -> stats -> chain ->
            # quant -> w2 per token tile; h and hqT are double-buffered
            # per-tile tiles instead of full [NT, H4] buffers.
            with tc.tile_pool(name="ps_w1", bufs=4, space="PSUM") as psw1, \
                 tc.tile_pool(name="ps_w2", bufs=3, space="PSUM") as psw2:
                for j in range(NT):
                    a3 = g.stat.tile([128, 1], F32, tag=f"a3_{j}",
                                     name=f"a3_{j}")
                    nc.vector.tensor_mul(a3, al_3[:, j:j + 1], msc['w1'])
                    h_j = ffn_act.tile([128, H4], BF16, tag="h_j",
                                       name="h_j", bufs=2)
                    for c in range(6):
                        ps = psw1.tile([128, 512], F32, tag="ps",
                                       name="ps_h")
                        for k in range(KT):
                            nc.tensor.matmul(
                                ps, x3qT[:, k, j * 128:(j + 1) * 128],
                                g.w['w1'][:, k, c * 512:(c + 1) * 512],
                                start=(k == 0), stop=(k == KT - 1))
                        nc.scalar.activation(
                            h_j[:, c * 512:(c + 1) * 512], ps,
                            ACT.Gelu, bias=0.0, scale=a3)
                    S2h = g.qpool.tile([128, 1], F32, tag=f"S2h{j}",
                                       name=f"S2h{j}")
                    amh = g.qpool.tile([128, 1], F32, tag=f"amh{j}",
                                       name=f"amh{j}")
                    _rms_stats_tile(g, h_j, H4, S2h, amh, 0)
                    al_h, srnd_h = _rms_chain(g, S2h, amh, H4, 1, f"h{j}")
                    hqT = ffn_act.tile([128, KTH, 128], BF16, tag="hqT",
                                       name="hqT", bufs=2)
                    _quant_tile(g, h_j, H4, srnd_h[:, 0:1], hqT, ffn_wk,
                                nc.sync)
                    ah = g.stat.tile([128, 1], F32, tag="ah", name="ah")
                    nc.vector.tensor_mul(ah, al_h[:, 0:1], msc['w2'])
                    xo = outp.tile([128, D], F32, tag="xo", name="xo")
                    for c in range(2):
                        ps = psw2.tile([128, 384], F32, tag="ps",
                                       name="ps_w2")
                        for k in range(KTH):
                            nc.tensor.matmul(
                                ps, hqT[:, k, :],
                                g.w['w2'][:, k, c * 384:(c + 1) * 384],
                                start=(k == 0), stop=(k == KTH - 1))
                        nc.vector.scalar_tensor_tensor(
                            xo[:, c * 384:(c + 1) * 384], ps, ah,
                            x3[j][:, c * 384:(c + 1) * 384], OP.mult,
                            OP.add)
                    nc.sync.dma_start(out_sh[j * 128:(j + 1) * 128, :], xo)
        es_ffnw.close()

    nc.finalize()
    return nc


def _get_program(key):
    if key not in _PROGRAM_CACHE:
        groups, fused = key
        _PROGRAM_CACHE[key] = build_program(
            GROUPS if groups == "full" else [[0]], fused_ln=fused)
    return _PROGRAM_CACHE[key]


LAST_RESULT = None


def _host_quant(w):
    """Exact ternary weight quant (same math as reference _weight_quant)."""
    w = np.asarray(w, np.float32)
    m = np.float32(np.mean(np.abs(w), dtype=np.float32))
    m = np.float32(max(m, np.float32(1e-5)))
    q = np.clip(np.rint(w / m), -1.0, 1.0)
    return q.astype(np.float32), m


def kernel(**inputs):
    """Full-input entry: shard across 8 cores, run, gather."""
    global LAST_RESULT
    x = np.ascontiguousarray(np.asarray(inputs['x'], dtype=np.float32))
    y = np.ascontiguousarray(np.asarray(inputs['y'], dtype=np.float32))

    fused = all(
        np.allclose(np.asarray(inputs[k], np.float32), v, atol=0.0)
        for k, v in (('sa_g', 1.0), ('sa_b', 0.0),
                     ('ca_g', 1.0), ('ca_b', 0.0)))
    nc = _get_program(("full", fused))

    qrows = np.concatenate([np.arange(h * 64, (h + 1) * 64)
                            for h in QPERM])
    combo = np.zeros((1, COMBO_W), np.float32)
    common = {}
    for i, name in enumerate(SCALE_SLOTS):
        q, m = _host_quant(inputs[name])
        if name in ('sa_wq', 'ca_wq'):
            q = q[qrows, :]
            m = m / np.float32(np.sqrt(np.float32(HEAD)))
        combo[0, i] = m
        common[f"{name}_q"] = np.ascontiguousarray(
            q.T.astype(ml_dtypes.bfloat16))
    for i, name in enumerate(('sa_g', 'sa_b', 'ca_g', 'ca_b')):
        combo[0, NSLOT + i * D:NSLOT + (i + 1) * D] = np.asarray(
            inputs[name], np.float32)
    common['combo'] = combo

    in_maps = []
    for c in range(NCORES):
        b, seg = c // GSZ, c % GSZ
        m = dict(common)
        m['x_sh'] = np.ascontiguousarray(x[b, seg * T:(seg + 1) * T, :])
        m['y_b'] = np.ascontiguousarray(y[b])
        in_maps.append(m)
    res = run_bass_kernel_spmd(nc, in_maps, core_ids=list(range(NCORES)))
    LAST_RESULT = res
    out = np.empty((B, N, D), np.float32)
    for c in range(NCORES):
        b, seg = c // GSZ, c % GSZ
        out[b, seg * T:(seg + 1) * T, :] = res.results[c]['out_sh']
    return out


# revision 47
# speedup vs baseline: 1.0820x; 1.0820x over previous
"""Trainium2 Bass kernel for nn_DecoderBlock (BitNet-style decoder block with
self-attention, cross-attention and BitFeedForward), data-parallel over
(batch x sequence) tokens across 8 NeuronCores.

Sharding: 4096 tokens (B=2 x N=2048) split into 8 shards of 512 tokens.
Cores 0-3 hold batch 0, cores 4-7 batch 1.  Self-attention K/V are
computed on local tokens and exchanged within each 4-core batch group by
FOUR pipelined AllGathers (one per local 128-token tile), and attention
consumes the gathered key tiles in four availability batches, carrying the
softmax numerator and denominator across batches in per-head SBUF
accumulators.  The first quarter lands while the input projections are
still finishing, so the exchange is almost entirely off the critical path.

Weights are ternary-quantized on the host (exact same math as the
reference's _weight_quant) and shipped as bf16 {-1,0,1} in transposed
[in, out] layout, plus one packed row of fp32 scales/LN params.
Activations are fake-quantized on device; integer-valued operands are
exact in bf16, so the bf16 matmul path is exact for the quantized matmuls
(fp32 PSUM accumulation).

Quant statistics run on the Scalar engine (Square with accum_out gives
sum(x^2) per token in one pass) plus one DVE absmax reduce, so the Vector
engine stays off the critical path.  When the LayerNorm params are g=1,b=0
(true for this problem; checked on the host), LayerNorm + the following
BitLinear RMSNorm collapse into one affine normalize:
    rmsnorm(LN(x)) = (x - mean) * rsqrt(var*(1+1e-6) + 1e-11)
with absmax(x - mean) = max(max(x)-mean, mean-min(x)); the LN apply pass
and its stats pass disappear.  The per-head attention-output normalize ops
carry accum_out, so sum(x) per token is free.

Attention: q heads are host-permuted into pairs (0,2),(1,3),(4,6)... so a
q-pair shares one K tile pair; the two 64-contraction score matmuls run
CONCURRENTLY in the PE array as row-tiles (base partitions 0 and 64),
writing one 2-bank PSUM pair that a single Exp activation consumes.
Softmax denominators come free via a ones-column appended to V.
"""

import numpy as np
import ml_dtypes
from contextlib import ExitStack

import concourse.bacc as bacc
import concourse.mybir as mybir
import concourse.tile as tile
from concourse.bass_utils import run_bass_kernel_spmd
from concourse.masks import make_identity

F32 = mybir.dt.float32
BF16 = mybir.dt.bfloat16
AX = mybir.AxisListType
OP = mybir.AluOpType
ACT = mybir.ActivationFunctionType

# model dims
B, N, S, D = 2, 2048, 256, 768
HQ, HK, HEAD = 12, 6, 64
DKV = HEAD * HK          # 384
H4 = 4 * D               # 3072
NCORES = 8
GROUPS = [[0, 1, 2, 3], [4, 5, 6, 7]]
GSZ = 4                  # cores per batch group
T = (B * N) // NCORES    # 512 tokens per core
NT = T // 128            # 4 token tiles per core
ST = S // 128            # 2 condition token tiles
KT = D // 128            # 6 feature tiles of D
KTH = H4 // 128          # 24 feature tiles of 4D
KP = DKV // 128          # 3 kv-head-pair tiles

# q heads permuted so psum pair tile mt holds (QPERM[2mt], QPERM[2mt+1]),
# and both heads of a pair read the same gathered K pair tile.
QPERM = [0, 2, 1, 3, 4, 6, 5, 7, 8, 10, 9, 11]

# (out_features, in_features); device gets ternary bf16 f"{name}_q" [I, O].
WSPECS = {
    'sa_wq': (D, D), 'sa_wk': (DKV, D), 'sa_wv': (DKV, D), 'sa_wo': (D, D),
    'ca_wq': (D, D), 'ca_wk': (DKV, D), 'ca_wv': (DKV, D), 'ca_wo': (D, D),
    'w_cond': (D, D), 'w1': (H4, D), 'w2': (D, H4),
}
SCALE_SLOTS = list(WSPECS)          # order of m scales in the combo row
NSLOT = 16                          # padded scale slots
COMBO_W = NSLOT + 4 * D             # + sa_g, sa_b, ca_g, ca_b

_PROGRAM_CACHE = {}

MAGIC = 12582912.0   # 1.5 * 2^23: fp32 add/sub forces round-half-even to int

# exchange-quarter sizes (elements, bf16)
KSLICE = KP * 128 * 128   # K columns for one 128-token tile, all kp rows
VSLICE = 128 * DKV        # V for one 128-token tile
QSZ = KSLICE + VSLICE


class Ctx:
    pass


# ---------------------------------------------------------------------------
# quant statistics + per-token scale chains
# ---------------------------------------------------------------------------

def _rms_stats_tile(g, X, F, S2, amax, j):
    """Per-tile stats: S2[:, j] = sum(X^2) (Scalar engine Square with
    accum_out), amax[:, j] = max|X| (one DVE reduce).
    (tensor_tensor_reduce would do the square on the DVE, but it crashes
    this runtime -- verified with a minimal repro.)"""
    nc = g.nc
    sq = g.sq_scratch(F)
    nc.scalar.activation(sq, X, ACT.Square, accum_out=S2[:, j:j + 1])
    nc.vector.tensor_reduce(amax[:, j:j + 1], X, axis=AX.X, op=OP.max,
                            apply_absolute_value=True)


def _rms_chain(g, S2, amax, F, nj, uid):
    """al = absmax_n/127 (dequant alpha), srnd = 127*r/absmax_n where
    r = rsqrt(sum(x^2)/F + 1e-6), absmax_n = clip(absmax*r, 1e-5)."""
    nc, qpool = g.nc, g.qpool
    sd = qpool.tile([128, nj], F32, tag=f"qs_{uid}", name=f"qs_{uid}")
    nc.scalar.activation(sd, S2, ACT.Sqrt, bias=g.eps6, scale=1.0 / F)
    r = qpool.tile([128, nj], F32, tag=f"qr_{uid}", name=f"qr_{uid}")
    nc.vector.reciprocal(r, sd)
    amn = qpool.tile([128, nj], F32, tag=f"qm_{uid}", name=f"qm_{uid}")
    nc.vector.tensor_mul(amn, amax, r)
    nc.vector.tensor_scalar_max(amn, amn, 1e-5)
    al = qpool.tile([128, nj], F32, tag=f"al_{uid}", name=f"al_{uid}")
    nc.vector.tensor_scalar_mul(al, amn, 1.0 / 127.0)
    ra = qpool.tile([128, nj], F32, tag=f"qi_{uid}", name=f"qi_{uid}")
    nc.vector.reciprocal(ra, amn)
    srnd = qpool.tile([128, nj], F32, tag=f"qn_{uid}", name=f"qn_{uid}")
    nc.vector.tensor_mul(srnd, ra, r)
    nc.vector.tensor_scalar_mul(srnd, srnd, 127.0)
    return al, srnd


def _quant_tile(g, X, F, srnd_col, dst3, wk, dma_eng, pe_pool=None):
    """Quantize one token tile: round(x*srnd) via the fp32 magic-number
    trick (DVE mul+add, Act sub) -- integer-exact in bf16; then the
    feature-major transpose into dst3 [128, F//128, 128].  With pe_pool
    the transpose runs on the PE + Scalar copy instead of the XBAR DMA
    path (which is blocked while any collective -- including the
    framework's start-of-model barrier -- is in flight)."""
    nc = g.nc
    tmp = wk.tile([128, F], F32, tag=f"qt_{F}", name="qt",
                  bufs=(2 if F <= 1024 else 1))
    nc.vector.tensor_scalar(tmp, X, srnd_col, MAGIC, OP.mult, OP.add)
    xq = wk.tile([128, F], BF16, tag=f"xq_{F}", name="xq", bufs=2)
    nc.scalar.activation(xq, tmp, ACT.Copy, bias=-MAGIC)
    if pe_pool is not None:
        for k in range(F // 128):
            ps_t = pe_pool.tile([128, 128], BF16, tag="qpe", name="qpe")
            nc.tensor.transpose(ps_t, xq[:, k * 128:(k + 1) * 128],
                                g.identb)
            if dma_eng is nc.vector:
                nc.vector.tensor_copy(dst3[:, k, :], ps_t)
            else:
                nc.scalar.copy(dst3[:, k, :], ps_t)
    else:
        dma_eng.dma_start(dst3, xq, transpose=True)
    return xq


def _fused_ln_chain(g, S1, S2, mx, mn, nj, uid):
    """g=1,b=0 fast path: rmsnorm(LN(a)) == (a - m) * R with m = S1/D,
    var = S2/D - m^2, R = rsqrt(var*(1+1e-6) + 1e-11).
    absmax = max(mx - m, m - mn) * R.  Quantized int = (a*cq - dq) - MAGIC
    with cq = R*127/clip(absmax,1e-5), dq = m*cq - MAGIC.
    Returns (al, cq, dq); al is the dequant alpha."""
    nc, qpool = g.nc, g.qpool

    def t(nm):
        return qpool.tile([128, nj], F32, tag=f"{nm}_{uid}",
                          name=f"{nm}_{uid}")
    m = t("lm")
    nc.vector.tensor_scalar_mul(m, S1, 1.0 / D)
    t1 = t("lt")
    nc.vector.tensor_scalar_mul(t1, S2, 1.0 / D)
    msq = t("lq")
    nc.vector.tensor_mul(msq, m, m)
    var = t("lv")
    nc.vector.tensor_sub(var, t1, msq)
    dd = t("ld")
    nc.vector.tensor_scalar(dd, var, 1.0 + 1e-6, 1e-11, OP.mult, OP.add)
    sd = t("ls")
    nc.scalar.activation(sd, dd, ACT.Sqrt)
    R = t("lr")
    nc.vector.reciprocal(R, sd)
    t3 = t("l3")
    nc.vector.tensor_sub(t3, mx, m)
    t4 = t("l4")
    nc.vector.tensor_sub(t4, m, mn)
    am = t("la")
    nc.vector.tensor_tensor(am, t3, t4, op=OP.max)
    amn = t("ln")
    nc.vector.tensor_mul(amn, am, R)
    nc.vector.tensor_scalar_max(amn, amn, 1e-5)
    al = t("al")
    nc.vector.tensor_scalar_mul(al, amn, 1.0 / 127.0)
    ra = t("li")
    nc.vector.reciprocal(ra, amn)
    cq = t("lc")
    nc.vector.tensor_mul(cq, ra, R)
    nc.vector.tensor_scalar_mul(cq, cq, 127.0)
    dq = t("lz")
    nc.vector.tensor_mul(dq, m, cq)
    nc.vector.tensor_scalar(dq, dq, MAGIC, None, OP.subtract)
    return al, cq, dq


def _fused_quant_tile(g, X, cq_col, dq_col, dst3, wk, dma_eng):
    """Quantize one fused-LN tile: (X*cq - dq) - MAGIC, then transpose."""
    nc = g.nc
    tmp = wk.tile([128, D], F32, tag="qt_768", name="qt", bufs=2)
    nc.vector.tensor_scalar(tmp, X, cq_col, dq_col, OP.mult, OP.subtract)
    xq = wk.tile([128, D], BF16, tag="xq_768", name="xq", bufs=2)
    nc.scalar.activation(xq, tmp, ACT.Copy, bias=-MAGIC)
    dma_eng.dma_start(dst3, xq, transpose=True)
    return xq


def _layernorm(g, a_tiles, g_bc, b_bc, out_tiles, uid):
    """General-g/b LayerNorm (fallback path)."""
    nc, qpool = g.nc, g.qpool
    nj = len(a_tiles)
    s1 = qpool.tile([128, nj], F32, tag=f"ls1_{uid}", name=f"ls1_{uid}")
    s2 = qpool.tile([128, nj], F32, tag=f"ls2_{uid}", name=f"ls2_{uid}")
    for j, A in enumerate(a_tiles):
        sq = g.sq_scratch(D)
        nc.scalar.activation(sq, A, ACT.Square, accum_out=s2[:, j:j + 1])
        nc.vector.tensor_reduce(s1[:, j:j + 1], A, axis=AX.X, op=OP.add)
    m = qpool.tile([128, nj], F32, tag=f"lmu_{uid}", name=f"lmu_{uid}")
    nc.vector.tensor_scalar_mul(m, s1, 1.0 / D)
    t1 = qpool.tile([128, nj], F32, tag=f"lt1_{uid}", name=f"lt1_{uid}")
    nc.vector.tensor_scalar_mul(t1, s2, 1.0 / D)
    msq = qpool.tile([128, nj], F32, tag=f"lms_{uid}", name=f"lms_{uid}")
    nc.vector.tensor_mul(msq, m, m)
    var = qpool.tile([128, nj], F32, tag=f"lva_{uid}", name=f"lva_{uid}")
    nc.vector.tensor_sub(var, t1, msq)
    sd = qpool.tile([128, nj], F32, tag=f"lsd_{uid}", name=f"lsd_{uid}")
    nc.scalar.activation(sd, var, ACT.Sqrt, bias=g.eps5)
    rs = qpool.tile([128, nj], F32, tag=f"lrs_{uid}", name=f"lrs_{uid}")
    nc.vector.reciprocal(rs, sd)
    for j, A in enumerate(a_tiles):
        X = out_tiles[j]
        nc.vector.tensor_scalar(X, A, m[:, j:j + 1], rs[:, j:j + 1],
                                OP.subtract, OP.mult)
        nc.vector.tensor_mul(X, X, g_bc)
        nc.vector.tensor_add(X, X, b_bc)


def _make_abc(g, al_mat, nj, Ttot, pool, uid):
    """Row-broadcast of per-token alpha: [128, nj] -> [128, Ttot], done
    entirely on the PE (transpose, then a rank-1 ones matmul per 128-token
    block) so it never queues behind gpsimd weight-prefetch DMAs."""
    nc = g.nc
    abc = pool.tile([128, Ttot], F32, tag=f"abc_{uid}", name=f"abc_{uid}")
    with g.tc.tile_pool(name=f"psabc_{uid}", bufs=1, space="PSUM") as pp:
        pst = pp.tile([nj, 128], F32, tag="ps_abc", name="pst")
        nc.tensor.transpose(pst, al_mat, g.ident)
        at = g.stat.tile([nj, 128], F32, tag="at", name="at", bufs=1)
        nc.scalar.copy(at, pst)
        arow = g.stat.tile([1, Ttot], F32, tag="arow", name="arow", bufs=1)
        for j in range(nj):
            nc.sync.dma_start(arow[0:1, j * 128:(j + 1) * 128],
                              at[j:j + 1, :])
        psb = pp.tile([128, Ttot], F32, tag="ps_abc2", name="psb")
        nc.tensor.matmul(psb, g.ones1, arow[0:1, :], start=True, stop=True)
        nc.vector.tensor_copy(abc, psb)
    return abc


# ---------------------------------------------------------------------------
# attention
# ---------------------------------------------------------------------------

def _attn_norm(g, h, o_sb, a_out, s1cols, psum_t):
    """Per-head transpose + softmax normalize; accum_out gives the
    per-token feature sum of the normalized head chunk for free."""
    nc = g.nc
    for j in range(NT):
        ps_t = psum_t.tile([128, 65], F32, tag="pst", name="ps_t")
        nc.tensor.transpose(ps_t, o_sb[:, j * 128:(j + 1) * 128],
                            g.ident[0:65, 0:65])
        rec = g.stat.tile([128, 1], F32, tag="rec", name="rec")
        nc.vector.reciprocal(rec, ps_t[:, 64:65])
        acc = s1cols[j][:, h:h + 1] if s1cols is not None else None
        nc.vector.tensor_scalar(a_out[j][:, h * 64:(h + 1) * 64],
                                ps_t[:, 0:64], rec, 0.0, OP.mult, OP.add,
                                accum_out=acc)


def _attention(g, batches, k_views, v_views, q_lo, q_hi, a_out, s1cols,
               psum_s, psum_o, psum_t, awork, acc_pool):
    """Batched paired GQA attention.  batches: list of lists of s-tile
    indices in availability order.  The first batch seeds per-head SBUF
    accumulators, middle batches add into them, the last merges and emits
    transposes + normalize.  Single-batch callers skip the accumulators.

    k_views[kp][s]: [128, 128] bf16 (k-heads 2kp/2kp+1 row-tiled);
    v_views[s]: [128, HK, HEAD+1] bf16 (ones column -> denominator)."""
    nc = g.nc
    nb = len(batches)
    accA = accB = None
    if nb > 1:
        accA = acc_pool.tile([65, HQ // 2, 512], F32, name="accA")
        accB = acc_pool.tile([65, HQ // 2, 512], F32, name="accB")
    for b, batch in enumerate(batches):
        first, last = b == 0, b == nb - 1
        for qp in range(HQ // 2):
            hA, hB = QPERM[2 * qp], QPERM[2 * qp + 1]
            khA, khB = hA // 2, hB // 2
            kp = khA // 2
            ps_oA = psum_o.tile([65, 512], F32, tag="pvA", name="pvA")
            ps_oB = psum_o.tile([65, 512], F32, tag="pvB", name="pvB")
            for i, s in enumerate(batch):
                ps_pair = psum_s.tile([128, 1024], F32, tag="pss",
                                      name="pss")
                ps_A, ps_B = ps_pair[:, 0:512], ps_pair[:, 512:1024]
                nc.tensor.matmul(ps_A, k_views[kp][s][0:64, :], q_lo[qp],
                                 start=True, stop=True)
                nc.tensor.matmul(ps_B, k_views[kp][s][64:128, :], q_hi[qp],
                                 start=True, stop=True)
                pT = awork.tile([128, 1024], BF16, tag="pT", name="pT",
                                bufs=2)
                nc.scalar.activation(pT, ps_pair, ACT.Exp)
                nc.tensor.matmul(ps_oA, v_views[s][:, khA, :],
                                 pT[:, 0:512], start=(i == 0),
                                 stop=(i == len(batch) - 1))
                nc.tensor.matmul(ps_oB, v_views[s][:, khB, :],
                                 pT[:, 512:1024], start=(i == 0),
                                 stop=(i == len(batch) - 1))
            if nb == 1:
                for h, ps_o in ((hA, ps_oA), (hB, ps_oB)):
                    o_sb = awork.tile([65, 512], F32, tag="osb",
                                      name="osb", bufs=2)
                    nc.vector.tensor_copy(o_sb, ps_o)
                    _attn_norm(g, h, o_sb, a_out, s1cols, psum_t)
            elif first:
                nc.vector.tensor_copy(accA[:, qp, :], ps_oA)
                nc.vector.tensor_copy(accB[:, qp, :], ps_oB)
            elif not last:
                nc.vector.tensor_add(accA[:, qp, :], accA[:, qp, :], ps_oA)
                nc.vector.tensor_add(accB[:, qp, :], accB[:, qp, :], ps_oB)
            else:
                for h, ps_o, acc in ((hA, ps_oA, accA), (hB, ps_oB, accB)):
                    o_sb = awork.tile([65, 512], F32, tag="osb",
                                      name="osb", bufs=2)
                    nc.vector.tensor_add(o_sb, acc[:, qp, :], ps_o)
                    _attn_norm(g, h, o_sb, a_out, s1cols, psum_t)


# ---------------------------------------------------------------------------
# program builder
# ---------------------------------------------------------------------------

def build_program(groups=None, fused_ln=True):
    if groups is None:
        groups = GROUPS
    gsz = len(groups[0])
    nc = bacc.Bacc()

    x_in = nc.declare_dram_parameter("x_sh", [T, D], F32, isOutput=False)
    y_in = nc.declare_dram_parameter("y_b", [S, D], F32, isOutput=False)
    wt_in = {}
    for name, (O, I) in WSPECS.items():
        wt_in[name] = nc.declare_dram_parameter(f"{name}_q", [I, O], BF16,
                                                isOutput=False)
    combo_in = nc.declare_dram_parameter("combo", [1, COMBO_W], F32,
                                         isOutput=False)
    out_sh = nc.declare_dram_parameter("out_sh", [T, D], F32, isOutput=True)

    g = Ctx()
    g.nc = nc

    with tile.TileContext(nc) as tc, ExitStack() as ctx:
        g.tc = tc
        g.const = ctx.enter_context(tc.tile_pool(name="const", bufs=1))
        g.stat = ctx.enter_context(tc.tile_pool(name="stat", bufs=4))
        g.work = ctx.enter_context(tc.tile_pool(name="work", bufs=2))
        g.qpool = ctx.enter_context(tc.tile_pool(name="qpool", bufs=1))
        sqpool = ctx.enter_context(tc.tile_pool(name="sqpool", bufs=1))
        dram = ctx.enter_context(tc.tile_pool(name="dram", bufs=1,
                                              space="DRAM"))

        def sq_scratch(F):
            return sqpool.tile([128, F], BF16, tag=f"sq_{F}", name="sq")
        g.sq_scratch = sq_scratch

        # four quarter-exchanges, one per local 128-token tile
        cc_in = [dram.tile([QSZ], BF16, name=f"cc_in{i}") for i in range(NT)]
        cc_out = [dram.tile([gsz, QSZ], BF16, name=f"cc_out{i}")
                  for i in range(NT)]

        g.eps6 = g.const.tile([128, 1], F32, name="eps6")
        nc.vector.memset(g.eps6, 1e-6)
        g.eps5 = g.const.tile([128, 1], F32, name="eps5")
        nc.vector.memset(g.eps5, 1e-5)
        g.ident = g.const.tile([128, 128], F32, name="ident")
        make_identity(nc, g.ident)
        identb = g.const.tile([128, 128], BF16, name="identb")
        nc.vector.tensor_copy(identb, g.ident)
        g.identb = identb
        g.ones1 = g.const.tile([1, 128], F32, name="ones1")
        nc.vector.memset(g.ones1, 1.0)

        # one DMA + partition broadcasts for the scales (+ LN params only
        # in the general-g/b fallback path)
        cbw = NSLOT if fused_ln else COMBO_W
        cb = g.const.tile([128, cbw], F32, name="cb")
        with tc.tile_pool(name="crowp", bufs=1) as crowp:
            crow = crowp.tile([1, COMBO_W], F32, name="crow")
            nc.scalar.dma_start(crow, combo_in[:, :])
            nc.gpsimd.partition_broadcast(cb[:, 0:NSLOT], crow[0:1, 0:NSLOT])
            if not fused_ln:
                for i in range(4):
                    sl = slice(NSLOT + i * D, NSLOT + (i + 1) * D)
                    nc.gpsimd.partition_broadcast(cb[:, sl], crow[0:1, sl])
        msc = {name: cb[:, i:i + 1] for i, name in enumerate(SCALE_SLOTS)}
        ln_bc = ({} if fused_ln else
                 {name: cb[:, NSLOT + i * D:NSLOT + (i + 1) * D]
                  for i, name in enumerate(('sa_g', 'sa_b',
                                            'ca_g', 'ca_b'))})

        g.ka_pool = None

        def keepalive(ap, n):
            """Dummy matmuls reading `ap` (bf16, <=512 cols) to hold the PE
            p-state up through otherwise PE-idle stretches."""
            if g.ka_pool is None:
                return
            for _ in range(n):
                ps = g.ka_pool.tile([128, 512], F32, tag="ka", name="ka")
                nc.tensor.matmul(ps, identb, ap, start=True, stop=True)
        g.keepalive = keepalive

        def load_weight(pool, name, eng):
            O, I = WSPECS[name]
            rows = I // 128
            wt = pool.tile([128, rows, O], BF16, tag=f"w_{name}",
                           name=f"w_{name}")
            for r in range(rows):
                eng.dma_start(wt[:, r, :],
                              wt_in[name][r * 128:(r + 1) * 128, :])
            return wt

        def proj_fm(wsb, xqT_all, mscale, abc, O, Ttot, pool, tag, ps_pool):
            """feature-major projection: O//128 tiles [128, Ttot] bf16."""
            nk = xqT_all.shape[1]
            outs = []
            for mt in range(O // 128):
                ps = ps_pool.tile([128, Ttot], F32, tag="ps", name="ps_pf")
                for k in range(nk):
                    nc.tensor.matmul(ps, wsb[:, k, mt * 128:(mt + 1) * 128],
                                     xqT_all[:, k, :], start=(k == 0),
                                     stop=(k == nk - 1))
                o = pool.tile([128, Ttot], BF16, tag=f"{tag}{mt}",
                              name=f"{tag}{mt}")
                nc.vector.scalar_tensor_tensor(o, ps, mscale, abc,
                                               OP.mult, OP.mult)
                outs.append(o)
            return outs

        def proj_tok_resid(xq_j, wsb, al_mat, mscale, resid_tiles,
                           out_tiles, ps_pool, nk=KT, pre=None, post=None):
            """token-major projection + dequant + residual add, with
            per-tile pre (quantize just-in-time) and post (stats of the
            produced residual tile) hooks so everything pipelines."""
            for j in range(NT):
                if pre is not None:
                    pre(j)
                xqj = xq_j(j)
                ao = g.stat.tile([128, 1], F32, tag="ao", name="ao")
                nc.vector.tensor_mul(ao, al_mat[:, j:j + 1], mscale)
                for c in range(2):
                    ps = ps_pool.tile([128, 384], F32, tag="ps",
                                      name="ps_pt")
                    for k in range(nk):
                        nc.tensor.matmul(
                            ps, xqj[:, k, :],
                            wsb[:, k, c * 384:(c + 1) * 384],
                            start=(k == 0), stop=(k == nk - 1))
                    nc.vector.scalar_tensor_tensor(
                        out_tiles[j][:, c * 384:(c + 1) * 384], ps, ao,
                        resid_tiles[j][:, c * 384:(c + 1) * 384],
                        OP.mult, OP.add)
                if post is not None:
                    post(j)

        # ------------------------------------------------------------------
        # scoped pools
        # ------------------------------------------------------------------
        es_wsa = ExitStack()
        es_wca = ExitStack()
        es_x = ExitStack()
        es_x2 = ExitStack()
        es_sa = ExitStack()
        es_cond = ExitStack()
        es_ffnw = ExitStack()
        es_saq = ExitStack()

        resid3 = ctx.enter_context(tc.tile_pool(name="resid3", bufs=1))
        x3_all = resid3.tile([128, NT, D], F32, name="x3_all")
        x3 = [x3_all[:, j, :] for j in range(NT)]
        x2pool = es_x2.enter_context(tc.tile_pool(name="x2pool", bufs=1,
                                                  side="right"))
        x2_all = x2pool.tile([128, NT, D], F32, name="x2_all")
        x2 = [x2_all[:, j, :] for j in range(NT)]

        # x first on the sync queue, then its stats/quant compute ops are
        # emitted BEFORE any weight-row DMA lands on a compute-engine
        # queue: HBM bandwidth is saturated during startup, so a weight
        # DMA ahead of the stats ops would stall them ~20us.
        with_wsa = es_wsa.enter_context(tc.tile_pool(name="w_sa", bufs=1))
        with_wca = es_wca.enter_context(tc.tile_pool(name="w_ca", bufs=1,
                                                     side="right"))
        xpool = es_x.enter_context(tc.tile_pool(name="xpool", bufs=1))
        x_all = xpool.tile([128, NT, D], F32, name="x_all")
        for j in range(NT):
            nc.sync.dma_start(x_all[:, j, :], x_in[j * 128:(j + 1) * 128, :])
        x_tiles = [x_all[:, j, :] for j in range(NT)]

        # K/V/Q weight rows immediately behind x on the sync queue: the
        # bandwidth window while the stats run is otherwise free (DMA
        # transposes of the quant tiles only start ~30us in).
        g.w = {}
        g.w['sa_wk'] = load_weight(with_wsa, 'sa_wk', nc.sync)
        g.w['sa_wv'] = load_weight(with_wsa, 'sa_wv', nc.sync)
        g.w['sa_wq'] = load_weight(with_wsa, 'sa_wq', nc.sync)

        sa_act = es_sa.enter_context(tc.tile_pool(name="sa_act", bufs=1))
        sa_xq = es_saq.enter_context(tc.tile_pool(name="sa_xq", bufs=1))

        # ---- SA input quant ----
        x1qT = sa_xq.tile([128, KT, T], BF16, name="x1qT")
        S2x = g.qpool.tile([128, NT], F32, tag="S2x1", name="S2x1")
        amx1 = g.qpool.tile([128, NT], F32, tag="amx1", name="amx1")
        for j in range(NT):
            _rms_stats_tile(g, x_tiles[j], D, S2x, amx1, j)
        al_x, srnd_x = _rms_chain(g, S2x, amx1, D, NT, "x1")
        first_xq = None
        with tc.tile_pool(name="ps_qpe", bufs=4, space="PSUM") as qpe:
            for j in range(NT):
                xq = _quant_tile(g, x_tiles[j], D, srnd_x[:, j:j + 1],
                                 x1qT[:, :, j * 128:(j + 1) * 128], g.work,
                                 nc.sync, pe_pool=qpe)
                if first_xq is None:
                    first_xq = xq
        abc_x = _make_abc(g, al_x, NT, T, sa_xq, "x1")

        # HAM warm-up: dense burst reading the first quant tile ramps the
        # PE clock while the remaining quant tiles stream.
        with tc.tile_pool(name="ps_warm0", bufs=1, space="PSUM") as psw:
            wps = psw.tile([128, 512], F32, tag="warm0", name="warm0")
            for _ in range(16):
                nc.tensor.matmul(wps, identb, first_xq[:, 0:512],
                                 start=True, stop=True)

        # ---- K, V projections; fire the four quarter-gathers; then Q ----
        with tc.tile_pool(name="ps_proj", bufs=2, space="PSUM") as psp:
            kf = proj_fm(g.w['sa_wk'], x1qT, msc['sa_wk'], abc_x, DKV, T,
                         sa_xq, "kf", psp)
            for j in range(NT):
                for t in range(KP):
                    dst = cc_in[j][t * 128 * 128:(t + 1) * 128 * 128]
                    nc.sync.dma_start(
                        dst.rearrange("(p t) -> p t", p=128),
                        kf[t][:, j * 128:(j + 1) * 128])
            for j in range(NT):
                ps = psp.tile([128, DKV], F32, tag="psv", name="ps_v")
                for k in range(KT):
                    nc.tensor.matmul(ps, x1qT[:, k, j * 128:(j + 1) * 128],
                                     g.w['sa_wv'][:, k, :], start=(k == 0),
                                     stop=(k == KT - 1))
                av = g.stat.tile([128, 1], F32, tag="av", name="av")
                nc.vector.tensor_mul(av, al_x[:, j:j + 1], msc['sa_wv'])
                vtok = g.work.tile([128, DKV], BF16, tag="vtok",
                                   name="vtok")
                nc.vector.tensor_scalar_mul(vtok, ps, av)
                nc.sync.dma_start(
                    cc_in[j][KSLICE:KSLICE + VSLICE].rearrange(
                        "(p f) -> p f", p=128), vtok)
                nc.gpsimd.collective_compute(
                    "AllGather", OP.bypass, replica_groups=groups,
                    ins=[cc_in[j][:].opt()],
                    outs=[cc_out[j][:, :].opt()])

            # deferred weight prefetch: the gpsimd SWDGE queue is blocked
            # by the gather triggers above until the K/V writes land, so
            # these streams start only once the startup crunch is over.
            for k in ('w_cond', 'ca_wk', 'ca_wv'):
                g.w[k] = load_weight(with_wca, k, nc.gpsimd)
            g.w['sa_wo'] = load_weight(with_wsa, 'sa_wo', nc.gpsimd)

            qpairs = proj_fm(g.w['sa_wq'], x1qT, msc['sa_wq'], abc_x, D, T,
                             sa_act, "qp", psp)
            es_saq.close()

            # ---- CA condition-side work (independent of x; overlaps the
            # gathers).  All its DMAs go on the scalar queue so they can
            # never sit behind a gather-dependent wait. ----
            ca_cond = es_cond.enter_context(tc.tile_pool(name="ca_cond",
                                                         bufs=1,
                                                         side="right"))
            with tc.tile_pool(name="ysc", bufs=1) as ysc:
                y_all = ysc.tile([128, ST, D], F32, name="y_all")
                for j in range(ST):
                    nc.scalar.dma_start(y_all[:, j, :],
                                        y_in[j * 128:(j + 1) * 128, :])
                y_tiles = [y_all[:, j, :] for j in range(ST)]
                yqT = ysc.tile([128, KT, S], BF16, name="yqT")
                S2y = g.qpool.tile([128, ST], F32, tag="S2y", name="S2y")
                amy = g.qpool.tile([128, ST], F32, tag="amy", name="amy")
                for j in range(ST):
                    _rms_stats_tile(g, y_tiles[j], D, S2y, amy, j)
                al_y, srnd_y = _rms_chain(g, S2y, amy, D, ST, "y")
                with tc.tile_pool(name="ps_qpy", bufs=2,
                                  space="PSUM") as qpy:
                    for j in range(ST):
                        _quant_tile(g, y_tiles[j], D, srnd_y[:, j:j + 1],
                                    yqT[:, :, j * 128:(j + 1) * 128],
                                    g.work, nc.vector, pe_pool=qpy)
                yc_all = ysc.tile([128, ST, D], F32, name="yc_all")
                yc = [yc_all[:, j, :] for j in range(ST)]
                for j in range(ST):
                    am = g.stat.tile([128, 1], F32, tag="am", name="am")
                    nc.vector.tensor_mul(am, al_y[:, j:j + 1],
                                         msc['w_cond'])
                    for c in range(2):
                        ps = psp.tile([128, 384], F32, tag="psy",
                                      name="ps_yc")
                        for k in range(KT):
                            nc.tensor.matmul(
                                ps, yqT[:, k, j * 128:(j + 1) * 128],
                                g.w['w_cond'][:, k, c * 384:(c + 1) * 384],
                                start=(k == 0), stop=(k == KT - 1))
                        nc.vector.tensor_scalar_mul(
                            yc[j][:, c * 384:(c + 1) * 384], ps, am)

                ycqT = ysc.tile([128, KT, S], BF16, name="ycqT")
                S2c = g.qpool.tile([128, ST], F32, tag="S2c", name="S2c")
                amc = g.qpool.tile([128, ST], F32, tag="amc", name="amc")
                for j in range(ST):
                    _rms_stats_tile(g, yc[j], D, S2c, amc, j)
                al_yc, srnd_yc = _rms_chain(g, S2c, amc, D, ST, "yc")
                with tc.tile_pool(name="ps_qpc", bufs=2,
                                  space="PSUM") as qpc:
                    for j in range(ST):
                        _quant_tile(g, yc[j], D, srnd_yc[:, j:j + 1],
                                    ycqT[:, :, j * 128:(j + 1) * 128],
                                    g.work, nc.vector, pe_pool=qpc)
                abc_yc = _make_abc(g, al_yc, ST, S, ysc, "yc")

                ca_kpairs = proj_fm(g.w['ca_wk'], ycqT, msc['ca_wk'],
                                    abc_yc, DKV, S, ca_cond, "ck", psp)
                v_ca = []
                for j in range(ST):
                    ps = psp.tile([128, DKV], F32, tag="psv", name="ps_vc")
                    for k in range(KT):
                        nc.tensor.matmul(
                            ps, ycqT[:, k, j * 128:(j + 1) * 128],
                            g.w['ca_wv'][:, k, :], start=(k == 0),
                            stop=(k == KT - 1))
                    av = g.stat.tile([128, 1], F32, tag="av", name="avc")
                    nc.vector.tensor_mul(av, al_yc[:, j:j + 1],
                                         msc['ca_wv'])
                    va = ca_cond.tile([128, HK, HEAD + 1], BF16,
                                      tag=f"vc{j}", name=f"vc{j}")
                    nc.vector.tensor_scalar_mul(
                        va[:, :, 0:HEAD],
                        ps.rearrange("p (h e) -> p h e", e=HEAD), av)
                    nc.vector.memset(va[:, :, HEAD:HEAD + 1], 1.0)
                    v_ca.append(va)

        # ---- gathered K/V tiles; s-tile index = quarter j * gsz + slot ----
        # (pool opened only now, after ysc closed, so the cond-side scratch
        # and the gathered K/V never coexist in SBUF)
        sa_kv = es_sa.enter_context(tc.tile_pool(name="sa_kv", bufs=1))
        kt_g = []
        for kp in range(KP):
            kt = sa_kv.tile([128, NT * gsz, 128], BF16, tag=f"kT{kp}",
                            name=f"kT{kp}")
            kt_g.append(kt)
        v_aug = []
        for s in range(NT * gsz):
            va = sa_kv.tile([128, HK, HEAD + 1], BF16, tag=f"va{s}",
                            name=f"va{s}")
            nc.vector.memset(va[:, :, HEAD:HEAD + 1], 1.0)
            v_aug.append(va)
        for j in range(NT):
            for kp in range(KP):
                src = cc_out[j][:, kp * 128 * 128:(kp + 1) * 128 * 128]
                nc.sync.dma_start(
                    kt_g[kp][:, j * gsz:(j + 1) * gsz, :],
                    src.rearrange("r (p t) -> p r t", p=128))
            for r in range(gsz):
                s = j * gsz + r
                src = cc_out[j][r, KSLICE:KSLICE + VSLICE]
                nc.sync.dma_start(
                    v_aug[s][:, :, 0:HEAD],
                    src.rearrange("(p h e) -> p h e", p=128, e=HEAD))
        k_views = [[kt_g[kp][:, s, :] for s in range(NT * gsz)]
                   for kp in range(KP)]

        # a second HAM warm-up right before attention
        with tc.tile_pool(name="ps_warm1", bufs=1, space="PSUM") as psw:
            wps = psw.tile([128, 512], F32, tag="warm1", name="warm1")
            for _ in range(12):
                nc.tensor.matmul(wps, identb, qpairs[0][:, 0:512],
                                 start=True, stop=True)

        q_lo = [qt[0:64, :] for qt in qpairs]
        q_hi = [qt[64:128, :] for qt in qpairs]

        a_pool = es_sa.enter_context(tc.tile_pool(name="a_pool", bufs=1))
        a_all = a_pool.tile([128, NT, D], F32, name="a_all")
        a_tok = [a_all[:, j, :] for j in range(NT)]
        s1c = None
        if fused_ln:
            s1c = [a_pool.tile([128, HQ], F32, tag=f"s1c{j}",
                               name=f"s1c{j}") for j in range(NT)]
        batches = [[j * gsz + r for r in range(gsz)] for j in range(NT)]
        with tc.tile_pool(name="awork", bufs=1) as awork, \
             tc.tile_pool(name="ps_s", bufs=2, space="PSUM") as psum_s, \
             tc.tile_pool(name="ps_o", bufs=1, space="PSUM") as psum_o, \
             tc.tile_pool(name="ps_t", bufs=2, space="PSUM") as psum_t:
            _attention(g, batches, k_views, v_aug, q_lo, q_hi, a_tok, s1c,
                       psum_s, psum_o, psum_t, awork, a_pool)

        # ---- LN1 (+fused rms) + quant + wo projection + residual ----
        def wo_block(a_tok, s1cols, gname, bname, woname, resid, out_tiles,
                     aqT, post, uid):
            es_ka = ExitStack()
            g.ka_pool = es_ka.enter_context(
                tc.tile_pool(name=f"ka_{uid}", bufs=1, space="PSUM"))
            S2m = g.qpool.tile([128, NT], F32, tag=f"wS2_{uid}",
                               name=f"wS2_{uid}")
            if fused_ln:
                S1m = g.qpool.tile([128, NT], F32, tag=f"wS1_{uid}",
                                   name=f"wS1_{uid}")
                mx = g.qpool.tile([128, NT], F32, tag=f"wmx_{uid}",
                                  name=f"wmx_{uid}")
                mn = g.qpool.tile([128, NT], F32, tag=f"wmn_{uid}",
                                  name=f"wmn_{uid}")
                for j in range(NT):
                    sq = g.sq_scratch(D)
                    nc.scalar.activation(sq, a_tok[j], ACT.Square,
                                         accum_out=S2m[:, j:j + 1])
                    nc.vector.tensor_reduce(S1m[:, j:j + 1], s1cols[j],
                                            axis=AX.X, op=OP.add)
                    nc.vector.tensor_reduce(mx[:, j:j + 1], a_tok[j],
                                            axis=AX.X, op=OP.max)
                    nc.vector.tensor_reduce(mn[:, j:j + 1], a_tok[j],
                                            axis=AX.X, op=OP.min)
                    g.keepalive(sq[:, 0:512], 2)
                al, cq, dq = _fused_ln_chain(g, S1m, S2m, mx, mn, NT, uid)

                def pre(j):
                    _fused_quant_tile(
                        g, a_tok[j], cq[:, j:j + 1], dq[:, j:j + 1],
                        aqT[:, :, j * 128:(j + 1) * 128], g.work, nc.sync)
            else:
                ln_t = a_tok
                _layernorm(g, a_tok, ln_bc[gname], ln_bc[bname], ln_t, uid)
                amax = g.qpool.tile([128, NT], F32, tag=f"wam_{uid}",
                                    name=f"wam_{uid}")
                for j in range(NT):
                    _rms_stats_tile(g, ln_t[j], D, S2m, amax, j)
                al, srnd = _rms_chain(g, S2m, amax, D, NT, uid)

                def pre(j):
                    _quant_tile(g, ln_t[j], D, srnd[:, j:j + 1],
                                aqT[:, :, j * 128:(j + 1) * 128],
                                g.work, nc.sync)
            with tc.tile_pool(name=f"pswo_{uid}", bufs=3,
                              space="PSUM") as pswo:
                proj_tok_resid(
                    lambda j: aqT[:, :, j * 128:(j + 1) * 128],
                    g.w[woname], al, msc[woname], resid, out_tiles,
                    pswo, pre=pre, post=post)
            g.ka_pool = None
            es_ka.close()

        # x2 quant stats pipeline into the wo projection
        S2x2 = g.qpool.tile([128, NT], F32, tag="S2x2", name="S2x2")
        amx2 = g.qpool.tile([128, NT], F32, tag="amx2", name="amx2")

        def post_x2(j):
            _rms_stats_tile(g, x2[j], D, S2x2, amx2, j)

        a1qT = es_sa.enter_context(
            tc.tile_pool(name="a1qT", bufs=1)).tile(
                [128, KT, T], BF16, name="a1qT")
        wo_block(a_tok, s1c, 'sa_g', 'sa_b', 'sa_wo', x_tiles, x2, a1qT,
                 post_x2, "l1")
        es_sa.close()
        es_x.close()
        es_wsa.close()

        ffn_w = es_ffnw.enter_context(tc.tile_pool(name="ffn_w", bufs=1))

        # ---- CA ----
        with tc.tile_pool(name="ca_act", bufs=1) as ca_act, \
             tc.tile_pool(name="awork2", bufs=1) as awork:
            x2qT = ca_act.tile([128, KT, T], BF16, name="x2qT")
            al_x2, srnd_x2 = _rms_chain(g, S2x2, amx2, D, NT, "x2")
            for j in range(NT):
                _quant_tile(g, x2[j], D, srnd_x2[:, j:j + 1],
                            x2qT[:, :, j * 128:(j + 1) * 128], g.work,
                            nc.sync)
            abc_x2 = _make_abc(g, al_x2, NT, T, ca_act, "x2")
            # weight prefetch emitted only now, so the DMA burst overlaps
            # the q2 projection + CA attention instead of the x2 transposes
            g.w['ca_wq'] = load_weight(with_wca, 'ca_wq', nc.sync)
            g.w['ca_wo'] = load_weight(with_wca, 'ca_wo', nc.gpsimd)
            g.w['w1'] = load_weight(ffn_w, 'w1', nc.gpsimd)
            with tc.tile_pool(name="ps_q2", bufs=3, space="PSUM") as psq:
                q2pairs = proj_fm(g.w['ca_wq'], x2qT, msc['ca_wq'], abc_x2,
                                  D, T, ca_act, "q2", psq)

            q2_lo = [qt[0:64, :] for qt in q2pairs]
            q2_hi = [qt[64:128, :] for qt in q2pairs]
            ck_views = [[ca_kpairs[kp][:, s * 128:(s + 1) * 128]
                         for s in range(ST)] for kp in range(KP)]

            # keep the PE warm into the short CA attention phase
            with tc.tile_pool(name="ps_warm2", bufs=1, space="PSUM") as psw:
                wps = psw.tile([128, 512], F32, tag="warm2", name="warm2")
                for _ in range(10):
                    nc.tensor.matmul(wps, identb, q2pairs[0][:, 0:512],
                                     start=True, stop=True)

            a2_all = ca_act.tile([128, NT, D], F32, name="a2_all")
            a2_tok = [a2_all[:, j, :] for j in range(NT)]
            s2c = None
            if fused_ln:
                s2c = [ca_act.tile([128, HQ], F32, tag=f"s2c{j}",
                                   name=f"s2c{j}") for j in range(NT)]
            with tc.tile_pool(name="ps_s2", bufs=2, space="PSUM") as psum_s, \
                 tc.tile_pool(name="ps_o2", bufs=1, space="PSUM") as psum_o, \
                 tc.tile_pool(name="ps_t2", bufs=2, space="PSUM") as psum_t:
                _attention(g, [list(range(ST))], ck_views, v_ca, q2_lo,
                           q2_hi, a2_tok, s2c, psum_s, psum_o, psum_t,
                           awork, ca_act)

            # x3 quant stats pipeline into the wo2 projection
            S2x3 = g.qpool.tile([128, NT], F32, tag="S2x3", name="S2x3")
            amx3 = g.qpool.tile([128, NT], F32, tag="amx3", name="amx3")

            def post_x3(j):
                _rms_stats_tile(g, x3[j], D, S2x3, amx3, j)

            a2qT = x2qT        # x2qT is dead after the q2 projection
            wo_block(a2_tok, s2c, 'ca_g', 'ca_b', 'ca_wo', x2, x3, a2qT,
                     post_x3, "l2")
        es_cond.close()
        es_wca.close()
        es_x2.close()

        g.w['w2'] = load_weight(ffn_w, 'w2', nc.gpsimd)

        # ---- FFN ----
        with tc.tile_pool(name="ffn_act", bufs=1) as ffn_act, \
             tc.tile_pool(name="ffn_wk", bufs=1) as ffn_wk, \
             tc.tile_pool(name="outp", bufs=2) as outp:
            x3qT = ffn_act.tile([128, KT, T], BF16, name="x3qT")
            al_3, srnd_3 = _rms_chain(g, S2x3, amx3, D, NT, "x3")
            for j in range(NT):
                _quant_tile(g, x3[j], D, srnd_3[:, j:j + 1],
                            x3qT[:, :, j * 128:(j + 1) * 128], g.work,
                            nc.sync)

            # per-tile pipeline, SKEWED two tiles: w2(j) is emitted after
            # w1(j+2), so the PE's in-order queue gives tile j's quant
            # chain two w1 blocks (~15us) of latency cover instead of
            # stalling the PE on every tile.
            hq_tiles = [None] * NT
            ah_tiles = [None] * NT

            def w1_block(j):
                a3 = g.stat.tile([128, 1], F32, tag=f"a3_{j}",
                                 name=f"a3_{j}")
                nc.vector.tensor_mul(a3, al_3[:, j:j + 1], msc['w1'])
                h_j = ffn_act.tile([128, H4], BF16, tag="h_j",
                                   name="h_j", bufs=3)
                for c in range(6):
                    ps = psw1.tile([128, 512], F32, tag="ps", name="ps_h")
                    for k in range(KT):
                        nc.tensor.matmul(
                            ps, x3qT[:, k, j * 128:(j + 1) * 128],
                            g.w['w1'][:, k, c * 512:(c + 1) * 512],
                            start=(k == 0), stop=(k == KT - 1))
                    nc.scalar.activation(h_j[:, c * 512:(c + 1) * 512],
                                         ps, ACT.Gelu, bias=0.0, scale=a3)
                S2h = g.qpool.tile([128, 1], F32, tag=f"S2h{j}",
                                   name=f"S2h{j}")
                amh = g.qpool.tile([128, 1], F32, tag=f"amh{j}",
                                   name=f"amh{j}")
                _rms_stats_tile(g, h_j, H4, S2h, amh, 0)
                al_h, srnd_h = _rms_chain(g, S2h, amh, H4, 1, f"h{j}")
                hqT = ffn_act.tile([128, KTH, 128], BF16, tag="hqT",
                                   name="hqT", bufs=3)
                _quant_tile(g, h_j, H4, srnd_h[:, 0:1], hqT, ffn_wk,
                            nc.sync)
                ah = g.stat.tile([128, 1], F32, tag=f"ah{j}",
                                 name=f"ah{j}")
                nc.vector.tensor_mul(ah, al_h[:, 0:1], msc['w2'])
                hq_tiles[j] = hqT
                ah_tiles[j] = ah

            def w2_block(j):
                hqT, ah = hq_tiles[j], ah_tiles[j]
                xo = outp.tile([128, D], F32, tag="xo", name="xo")
                for c in range(2):
                    ps = psw2.tile([128, 384], F32, tag="ps", name="ps_w2")
                    for k in range(KTH):
                        nc.tensor.matmul(
                            ps, hqT[:, k, :],
                            g.w['w2'][:, k, c * 384:(c + 1) * 384],
                            start=(k == 0), stop=(k == KTH - 1))
                    nc.vector.scalar_tensor_tensor(
                        xo[:, c * 384:(c + 1) * 384], ps, ah,
                        x3[j][:, c * 384:(c + 1) * 384], OP.mult, OP.add)
                nc.sync.dma_start(out_sh[j * 128:(j + 1) * 128, :], xo)

            with tc.tile_pool(name="ps_w1", bufs=4, space="PSUM") as psw1, \
                 tc.tile_pool(name="ps_w2", bufs=3, space="PSUM") as psw2:
                for j in range(NT + 2):
                    if j < NT:
                        w1_block(j)
                    if j >= 2:
                        w2_block(j - 2)
        es_ffnw.close()

    nc.finalize()
    return nc


def _get_program(key):
    if key not in _PROGRAM_CACHE:
        groups, fused = key
        _PROGRAM_CACHE[key] = build_program(
            GROUPS if groups == "full" else [[0]], fused_ln=fused)
    return _PROGRAM_CACHE[key]


LAST_RESULT = None


def _host_quant(w):
    """Exact ternary weight quant (same math as reference _weight_quant)."""
    w = np.asarray(w, np.float32)
    m = np.float32(np.mean(np.abs(w), dtype=np.float32))
    m = np.float32(max(m, np.float32(1e-5)))
    q = np.clip(np.rint(w / m), -1.0, 1.0)
    return q.astype(np.float32), m


def kernel(**inputs):
    """Full-input entry: shard across 8 cores, run, gather."""
    global LAST_RESULT
    x = np.ascontiguousarray(np.asarray(inputs['x'], dtype=np.float32))
    y = np.ascontiguousarray(np.asarray(inputs['y'], dtype=np.float32))

    fused = all(
        np.allclose(np.asarray(inputs[k], np.float32), v, atol=0.0)
        for k, v in (('sa_g', 1.0), ('sa_b', 0.0),
                     ('ca_g', 1.0), ('ca_b', 0.0)))
    nc = _get_program(("full", fused))

    qrows = np.concatenate([np.arange(h * 64, (h + 1) * 64)
                            for h in QPERM])
    combo = np.zeros((1, COMBO_W), np.float32)
    common = {}
    for i, name in enumerate(SCALE_SLOTS):
        q, m = _host_quant(inputs[name])
        if name in ('sa_wq', 'ca_wq'):
            q = q[qrows, :]
            m = m / np.float32(np.sqrt(np.float32(HEAD)))
        combo[0, i] = m
        common[f"{name}_q"] = np.ascontiguousarray(
            q.T.astype(ml_dtypes.bfloat16))
    for i, name in enumerate(('sa_g', 'sa_b', 'ca_g', 'ca_b')):
        combo[0, NSLOT + i * D:NSLOT + (i + 1) * D] = np.asarray(
            inputs[name], np.float32)
    common['combo'] = combo

    in_maps = []
    for c in range(NCORES):
        b, seg = c // GSZ, c % GSZ
        m = dict(common)
        m['x_sh'] = np.ascontiguousarray(x[b, seg * T:(seg + 1) * T, :])
        m['y_b'] = np.ascontiguousarray(y[b])
        in_maps.append(m)
    res = run_bass_kernel_spmd(nc, in_maps, core_ids=list(range(NCORES)))
    LAST_RESULT = res
    out = np.empty((B, N, D), np.float32)
    for c in range(NCORES):
        b, seg = c // GSZ, c % GSZ
        out[b, seg * T:(seg + 1) * T, :] = res.results[c]['out_sh']
    return out


# revision 58
# speedup vs baseline: 1.0995x; 1.0162x over previous
"""Trainium2 Bass kernel for nn_DecoderBlock (BitNet-style decoder block with
self-attention, cross-attention and BitFeedForward), data-parallel over
(batch x sequence) tokens across 8 NeuronCores.

Sharding: 4096 tokens (B=2 x N=2048) split into 8 shards of 512 tokens.
Cores 0-3 hold batch 0, cores 4-7 batch 1.  Self-attention K/V are
computed on local tokens and exchanged within each 4-core batch group by
FOUR pipelined AllGathers (one per local 128-token tile), and attention
consumes the gathered key tiles in four availability batches, carrying the
softmax numerator and denominator across batches in per-head SBUF
accumulators.  The first quarter lands while the input projections are
still finishing, so the exchange is almost entirely off the critical path.

Weights are ternary-quantized on the host (exact same math as the
reference's _weight_quant) and shipped as bf16 {-1,0,1} in transposed
[in, out] layout, plus one packed row of fp32 scales/LN params.
Activations are fake-quantized on device; integer-valued operands are
exact in bf16, so the bf16 matmul path is exact for the quantized matmuls
(fp32 PSUM accumulation).

Quant statistics run on the Scalar engine (Square with accum_out gives
sum(x^2) per token in one pass) plus one DVE absmax reduce, so the Vector
engine stays off the critical path.  When the LayerNorm params are g=1,b=0
(true for this problem; checked on the host), LayerNorm + the following
BitLinear RMSNorm collapse into one affine normalize:
    rmsnorm(LN(x)) = (x - mean) * rsqrt(var*(1+1e-6) + 1e-11)
with absmax(x - mean) = max(max(x)-mean, mean-min(x)); the LN apply pass
and its stats pass disappear.  The per-head attention-output normalize ops
carry accum_out, so sum(x) per token is free.

Attention: q heads are host-permuted into pairs (0,2),(1,3),(4,6)... so a
q-pair shares one K tile pair; the two 64-contraction score matmuls run
CONCURRENTLY in the PE array as row-tiles (base partitions 0 and 64),
writing one 2-bank PSUM pair that a single Exp activation consumes.
Softmax denominators come free via a ones-column appended to V.
"""

import numpy as np
import ml_dtypes
from contextlib import ExitStack

import concourse.bacc as bacc
import concourse.mybir as mybir
import concourse.tile as tile
from concourse.bass_utils import run_bass_kernel_spmd
from concourse.masks import make_identity

F32 = mybir.dt.float32
BF16 = mybir.dt.bfloat16
AX = mybir.AxisListType
OP = mybir.AluOpType
ACT = mybir.ActivationFunctionType

# model dims
B, N, S, D = 2, 2048, 256, 768
HQ, HK, HEAD = 12, 6, 64
DKV = HEAD * HK          # 384
H4 = 4 * D               # 3072
NCORES = 8
GROUPS = [[0, 1, 2, 3], [4, 5, 6, 7]]
GSZ = 4                  # cores per batch group
T = (B * N) // NCORES    # 512 tokens per core
NT = T // 128            # 4 token tiles per core
ST = S // 128            # 2 condition token tiles
KT = D // 128            # 6 feature tiles of D
KTH = H4 // 128          # 24 feature tiles of 4D
KP = DKV // 128          # 3 kv-head-pair tiles

# q heads permuted so psum pair tile mt holds (QPERM[2mt], QPERM[2mt+1]),
# and both heads of a pair read the same gathered K pair tile.
QPERM = [0, 2, 1, 3, 4, 6, 5, 7, 8, 10, 9, 11]

# (out_features, in_features); device gets ternary bf16 f"{name}_q" [I, O].
WSPECS = {
    'sa_wq': (D, D), 'sa_wk': (DKV, D), 'sa_wv': (DKV, D), 'sa_wo': (D, D),
    'ca_wq': (D, D), 'ca_wk': (DKV, D), 'ca_wv': (DKV, D), 'ca_wo': (D, D),
    'w_cond': (D, D), 'w1': (H4, D), 'w2': (D, H4),
}
SCALE_SLOTS = list(WSPECS)          # order of m scales in the combo row
NSLOT = 16                          # padded scale slots
COMBO_W = NSLOT + 4 * D             # + sa_g, sa_b, ca_g, ca_b

_PROGRAM_CACHE = {}

MAGIC = 12582912.0   # 1.5 * 2^23: fp32 add/sub forces round-half-even to int

# exchange-quarter sizes (elements, bf16)
KSLICE = KP * 128 * 128   # K columns for one 128-token tile, all kp rows
VSLICE = 128 * DKV        # V for one 128-token tile
QSZ = KSLICE + VSLICE


class Ctx:
    pass


# ---------------------------------------------------------------------------
# quant statistics + per-token scale chains
# ---------------------------------------------------------------------------

def _rms_stats_tile(g, X, F, S2, amax, j):
    """Per-tile stats: S2[:, j] = sum(X^2) (Scalar engine Square with
    accum_out), amax[:, j] = max|X| (one DVE reduce).
    (tensor_tensor_reduce would do the square on the DVE, but it crashes
    this runtime -- verified with a minimal repro.)"""
    nc = g.nc
    sq = g.sq_scratch(F)
    nc.scalar.activation(sq, X, ACT.Square, accum_out=S2[:, j:j + 1])
    nc.vector.tensor_reduce(amax[:, j:j + 1], X, axis=AX.X, op=OP.max,
                            apply_absolute_value=True)


def _rms_chain(g, S2, amax, F, nj, uid):
    """al = absmax_n/127 (dequant alpha), srnd = 127*r/absmax_n where
    r = rsqrt(sum(x^2)/F + 1e-6), absmax_n = clip(absmax*r, 1e-5)."""
    nc, qpool = g.nc, g.qpool
    sd = qpool.tile([128, nj], F32, tag=f"qs_{uid}", name=f"qs_{uid}")
    nc.scalar.activation(sd, S2, ACT.Sqrt, bias=g.eps6, scale=1.0 / F)
    r = qpool.tile([128, nj], F32, tag=f"qr_{uid}", name=f"qr_{uid}")
    nc.vector.reciprocal(r, sd)
    amn = qpool.tile([128, nj], F32, tag=f"qm_{uid}", name=f"qm_{uid}")
    nc.vector.tensor_mul(amn, amax, r)
    nc.vector.tensor_scalar_max(amn, amn, 1e-5)
    al = qpool.tile([128, nj], F32, tag=f"al_{uid}", name=f"al_{uid}")
    nc.vector.tensor_scalar_mul(al, amn, 1.0 / 127.0)
    ra = qpool.tile([128, nj], F32, tag=f"qi_{uid}", name=f"qi_{uid}")
    nc.vector.reciprocal(ra, amn)
    srnd = qpool.tile([128, nj], F32, tag=f"qn_{uid}", name=f"qn_{uid}")
    nc.vector.tensor_mul(srnd, ra, r)
    nc.vector.tensor_scalar_mul(srnd, srnd, 127.0)
    return al, srnd


def _quant_tile(g, X, F, srnd_col, dst3, wk, dma_eng, pe_pool=None):
    """Quantize one token tile: round(x*srnd) via the fp32 magic-number
    trick (DVE mul+add, Act sub) -- integer-exact in bf16; then the
    feature-major transpose into dst3 [128, F//128, 128].  With pe_pool
    the transpose runs on the PE + Scalar copy instead of the XBAR DMA
    path (which is blocked while any collective -- including the
    framework's start-of-model barrier -- is in flight)."""
    nc = g.nc
    tmp = wk.tile([128, F], F32, tag=f"qt_{F}", name="qt", bufs=1)
    nc.vector.tensor_scalar(tmp, X, srnd_col, MAGIC, OP.mult, OP.add)
    xq = wk.tile([128, F], BF16, tag=f"xq_{F}", name="xq", bufs=2)
    nc.scalar.activation(xq, tmp, ACT.Copy, bias=-MAGIC)
    if pe_pool is not None:
        for k in range(F // 128):
            ps_t = pe_pool.tile([128, 128], BF16, tag="qpe", name="qpe")
            nc.tensor.transpose(ps_t, xq[:, k * 128:(k + 1) * 128],
                                g.identb)
            if dma_eng is nc.vector:
                nc.vector.tensor_copy(dst3[:, k, :], ps_t)
            else:
                nc.scalar.copy(dst3[:, k, :], ps_t)
    else:
        dma_eng.dma_start(dst3, xq, transpose=True)
    return xq


def _fused_ln_chain(g, S1, S2, mx, mn, nj, uid):
    """g=1,b=0 fast path: rmsnorm(LN(a)) == (a - m) * R with m = S1/D,
    var = S2/D - m^2, R = rsqrt(var*(1+1e-6) + 1e-11).
    absmax = max(mx - m, m - mn) * R.  Quantized int = (a*cq - dq) - MAGIC
    with cq = R*127/clip(absmax,1e-5), dq = m*cq - MAGIC.
    Returns (al, cq, dq); al is the dequant alpha."""
    nc, qpool = g.nc, g.qpool

    def t(nm):
        return qpool.tile([128, nj], F32, tag=f"{nm}_{uid}",
                          name=f"{nm}_{uid}")
    m = t("lm")
    nc.vector.tensor_scalar_mul(m, S1, 1.0 / D)
    t1 = t("lt")
    nc.vector.tensor_scalar_mul(t1, S2, 1.0 / D)
    msq = t("lq")
    nc.vector.tensor_mul(msq, m, m)
    var = t("lv")
    nc.vector.tensor_sub(var, t1, msq)
    dd = t("ld")
    nc.vector.tensor_scalar(dd, var, 1.0 + 1e-6, 1e-11, OP.mult, OP.add)
    sd = t("ls")
    nc.scalar.activation(sd, dd, ACT.Sqrt)
    R = t("lr")
    nc.vector.reciprocal(R, sd)
    t3 = t("l3")
    nc.vector.tensor_sub(t3, mx, m)
    t4 = t("l4")
    nc.vector.tensor_sub(t4, m, mn)
    am = t("la")
    nc.vector.tensor_tensor(am, t3, t4, op=OP.max)
    amn = t("ln")
    nc.vector.tensor_mul(amn, am, R)
    nc.vector.tensor_scalar_max(amn, amn, 1e-5)
    al = t("al")
    nc.vector.tensor_scalar_mul(al, amn, 1.0 / 127.0)
    ra = t("li")
    nc.vector.reciprocal(ra, amn)
    cq = t("lc")
    nc.vector.tensor_mul(cq, ra, R)
    nc.vector.tensor_scalar_mul(cq, cq, 127.0)
    dq = t("lz")
    nc.vector.tensor_mul(dq, m, cq)
    nc.vector.tensor_scalar(dq, dq, MAGIC, None, OP.subtract)
    return al, cq, dq


def _fused_quant_tile(g, X, cq_col, dq_col, dst3, wk, dma_eng):
    """Quantize one fused-LN tile: (X*cq - dq) - MAGIC, then transpose."""
    nc = g.nc
    tmp = wk.tile([128, D], F32, tag="qt_768", name="qt", bufs=1)
    nc.vector.tensor_scalar(tmp, X, cq_col, dq_col, OP.mult, OP.subtract)
    xq = wk.tile([128, D], BF16, tag="xq_768", name="xq", bufs=2)
    nc.scalar.activation(xq, tmp, ACT.Copy, bias=-MAGIC)
    dma_eng.dma_start(dst3, xq, transpose=True)
    return xq


def _layernorm(g, a_tiles, g_bc, b_bc, out_tiles, uid):
    """General-g/b LayerNorm (fallback path)."""
    nc, qpool = g.nc, g.qpool
    nj = len(a_tiles)
    s1 = qpool.tile([128, nj], F32, tag=f"ls1_{uid}", name=f"ls1_{uid}")
    s2 = qpool.tile([128, nj], F32, tag=f"ls2_{uid}", name=f"ls2_{uid}")
    for j, A in enumerate(a_tiles):
        sq = g.sq_scratch(D)
        nc.scalar.activation(sq, A, ACT.Square, accum_out=s2[:, j:j + 1])
        nc.vector.tensor_reduce(s1[:, j:j + 1], A, axis=AX.X, op=OP.add)
    m = qpool.tile([128, nj], F32, tag=f"lmu_{uid}", name=f"lmu_{uid}")
    nc.vector.tensor_scalar_mul(m, s1, 1.0 / D)
    t1 = qpool.tile([128, nj], F32, tag=f"lt1_{uid}", name=f"lt1_{uid}")
    nc.vector.tensor_scalar_mul(t1, s2, 1.0 / D)
    msq = qpool.tile([128, nj], F32, tag=f"lms_{uid}", name=f"lms_{uid}")
    nc.vector.tensor_mul(msq, m, m)
    var = qpool.tile([128, nj], F32, tag=f"lva_{uid}", name=f"lva_{uid}")
    nc.vector.tensor_sub(var, t1, msq)
    sd = qpool.tile([128, nj], F32, tag=f"lsd_{uid}", name=f"lsd_{uid}")
    nc.scalar.activation(sd, var, ACT.Sqrt, bias=g.eps5)
    rs = qpool.tile([128, nj], F32, tag=f"lrs_{uid}", name=f"lrs_{uid}")
    nc.vector.reciprocal(rs, sd)
    for j, A in enumerate(a_tiles):
        X = out_tiles[j]
        nc.vector.tensor_scalar(X, A, m[:, j:j + 1], rs[:, j:j + 1],
                                OP.subtract, OP.mult)
        nc.vector.tensor_mul(X, X, g_bc)
        nc.vector.tensor_add(X, X, b_bc)


def _make_abc(g, al_mat, nj, Ttot, pool, uid):
    """Row-broadcast of per-token alpha: [128, nj] -> [128, Ttot], done
    entirely on the PE (transpose, then a rank-1 ones matmul per 128-token
    block) so it never queues behind gpsimd weight-prefetch DMAs."""
    nc = g.nc
    abc = pool.tile([128, Ttot], F32, tag=f"abc_{uid}", name=f"abc_{uid}")
    with g.tc.tile_pool(name=f"psabc_{uid}", bufs=1, space="PSUM") as pp:
        pst = pp.tile([nj, 128], F32, tag="ps_abc", name="pst")
        nc.tensor.transpose(pst, al_mat, g.ident)
        at = g.stat.tile([nj, 128], F32, tag="at", name="at", bufs=1)
        nc.scalar.copy(at, pst)
        arow = g.stat.tile([1, Ttot], F32, tag="arow", name="arow", bufs=1)
        for j in range(nj):
            nc.sync.dma_start(arow[0:1, j * 128:(j + 1) * 128],
                              at[j:j + 1, :])
        psb = pp.tile([128, Ttot], F32, tag="ps_abc2", name="psb")
        nc.tensor.matmul(psb, g.ones1, arow[0:1, :], start=True, stop=True)
        nc.vector.tensor_copy(abc, psb)
    return abc


# ---------------------------------------------------------------------------
# attention
# ---------------------------------------------------------------------------

def _attn_norm(g, h, o_sb, a_out, s1cols, psum_t, post_j=None):
    """Per-head transpose + softmax normalize; accum_out gives the
    per-token feature sum of the normalized head chunk for free.
    post_j fires after tile j's chunk is written (used by the final head
    to pipeline the downstream LayerNorm stats into the attention tail)."""
    nc = g.nc
    for j in range(NT):
        ps_t = psum_t.tile([128, 65], F32, tag="pst", name="ps_t")
        nc.tensor.transpose(ps_t, o_sb[:, j * 128:(j + 1) * 128],
                            g.ident[0:65, 0:65])
        rec = g.stat.tile([128, 1], F32, tag="rec", name="rec")
        nc.vector.reciprocal(rec, ps_t[:, 64:65])
        acc = s1cols[j][:, h:h + 1] if s1cols is not None else None
        nc.vector.tensor_scalar(a_out[j][:, h * 64:(h + 1) * 64],
                                ps_t[:, 0:64], rec, 0.0, OP.mult, OP.add,
                                accum_out=acc)
        if post_j is not None:
            post_j(j)


def _attention(g, batches, k_views, v_views, q_lo, q_hi, a_out, s1cols,
               psum_s, psum_o, psum_t, awork, acc_pool, tail_cb=None):
    """Batched paired GQA attention.  batches: list of lists of s-tile
    indices in availability order.  The first batch seeds per-head SBUF
    accumulators, middle batches add into them, the last merges and emits
    transposes + normalize.  Single-batch callers skip the accumulators.

    k_views[kp][s]: [128, 128] bf16 (k-heads 2kp/2kp+1 row-tiled);
    v_views[s]: [128, HK, HEAD+1] bf16 (ones column -> denominator)."""
    nc = g.nc
    nb = len(batches)
    accA = accB = None
    if nb > 1:
        accA = acc_pool.tile([65, HQ // 2, 512], BF16, name="accA")
        accB = acc_pool.tile([65, HQ // 2, 512], BF16, name="accB")
    for b, batch in enumerate(batches):
        first, last = b == 0, b == nb - 1
        for qp in range(HQ // 2):
            hA, hB = QPERM[2 * qp], QPERM[2 * qp + 1]
            khA, khB = hA // 2, hB // 2
            kp = khA // 2
            ps_oA = psum_o.tile([65, 512], F32, tag="pvA", name="pvA")
            ps_oB = psum_o.tile([65, 512], F32, tag="pvB", name="pvB")
            for i, s in enumerate(batch):
                ps_pair = psum_s.tile([128, 1024], F32, tag="pss",
                                      name="pss")
                ps_A, ps_B = ps_pair[:, 0:512], ps_pair[:, 512:1024]
                nc.tensor.matmul(ps_A, k_views[kp][s][0:64, :], q_lo[qp],
                                 start=True, stop=True)
                nc.tensor.matmul(ps_B, k_views[kp][s][64:128, :], q_hi[qp],
                                 start=True, stop=True)
                pT = awork.tile([128, 1024], BF16, tag="pT", name="pT",
                                bufs=2)
                nc.scalar.activation(pT, ps_pair, ACT.Exp)
                nc.tensor.matmul(ps_oA, v_views[s][:, khA, :],
                                 pT[:, 0:512], start=(i == 0),
                                 stop=(i == len(batch) - 1))
                nc.tensor.matmul(ps_oB, v_views[s][:, khB, :],
                                 pT[:, 512:1024], start=(i == 0),
                                 stop=(i == len(batch) - 1))
            last_qp = qp == HQ // 2 - 1
            if nb == 1:
                for h, ps_o in ((hA, ps_oA), (hB, ps_oB)):
                    o_sb = awork.tile([65, 512], F32, tag="osb",
                                      name="osb", bufs=2)
                    nc.vector.tensor_copy(o_sb, ps_o)
                    _attn_norm(g, h, o_sb, a_out, s1cols, psum_t,
                               post_j=(tail_cb if last_qp and h == hB
                                       else None))
            elif first:
                nc.vector.tensor_copy(accA[:, qp, :], ps_oA)
                nc.vector.tensor_copy(accB[:, qp, :], ps_oB)
            elif not last:
                nc.vector.tensor_add(accA[:, qp, :], accA[:, qp, :], ps_oA)
                nc.vector.tensor_add(accB[:, qp, :], accB[:, qp, :], ps_oB)
            else:
                for h, ps_o, acc in ((hA, ps_oA, accA), (hB, ps_oB, accB)):
                    o_sb = awork.tile([65, 512], F32, tag="osb",
                                      name="osb", bufs=2)
                    nc.vector.tensor_add(o_sb, acc[:, qp, :], ps_o)
                    _attn_norm(g, h, o_sb, a_out, s1cols, psum_t,
                               post_j=(tail_cb if last_qp and h == hB
                                       else None))


# ---------------------------------------------------------------------------
# program builder
# ---------------------------------------------------------------------------

def build_program(groups=None, fused_ln=True):
    if groups is None:
        groups = GROUPS
    gsz = len(groups[0])
    nc = bacc.Bacc()

    x_in = nc.declare_dram_parameter("x_sh", [T, D], F32, isOutput=False)
    y_in = nc.declare_dram_parameter("y_b", [S, D], F32, isOutput=False)
    wt_in = {}
    for name, (O, I) in WSPECS.items():
        wt_in[name] = nc.declare_dram_parameter(f"{name}_q", [I, O], BF16,
                                                isOutput=False)
    combo_in = nc.declare_dram_parameter("combo", [1, COMBO_W], F32,
                                         isOutput=False)
    out_sh = nc.declare_dram_parameter("out_sh", [T, D], F32, isOutput=True)

    g = Ctx()
    g.nc = nc

    with tile.TileContext(nc) as tc, ExitStack() as ctx:
        g.tc = tc
        g.const = ctx.enter_context(tc.tile_pool(name="const", bufs=1))
        g.stat = ctx.enter_context(tc.tile_pool(name="stat", bufs=4))
        g.work = ctx.enter_context(tc.tile_pool(name="work", bufs=2))
        g.qpool = ctx.enter_context(tc.tile_pool(name="qpool", bufs=1))
        sqpool = ctx.enter_context(tc.tile_pool(name="sqpool", bufs=1))
        dram = ctx.enter_context(tc.tile_pool(name="dram", bufs=1,
                                              space="DRAM"))

        def sq_scratch(F):
            return sqpool.tile([128, F], BF16, tag=f"sq_{F}", name="sq")
        g.sq_scratch = sq_scratch

        # four quarter-exchanges, one per local 128-token tile
        cc_in = [dram.tile([QSZ], BF16, name=f"cc_in{i}") for i in range(NT)]
        cc_out = [dram.tile([gsz, QSZ], BF16, name=f"cc_out{i}")
                  for i in range(NT)]

        g.eps6 = g.const.tile([128, 1], F32, name="eps6")
        nc.vector.memset(g.eps6, 1e-6)
        g.eps5 = g.const.tile([128, 1], F32, name="eps5")
        nc.vector.memset(g.eps5, 1e-5)
        g.ident = g.const.tile([128, 128], F32, name="ident")
        make_identity(nc, g.ident)
        identb = g.const.tile([128, 128], BF16, name="identb")
        nc.vector.tensor_copy(identb, g.ident)
        g.identb = identb
        g.ones1 = g.const.tile([1, 128], F32, name="ones1")
        nc.vector.memset(g.ones1, 1.0)

        # one DMA + partition broadcasts for the scales (+ LN params only
        # in the general-g/b fallback path)
        cbw = NSLOT if fused_ln else COMBO_W
        cb = g.const.tile([128, cbw], F32, name="cb")
        with tc.tile_pool(name="crowp", bufs=1) as crowp:
            crow = crowp.tile([1, COMBO_W], F32, name="crow")
            nc.scalar.dma_start(crow, combo_in[:, :])
            nc.gpsimd.partition_broadcast(cb[:, 0:NSLOT], crow[0:1, 0:NSLOT])
            if not fused_ln:
                for i in range(4):
                    sl = slice(NSLOT + i * D, NSLOT + (i + 1) * D)
                    nc.gpsimd.partition_broadcast(cb[:, sl], crow[0:1, sl])
        msc = {name: cb[:, i:i + 1] for i, name in enumerate(SCALE_SLOTS)}
        ln_bc = ({} if fused_ln else
                 {name: cb[:, NSLOT + i * D:NSLOT + (i + 1) * D]
                  for i, name in enumerate(('sa_g', 'sa_b',
                                            'ca_g', 'ca_b'))})

        g.ka_pool = None

        def keepalive(ap, n):
            """Dummy matmuls reading `ap` (bf16, <=512 cols) to hold the PE
            p-state up through otherwise PE-idle stretches."""
            if g.ka_pool is None:
                return
            for _ in range(n):
                ps = g.ka_pool.tile([128, 512], F32, tag="ka", name="ka")
                nc.tensor.matmul(ps, identb, ap, start=True, stop=True)
        g.keepalive = keepalive

        def load_weight(pool, name, eng):
            O, I = WSPECS[name]
            rows = I // 128
            wt = pool.tile([128, rows, O], BF16, tag=f"w_{name}",
                           name=f"w_{name}")
            for r in range(rows):
                eng.dma_start(wt[:, r, :],
                              wt_in[name][r * 128:(r + 1) * 128, :])
            return wt

        def proj_fm(wsb, xqT_all, mscale, abc, O, Ttot, pool, tag, ps_pool):
            """feature-major projection: O//128 tiles [128, Ttot] bf16."""
            nk = xqT_all.shape[1]
            outs = []
            for mt in range(O // 128):
                ps = ps_pool.tile([128, Ttot], F32, tag="ps", name="ps_pf")
                for k in range(nk):
                    nc.tensor.matmul(ps, wsb[:, k, mt * 128:(mt + 1) * 128],
                                     xqT_all[:, k, :], start=(k == 0),
                                     stop=(k == nk - 1))
                o = pool.tile([128, Ttot], BF16, tag=f"{tag}{mt}",
                              name=f"{tag}{mt}")
                nc.vector.scalar_tensor_tensor(o, ps, mscale, abc,
                                               OP.mult, OP.mult)
                outs.append(o)
            return outs

        def proj_tok_resid(xq_j, wsb, al_mat, mscale, resid_tiles,
                           out_tiles, ps_pool, nk=KT, pre=None, post=None):
            """token-major projection + dequant + residual add, with
            per-tile pre (quantize just-in-time) and post (stats of the
            produced residual tile) hooks so everything pipelines."""
            for j in range(NT):
                if pre is not None:
                    pre(j)
                xqj = xq_j(j)
                ao = g.stat.tile([128, 1], F32, tag="ao", name="ao")
                nc.vector.tensor_mul(ao, al_mat[:, j:j + 1], mscale)
                for c in range(2):
                    ps = ps_pool.tile([128, 384], F32, tag="ps",
                                      name="ps_pt")
                    for k in range(nk):
                        nc.tensor.matmul(
                            ps, xqj[:, k, :],
                            wsb[:, k, c * 384:(c + 1) * 384],
                            start=(k == 0), stop=(k == nk - 1))
                    nc.vector.scalar_tensor_tensor(
                        out_tiles[j][:, c * 384:(c + 1) * 384], ps, ao,
                        resid_tiles[j][:, c * 384:(c + 1) * 384],
                        OP.mult, OP.add)
                if post is not None:
                    post(j)

        # ---- LN1 (+fused rms) + quant + wo projection + residual ----
        def make_ln_stats(uid):
            """Stats tiles + a tail callback that fills them per tile as
            the final attention head lands (fused path only)."""
            S2m = g.qpool.tile([128, NT], F32, tag=f"wS2_{uid}",
                               name=f"wS2_{uid}")
            if not fused_ln:
                return (S2m, None, None, None), None, None

            S1m = g.qpool.tile([128, NT], F32, tag=f"wS1_{uid}",
                               name=f"wS1_{uid}")
            mx = g.qpool.tile([128, NT], F32, tag=f"wmx_{uid}",
                              name=f"wmx_{uid}")
            mn = g.qpool.tile([128, NT], F32, tag=f"wmn_{uid}",
                              name=f"wmn_{uid}")
            holder = {}

            def tail_cb(j):
                a_tok, s1cols = holder['a_tok'], holder['s1cols']
                sq = g.sq_scratch(D)
                nc.scalar.activation(sq, a_tok[j], ACT.Square,
                                     accum_out=S2m[:, j:j + 1])
                nc.vector.tensor_reduce(S1m[:, j:j + 1], s1cols[j],
                                        axis=AX.X, op=OP.add)
                nc.vector.tensor_reduce(mx[:, j:j + 1], a_tok[j],
                                        axis=AX.X, op=OP.max)
                nc.vector.tensor_reduce(mn[:, j:j + 1], a_tok[j],
                                        axis=AX.X, op=OP.min)
            return (S2m, S1m, mx, mn), tail_cb, holder

        def wo_block(a_tok, s1cols, gname, bname, woname, resid, out_tiles,
                     aqT, post, uid, stats):
            es_ka = ExitStack()
            g.ka_pool = es_ka.enter_context(
                tc.tile_pool(name=f"ka_{uid}", bufs=1, space="PSUM"))
            S2m, S1m, mx, mn = stats
            if fused_ln:
                al, cq, dq = _fused_ln_chain(g, S1m, S2m, mx, mn, NT, uid)

                def pre(j):
                    _fused_quant_tile(
                        g, a_tok[j], cq[:, j:j + 1], dq[:, j:j + 1],
                        aqT[:, :, j * 128:(j + 1) * 128], g.work, nc.sync)
            else:
                ln_t = a_tok
                _layernorm(g, a_tok, ln_bc[gname], ln_bc[bname], ln_t, uid)
                amax = g.qpool.tile([128, NT], F32, tag=f"wam_{uid}",
                                    name=f"wam_{uid}")
                for j in range(NT):
                    _rms_stats_tile(g, ln_t[j], D, S2m, amax, j)
                al, srnd = _rms_chain(g, S2m, amax, D, NT, uid)

                def pre(j):
                    _quant_tile(g, ln_t[j], D, srnd[:, j:j + 1],
                                aqT[:, :, j * 128:(j + 1) * 128],
                                g.work, nc.sync)
            with tc.tile_pool(name=f"pswo_{uid}", bufs=3,
                              space="PSUM") as pswo:
                proj_tok_resid(
                    lambda j: aqT[:, :, j * 128:(j + 1) * 128],
                    g.w[woname], al, msc[woname], resid, out_tiles,
                    pswo, pre=pre, post=post)
            g.ka_pool = None
            es_ka.close()


        # ------------------------------------------------------------------
        # scoped pools
        # ------------------------------------------------------------------
        es_wsa = ExitStack()
        es_wca = ExitStack()
        es_x = ExitStack()
        es_x2 = ExitStack()
        es_sa = ExitStack()
        es_cond = ExitStack()
        es_ffnw = ExitStack()
        es_saq = ExitStack()

        resid3 = ctx.enter_context(tc.tile_pool(name="resid3", bufs=1))
        x3_all = resid3.tile([128, NT, D], F32, name="x3_all")
        x3 = [x3_all[:, j, :] for j in range(NT)]
        x2pool = es_x2.enter_context(tc.tile_pool(name="x2pool", bufs=1,
                                                  side="right"))
        x2_all = x2pool.tile([128, NT, D], F32, name="x2_all")
        x2 = [x2_all[:, j, :] for j in range(NT)]

        # x first on the sync queue, then its stats/quant compute ops are
        # emitted BEFORE any weight-row DMA lands on a compute-engine
        # queue: HBM bandwidth is saturated during startup, so a weight
        # DMA ahead of the stats ops would stall them ~20us.
        with_wsa = es_wsa.enter_context(tc.tile_pool(name="w_sa", bufs=1))
        with_wca = es_wca.enter_context(tc.tile_pool(name="w_ca", bufs=1,
                                                     side="right"))
        xpool = es_x.enter_context(tc.tile_pool(name="xpool", bufs=1))
        x_all = xpool.tile([128, NT, D], F32, name="x_all")
        for j in range(NT):
            nc.sync.dma_start(x_all[:, j, :], x_in[j * 128:(j + 1) * 128, :])
        x_tiles = [x_all[:, j, :] for j in range(NT)]

        # K/V/Q weight rows immediately behind x on the sync queue: the
        # bandwidth window while the stats run is otherwise free (DMA
        # transposes of the quant tiles only start ~30us in).
        g.w = {}
        g.w['sa_wk'] = load_weight(with_wsa, 'sa_wk', nc.sync)
        g.w['sa_wv'] = load_weight(with_wsa, 'sa_wv', nc.sync)
        g.w['sa_wq'] = load_weight(with_wsa, 'sa_wq', nc.sync)

        sa_act = es_sa.enter_context(tc.tile_pool(name="sa_act", bufs=1))
        sa_xq = es_saq.enter_context(tc.tile_pool(name="sa_xq", bufs=1))

        # ---- SA input quant ----
        x1qT = sa_xq.tile([128, KT, T], BF16, name="x1qT")
        S2x = g.qpool.tile([128, NT], F32, tag="S2x1", name="S2x1")
        amx1 = g.qpool.tile([128, NT], F32, tag="amx1", name="amx1")
        for j in range(NT):
            _rms_stats_tile(g, x_tiles[j], D, S2x, amx1, j)
        al_x, srnd_x = _rms_chain(g, S2x, amx1, D, NT, "x1")
        first_xq = None
        with tc.tile_pool(name="ps_qpe", bufs=4, space="PSUM") as qpe:
            for j in range(NT):
                xq = _quant_tile(g, x_tiles[j], D, srnd_x[:, j:j + 1],
                                 x1qT[:, :, j * 128:(j + 1) * 128], g.work,
                                 nc.sync, pe_pool=qpe)
                if first_xq is None:
                    first_xq = xq
        abc_x = _make_abc(g, al_x, NT, T, sa_xq, "x1")

        # HAM warm-up: dense burst reading the first quant tile ramps the
        # PE clock while the remaining quant tiles stream.
        with tc.tile_pool(name="ps_warm0", bufs=1, space="PSUM") as psw:
            wps = psw.tile([128, 512], F32, tag="warm0", name="warm0")
            for _ in range(16):
                nc.tensor.matmul(wps, identb, first_xq[:, 0:512],
                                 start=True, stop=True)

        # ---- K, V projections; fire the four quarter-gathers; then Q ----
        with tc.tile_pool(name="ps_proj", bufs=2, space="PSUM") as psp:
            kf = proj_fm(g.w['sa_wk'], x1qT, msc['sa_wk'], abc_x, DKV, T,
                         sa_xq, "kf", psp)
            for j in range(NT):
                for t in range(KP):
                    dst = cc_in[j][t * 128 * 128:(t + 1) * 128 * 128]
                    nc.sync.dma_start(
                        dst.rearrange("(p t) -> p t", p=128),
                        kf[t][:, j * 128:(j + 1) * 128])
            for j in range(NT):
                ps = psp.tile([128, DKV], F32, tag="psv", name="ps_v")
                for k in range(KT):
                    nc.tensor.matmul(ps, x1qT[:, k, j * 128:(j + 1) * 128],
                                     g.w['sa_wv'][:, k, :], start=(k == 0),
                                     stop=(k == KT - 1))
                av = g.stat.tile([128, 1], F32, tag="av", name="av")
                nc.vector.tensor_mul(av, al_x[:, j:j + 1], msc['sa_wv'])
                vtok = g.work.tile([128, DKV], BF16, tag="vtok",
                                   name="vtok")
                nc.vector.tensor_scalar_mul(vtok, ps, av)
                nc.sync.dma_start(
                    cc_in[j][KSLICE:KSLICE + VSLICE].rearrange(
                        "(p f) -> p f", p=128), vtok)
                nc.gpsimd.collective_compute(
                    "AllGather", OP.bypass, replica_groups=groups,
                    ins=[cc_in[j][:].opt()],
                    outs=[cc_out[j][:, :].opt()])

            # deferred weight prefetch: the gpsimd SWDGE queue is blocked
            # by the gather triggers above until the K/V writes land, so
            # these streams start only once the startup crunch is over.
            for k in ('w_cond', 'ca_wk', 'ca_wv'):
                g.w[k] = load_weight(with_wca, k, nc.gpsimd)
            g.w['sa_wo'] = load_weight(with_wsa, 'sa_wo', nc.gpsimd)

            qpairs = proj_fm(g.w['sa_wq'], x1qT, msc['sa_wq'], abc_x, D, T,
                             sa_act, "qp", psp)
            es_saq.close()

            # ---- CA condition-side work (independent of x; overlaps the
            # gathers).  All its DMAs go on the scalar queue so they can
            # never sit behind a gather-dependent wait. ----
            ca_cond = es_cond.enter_context(tc.tile_pool(name="ca_cond",
                                                         bufs=1,
                                                         side="right"))
            with tc.tile_pool(name="ysc", bufs=1) as ysc:
                y_all = ysc.tile([128, ST, D], F32, name="y_all")
                for j in range(ST):
                    nc.scalar.dma_start(y_all[:, j, :],
                                        y_in[j * 128:(j + 1) * 128, :])
                y_tiles = [y_all[:, j, :] for j in range(ST)]
                yqT = ysc.tile([128, KT, S], BF16, name="yqT")
                S2y = g.qpool.tile([128, ST], F32, tag="S2y", name="S2y")
                amy = g.qpool.tile([128, ST], F32, tag="amy", name="amy")
                for j in range(ST):
                    _rms_stats_tile(g, y_tiles[j], D, S2y, amy, j)
                al_y, srnd_y = _rms_chain(g, S2y, amy, D, ST, "y")
                with tc.tile_pool(name="ps_qpy", bufs=2,
                                  space="PSUM") as qpy:
                    for j in range(ST):
                        _quant_tile(g, y_tiles[j], D, srnd_y[:, j:j + 1],
                                    yqT[:, :, j * 128:(j + 1) * 128],
                                    g.work, nc.vector, pe_pool=qpy)
                yc_all = ysc.tile([128, ST, D], F32, name="yc_all")
                yc = [yc_all[:, j, :] for j in range(ST)]
                for j in range(ST):
                    am = g.stat.tile([128, 1], F32, tag="am", name="am")
                    nc.vector.tensor_mul(am, al_y[:, j:j + 1],
                                         msc['w_cond'])
                    for c in range(2):
                        ps = psp.tile([128, 384], F32, tag="psy",
                                      name="ps_yc")
                        for k in range(KT):
                            nc.tensor.matmul(
                                ps, yqT[:, k, j * 128:(j + 1) * 128],
                                g.w['w_cond'][:, k, c * 384:(c + 1) * 384],
                                start=(k == 0), stop=(k == KT - 1))
                        nc.vector.tensor_scalar_mul(
                            yc[j][:, c * 384:(c + 1) * 384], ps, am)

                ycqT = ysc.tile([128, KT, S], BF16, name="ycqT")
                S2c = g.qpool.tile([128, ST], F32, tag="S2c", name="S2c")
                amc = g.qpool.tile([128, ST], F32, tag="amc", name="amc")
                for j in range(ST):
                    _rms_stats_tile(g, yc[j], D, S2c, amc, j)
                al_yc, srnd_yc = _rms_chain(g, S2c, amc, D, ST, "yc")
                with tc.tile_pool(name="ps_qpc", bufs=2,
                                  space="PSUM") as qpc:
                    for j in range(ST):
                        _quant_tile(g, yc[j], D, srnd_yc[:, j:j + 1],
                                    ycqT[:, :, j * 128:(j + 1) * 128],
                                    g.work, nc.vector, pe_pool=qpc)
                abc_yc = _make_abc(g, al_yc, ST, S, ysc, "yc")

                ca_kpairs = proj_fm(g.w['ca_wk'], ycqT, msc['ca_wk'],
                                    abc_yc, DKV, S, ca_cond, "ck", psp)
                v_ca = []
                for j in range(ST):
                    ps = psp.tile([128, DKV], F32, tag="psv", name="ps_vc")
                    for k in range(KT):
                        nc.tensor.matmul(
                            ps, ycqT[:, k, j * 128:(j + 1) * 128],
                            g.w['ca_wv'][:, k, :], start=(k == 0),
                            stop=(k == KT - 1))
                    av = g.stat.tile([128, 1], F32, tag="av", name="avc")
                    nc.vector.tensor_mul(av, al_yc[:, j:j + 1],
                                         msc['ca_wv'])
                    va = ca_cond.tile([128, HK, HEAD + 1], BF16,
                                      tag=f"vc{j}", name=f"vc{j}")
                    nc.vector.tensor_scalar_mul(
                        va[:, :, 0:HEAD],
                        ps.rearrange("p (h e) -> p h e", e=HEAD), av)
                    nc.vector.memset(va[:, :, HEAD:HEAD + 1], 1.0)
                    v_ca.append(va)

        # ---- gathered K/V tiles; s-tile index = quarter j * gsz + slot ----
        # (pool opened only now, after ysc closed, so the cond-side scratch
        # and the gathered K/V never coexist in SBUF)
        sa_kv = es_sa.enter_context(tc.tile_pool(name="sa_kv", bufs=1))
        kt_g = []
        for kp in range(KP):
            kt = sa_kv.tile([128, NT * gsz, 128], BF16, tag=f"kT{kp}",
                            name=f"kT{kp}")
            kt_g.append(kt)
        v_aug = []
        for s in range(NT * gsz):
            va = sa_kv.tile([128, HK, HEAD + 1], BF16, tag=f"va{s}",
                            name=f"va{s}")
            nc.vector.memset(va[:, :, HEAD:HEAD + 1], 1.0)
            v_aug.append(va)
        for j in range(NT):
            for kp in range(KP):
                src = cc_out[j][:, kp * 128 * 128:(kp + 1) * 128 * 128]
                nc.sync.dma_start(
                    kt_g[kp][:, j * gsz:(j + 1) * gsz, :],
                    src.rearrange("r (p t) -> p r t", p=128))
            for r in range(gsz):
                s = j * gsz + r
                src = cc_out[j][r, KSLICE:KSLICE + VSLICE]
                nc.sync.dma_start(
                    v_aug[s][:, :, 0:HEAD],
                    src.rearrange("(p h e) -> p h e", p=128, e=HEAD))
        k_views = [[kt_g[kp][:, s, :] for s in range(NT * gsz)]
                   for kp in range(KP)]

        # a second HAM warm-up right before attention
        with tc.tile_pool(name="ps_warm1", bufs=1, space="PSUM") as psw:
            wps = psw.tile([128, 512], F32, tag="warm1", name="warm1")
            for _ in range(12):
                nc.tensor.matmul(wps, identb, qpairs[0][:, 0:512],
                                 start=True, stop=True)

        q_lo = [qt[0:64, :] for qt in qpairs]
        q_hi = [qt[64:128, :] for qt in qpairs]

        a_pool = es_sa.enter_context(tc.tile_pool(name="a_pool", bufs=1))
        a_all = a_pool.tile([128, NT, D], F32, name="a_all")
        a_tok = [a_all[:, j, :] for j in range(NT)]
        s1c = None
        if fused_ln:
            s1c = [a_pool.tile([128, HQ], F32, tag=f"s1c{j}",
                               name=f"s1c{j}") for j in range(NT)]
        stats_l1, tail_l1, hold_l1 = make_ln_stats("l1")
        if hold_l1 is not None:
            hold_l1['a_tok'], hold_l1['s1cols'] = a_tok, s1c
        batches = [[j * gsz + r for r in range(gsz)] for j in range(NT)]
        with tc.tile_pool(name="awork", bufs=1) as awork, \
             tc.tile_pool(name="ps_s", bufs=2, space="PSUM") as psum_s, \
             tc.tile_pool(name="ps_o", bufs=1, space="PSUM") as psum_o, \
             tc.tile_pool(name="ps_t", bufs=2, space="PSUM") as psum_t:
            _attention(g, batches, k_views, v_aug, q_lo, q_hi, a_tok, s1c,
                       psum_s, psum_o, psum_t, awork, a_pool,
                       tail_cb=tail_l1)

        # x2 stats + per-tile chain + quant pipeline into the wo
        # projection, so the q2 projection can start right after wo(3)
        S2x2 = g.qpool.tile([128, NT], F32, tag="S2x2", name="S2x2")
        amx2 = g.qpool.tile([128, NT], F32, tag="amx2", name="amx2")
        al_x2m = g.qpool.tile([128, NT], F32, tag="alx2m", name="alx2m")
        x2qT = x2pool.tile([128, KT, T], BF16, name="x2qT")

        def post_x2(j):
            _rms_stats_tile(g, x2[j], D, S2x2, amx2, j)
            al_j, srnd_j = _rms_chain(g, S2x2[:, j:j + 1],
                                      amx2[:, j:j + 1], D, 1, f"x2{j}")
            nc.vector.tensor_copy(al_x2m[:, j:j + 1], al_j)
            _quant_tile(g, x2[j], D, srnd_j[:, 0:1],
                        x2qT[:, :, j * 128:(j + 1) * 128], g.work, nc.sync)

        a1qT = es_sa.enter_context(
            tc.tile_pool(name="a1qT", bufs=1)).tile(
                [128, KT, T], BF16, name="a1qT")
        wo_block(a_tok, s1c, 'sa_g', 'sa_b', 'sa_wo', x_tiles, x2, a1qT,
                 post_x2, "l1", stats_l1)
        es_sa.close()
        es_x.close()
        es_wsa.close()

        ffn_w = es_ffnw.enter_context(tc.tile_pool(name="ffn_w", bufs=1))

        # ---- CA ----
        with tc.tile_pool(name="ca_act", bufs=1) as ca_act, \
             tc.tile_pool(name="awork2", bufs=1) as awork:
            abc_x2 = _make_abc(g, al_x2m, NT, T, ca_act, "x2")
            # weight prefetch emitted only now, so the DMA burst overlaps
            # the q2 projection + CA attention instead of the x2 transposes
            g.w['ca_wq'] = load_weight(with_wca, 'ca_wq', nc.sync)
            g.w['ca_wo'] = load_weight(with_wca, 'ca_wo', nc.gpsimd)
            g.w['w1'] = load_weight(ffn_w, 'w1', nc.gpsimd)
            with tc.tile_pool(name="ps_q2", bufs=3, space="PSUM") as psq:
                q2pairs = proj_fm(g.w['ca_wq'], x2qT, msc['ca_wq'], abc_x2,
                                  D, T, ca_act, "q2", psq)

            q2_lo = [qt[0:64, :] for qt in q2pairs]
            q2_hi = [qt[64:128, :] for qt in q2pairs]
            ck_views = [[ca_kpairs[kp][:, s * 128:(s + 1) * 128]
                         for s in range(ST)] for kp in range(KP)]

            # keep the PE warm into the short CA attention phase
            with tc.tile_pool(name="ps_warm2", bufs=1, space="PSUM") as psw:
                wps = psw.tile([128, 512], F32, tag="warm2", name="warm2")
                for _ in range(10):
                    nc.tensor.matmul(wps, identb, q2pairs[0][:, 0:512],
                                     start=True, stop=True)

            a2_all = ca_act.tile([128, NT, D], F32, name="a2_all")
            a2_tok = [a2_all[:, j, :] for j in range(NT)]
            s2c = None
            if fused_ln:
                s2c = [ca_act.tile([128, HQ], F32, tag=f"s2c{j}",
                                   name=f"s2c{j}") for j in range(NT)]
            stats_l2, tail_l2, hold_l2 = make_ln_stats("l2")
            if hold_l2 is not None:
                hold_l2['a_tok'], hold_l2['s1cols'] = a2_tok, s2c
            with tc.tile_pool(name="ps_s2", bufs=2, space="PSUM") as psum_s, \
                 tc.tile_pool(name="ps_o2", bufs=1, space="PSUM") as psum_o, \
                 tc.tile_pool(name="ps_t2", bufs=2, space="PSUM") as psum_t:
                _attention(g, [list(range(ST))], ck_views, v_ca, q2_lo,
                           q2_hi, a2_tok, s2c, psum_s, psum_o, psum_t,
                           awork, ca_act, tail_cb=tail_l2)

            # x3 stats + per-tile chain + quant into the wo2 projection
            S2x3 = g.qpool.tile([128, NT], F32, tag="S2x3", name="S2x3")
            amx3 = g.qpool.tile([128, NT], F32, tag="amx3", name="amx3")
            al_x3m = g.qpool.tile([128, NT], F32, tag="alx3m",
                                  name="alx3m")
            x3qT = resid3.tile([128, KT, T], BF16, name="x3qT")

            def post_x3(j):
                _rms_stats_tile(g, x3[j], D, S2x3, amx3, j)
                al_j, srnd_j = _rms_chain(g, S2x3[:, j:j + 1],
                                          amx3[:, j:j + 1], D, 1, f"x3{j}")
                nc.vector.tensor_copy(al_x3m[:, j:j + 1], al_j)
                _quant_tile(g, x3[j], D, srnd_j[:, 0:1],
                            x3qT[:, :, j * 128:(j + 1) * 128], g.work,
                            nc.sync)

            a2qT = x2qT        # x2qT is dead after the q2 projection
            wo_block(a2_tok, s2c, 'ca_g', 'ca_b', 'ca_wo', x2, x3, a2qT,
                     post_x3, "l2", stats_l2)
        es_cond.close()
        es_wca.close()
        es_x2.close()

        g.w['w2'] = load_weight(ffn_w, 'w2', nc.gpsimd)

        # ---- FFN ----
        with tc.tile_pool(name="ffn_act", bufs=1) as ffn_act, \
             tc.tile_pool(name="ffn_wk", bufs=1) as ffn_wk, \
             tc.tile_pool(name="outp", bufs=2) as outp:
            # per-tile pipeline, SKEWED two tiles: w2(j) is emitted after
            # w1(j+2), so the PE's in-order queue gives tile j's quant
            # chain two w1 blocks (~15us) of latency cover instead of
            # stalling the PE on every tile.
            hq_tiles = [None] * NT
            ah_tiles = [None] * NT

            def w1_block(j):
                a3 = g.stat.tile([128, 1], F32, tag=f"a3_{j}",
                                 name=f"a3_{j}")
                nc.vector.tensor_mul(a3, al_x3m[:, j:j + 1], msc['w1'])
                h_j = ffn_act.tile([128, H4], BF16, tag="h_j",
                                   name="h_j", bufs=3)
                for c in range(6):
                    ps = psw1.tile([128, 512], F32, tag="ps", name="ps_h")
                    for k in range(KT):
                        nc.tensor.matmul(
                            ps, x3qT[:, k, j * 128:(j + 1) * 128],
                            g.w['w1'][:, k, c * 512:(c + 1) * 512],
                            start=(k == 0), stop=(k == KT - 1))
                    nc.scalar.activation(h_j[:, c * 512:(c + 1) * 512],
                                         ps, ACT.Gelu, bias=0.0, scale=a3)
                S2h = g.qpool.tile([128, 1], F32, tag=f"S2h{j}",
                                   name=f"S2h{j}")
                amh = g.qpool.tile([128, 1], F32, tag=f"amh{j}",
                                   name=f"amh{j}")
                _rms_stats_tile(g, h_j, H4, S2h, amh, 0)
                al_h, srnd_h = _rms_chain(g, S2h, amh, H4, 1, f"h{j}")
                hqT = ffn_act.tile([128, KTH, 128], BF16, tag="hqT",
                                   name="hqT", bufs=3)
                _quant_tile(g, h_j, H4, srnd_h[:, 0:1], hqT, ffn_wk,
                            nc.sync)
                ah = g.stat.tile([128, 1], F32, tag=f"ah{j}",
                                 name=f"ah{j}")
                nc.vector.tensor_mul(ah, al_h[:, 0:1], msc['w2'])
                hq_tiles[j] = hqT
                ah_tiles[j] = ah

            def w2_block(j):
                hqT, ah = hq_tiles[j], ah_tiles[j]
                xo = outp.tile([128, D], F32, tag="xo", name="xo")
                for c in range(2):
                    ps = psw2.tile([128, 384], F32, tag="ps", name="ps_w2")
                    for k in range(KTH):
                        nc.tensor.matmul(
                            ps, hqT[:, k, :],
                            g.w['w2'][:, k, c * 384:(c + 1) * 384],
                            start=(k == 0), stop=(k == KTH - 1))
                    nc.vector.scalar_tensor_tensor(
                        xo[:, c * 384:(c + 1) * 384], ps, ah,
                        x3[j][:, c * 384:(c + 1) * 384], OP.mult, OP.add)
                nc.sync.dma_start(out_sh[j * 128:(j + 1) * 128, :], xo)

            with tc.tile_pool(name="ps_w1", bufs=4, space="PSUM") as psw1, \
                 tc.tile_pool(name="ps_w2", bufs=3, space="PSUM") as psw2:
                for j in range(NT + 2):
                    if j < NT:
                        w1_block(j)
                    if j >= 2:
                        w2_block(j - 2)
        es_ffnw.close()

    nc.finalize()
    return nc


def _get_program(key):
    if key not in _PROGRAM_CACHE:
        groups, fused = key
        _PROGRAM_CACHE[key] = build_program(
            GROUPS if groups == "full" else [[0]], fused_ln=fused)
    return _PROGRAM_CACHE[key]


LAST_RESULT = None


def _host_quant(w):
    """Exact ternary weight quant (same math as reference _weight_quant)."""
    w = np.asarray(w, np.float32)
    m = np.float32(np.mean(np.abs(w), dtype=np.float32))
    m = np.float32(max(m, np.float32(1e-5)))
    q = np.clip(np.rint(w / m), -1.0, 1.0)
    return q.astype(np.float32), m


def kernel(**inputs):
    """Full-input entry: shard across 8 cores, run, gather."""
    global LAST_RESULT
    x = np.ascontiguousarray(np.asarray(inputs['x'], dtype=np.float32))
    y = np.ascontiguousarray(np.asarray(inputs['y'], dtype=np.float32))

    fused = all(
        np.allclose(np.asarray(inputs[k], np.float32), v, atol=0.0)
        for k, v in (('sa_g', 1.0), ('sa_b', 0.0),
                     ('ca_g', 1.0), ('ca_b', 0.0)))
    nc = _get_program(("full", fused))

    qrows = np.concatenate([np.arange(h * 64, (h + 1) * 64)
                            for h in QPERM])
    combo = np.zeros((1, COMBO_W), np.float32)
    common = {}
    for i, name in enumerate(SCALE_SLOTS):
        q, m = _host_quant(inputs[name])
        if name in ('sa_wq', 'ca_wq'):
            q = q[qrows, :]
            m = m / np.float32(np.sqrt(np.float32(HEAD)))
        combo[0, i] = m
        common[f"{name}_q"] = np.ascontiguousarray(
            q.T.astype(ml_dtypes.bfloat16))
    for i, name in enumerate(('sa_g', 'sa_b', 'ca_g', 'ca_b')):
        combo[0, NSLOT + i * D:NSLOT + (i + 1) * D] = np.asarray(
            inputs[name], np.float32)
    common['combo'] = combo

    in_maps = []
    for c in range(NCORES):
        b, seg = c // GSZ, c % GSZ
        m = dict(common)
        m['x_sh'] = np.ascontiguousarray(x[b, seg * T:(seg + 1) * T, :])
        m['y_b'] = np.ascontiguousarray(y[b])
        in_maps.append(m)
    res = run_bass_kernel_spmd(nc, in_maps, core_ids=list(range(NCORES)))
    LAST_RESULT = res
    out = np.empty((B, N, D), np.float32)
    for c in range(NCORES):
        b, seg = c // GSZ, c % GSZ
        out[b, seg * T:(seg + 1) * T, :] = res.results[c]['out_sh']
    return out


# revision 59
# speedup vs baseline: 1.1927x; 1.0848x over previous
"""Trainium2 Bass kernel for nn_DecoderBlock (BitNet-style decoder block with
self-attention, cross-attention and BitFeedForward), data-parallel over
(batch x sequence) tokens across 8 NeuronCores.

Sharding: 4096 tokens (B=2 x N=2048) split into 8 shards of 512 tokens.
Cores 0-3 hold batch 0, cores 4-7 batch 1.  Self-attention K/V are
computed on local tokens and exchanged within each 4-core batch group by
FOUR pipelined AllGathers (one per local 128-token tile), and attention
consumes the gathered key tiles in four availability batches, carrying the
softmax numerator and denominator across batches in per-head SBUF
accumulators.  The first quarter lands while the input projections are
still finishing, so the exchange is almost entirely off the critical path.

Weights are ternary-quantized on the host (exact same math as the
reference's _weight_quant) and shipped as bf16 {-1,0,1} in transposed
[in, out] layout, plus one packed row of fp32 scales/LN params.
Activations are fake-quantized on device; integer-valued operands are
exact in bf16, so the bf16 matmul path is exact for the quantized matmuls
(fp32 PSUM accumulation).

Quant statistics run on the Scalar engine (Square with accum_out gives
sum(x^2) per token in one pass) plus one DVE absmax reduce, so the Vector
engine stays off the critical path.  When the LayerNorm params are g=1,b=0
(true for this problem; checked on the host), LayerNorm + the following
BitLinear RMSNorm collapse into one affine normalize:
    rmsnorm(LN(x)) = (x - mean) * rsqrt(var*(1+1e-6) + 1e-11)
with absmax(x - mean) = max(max(x)-mean, mean-min(x)); the LN apply pass
and its stats pass disappear.  The per-head attention-output normalize ops
carry accum_out, so sum(x) per token is free.

Attention: q heads are host-permuted into pairs (0,2),(1,3),(4,6)... so a
q-pair shares one K tile pair; the two 64-contraction score matmuls run
CONCURRENTLY in the PE array as row-tiles (base partitions 0 and 64),
writing one 2-bank PSUM pair that a single Exp activation consumes.
Softmax denominators come free via a ones-column appended to V.
"""

import numpy as np
import ml_dtypes
from contextlib import ExitStack

import concourse.bacc as bacc
import concourse.mybir as mybir
import concourse.tile as tile
from concourse.bass_utils import run_bass_kernel_spmd
from concourse.masks import make_identity

F32 = mybir.dt.float32
BF16 = mybir.dt.bfloat16
AX = mybir.AxisListType
OP = mybir.AluOpType
ACT = mybir.ActivationFunctionType

# model dims
B, N, S, D = 2, 2048, 256, 768
HQ, HK, HEAD = 12, 6, 64
DKV = HEAD * HK          # 384
H4 = 4 * D               # 3072
NCORES = 8
GROUPS = [[0, 1, 2, 3], [4, 5, 6, 7]]
GSZ = 4                  # cores per batch group
T = (B * N) // NCORES    # 512 tokens per core
NT = T // 128            # 4 token tiles per core
ST = S // 128            # 2 condition token tiles
KT = D // 128            # 6 feature tiles of D
KTH = H4 // 128          # 24 feature tiles of 4D
KP = DKV // 128          # 3 kv-head-pair tiles

# q heads permuted so psum pair tile mt holds (QPERM[2mt], QPERM[2mt+1]),
# and both heads of a pair read the same gathered K pair tile.
QPERM = [0, 2, 1, 3, 4, 6, 5, 7, 8, 10, 9, 11]

# (out_features, in_features); device gets ternary bf16 f"{name}_q" [I, O].
WSPECS = {
    'sa_wq': (D, D), 'sa_wk': (DKV, D), 'sa_wv': (DKV, D), 'sa_wo': (D, D),
    'ca_wq': (D, D), 'ca_wk': (DKV, D), 'ca_wv': (DKV, D), 'ca_wo': (D, D),
    'w_cond': (D, D), 'w1': (H4, D), 'w2': (D, H4),
}
SCALE_SLOTS = list(WSPECS)          # order of m scales in the combo row
NSLOT = 16                          # padded scale slots
COMBO_W = NSLOT + 4 * D             # + sa_g, sa_b, ca_g, ca_b

_PROGRAM_CACHE = {}

MAGIC = 12582912.0   # 1.5 * 2^23: fp32 add/sub forces round-half-even to int

# exchange-quarter sizes (elements, bf16)
KSLICE = KP * 128 * 128   # K columns for one 128-token tile, all kp rows
VSLICE = 128 * DKV        # V for one 128-token tile
QSZ = KSLICE + VSLICE


class Ctx:
    pass


# ---------------------------------------------------------------------------
# quant statistics + per-token scale chains
# ---------------------------------------------------------------------------

def _rms_stats_tile(g, X, F, S2, amax, j):
    """Per-tile stats: S2[:, j] = sum(X^2) (Scalar engine Square with
    accum_out), amax[:, j] = max|X| (one DVE reduce).
    (tensor_tensor_reduce would do the square on the DVE, but it crashes
    this runtime -- verified with a minimal repro.)"""
    nc = g.nc
    sq = g.sq_scratch(F)
    nc.scalar.activation(sq, X, ACT.Square, accum_out=S2[:, j:j + 1])
    nc.vector.tensor_reduce(amax[:, j:j + 1], X, axis=AX.X, op=OP.max,
                            apply_absolute_value=True)


def _rms_chain(g, S2, amax, F, nj, uid):
    """al = absmax_n/127 (dequant alpha), srnd = 127*r/absmax_n where
    r = rsqrt(sum(x^2)/F + 1e-6), absmax_n = clip(absmax*r, 1e-5)."""
    nc, qpool = g.nc, g.qpool
    sd = qpool.tile([128, nj], F32, tag=f"qs_{uid}", name=f"qs_{uid}")
    nc.scalar.activation(sd, S2, ACT.Sqrt, bias=g.eps6, scale=1.0 / F)
    r = qpool.tile([128, nj], F32, tag=f"qr_{uid}", name=f"qr_{uid}")
    nc.vector.reciprocal(r, sd)
    amn = qpool.tile([128, nj], F32, tag=f"qm_{uid}", name=f"qm_{uid}")
    nc.vector.tensor_mul(amn, amax, r)
    nc.vector.tensor_scalar_max(amn, amn, 1e-5)
    al = qpool.tile([128, nj], F32, tag=f"al_{uid}", name=f"al_{uid}")
    nc.vector.tensor_scalar_mul(al, amn, 1.0 / 127.0)
    ra = qpool.tile([128, nj], F32, tag=f"qi_{uid}", name=f"qi_{uid}")
    nc.vector.reciprocal(ra, amn)
    srnd = qpool.tile([128, nj], F32, tag=f"qn_{uid}", name=f"qn_{uid}")
    nc.vector.tensor_mul(srnd, ra, r)
    nc.vector.tensor_scalar_mul(srnd, srnd, 127.0)
    return al, srnd


def _quant_tile(g, X, F, srnd_col, dst3, wk, dma_eng, pe_pool=None):
    """Quantize one token tile: round(x*srnd) via the fp32 magic-number
    trick (DVE mul+add, Act sub) -- integer-exact in bf16; then the
    feature-major transpose into dst3 [128, F//128, 128].  With pe_pool
    the transpose runs on the PE + Scalar copy instead of the XBAR DMA
    path (which is blocked while any collective -- including the
    framework's start-of-model barrier -- is in flight)."""
    nc = g.nc
    tmp = wk.tile([128, F], F32, tag=f"qt_{F}", name="qt", bufs=1)
    nc.vector.tensor_scalar(tmp, X, srnd_col, MAGIC, OP.mult, OP.add)
    xq = wk.tile([128, F], BF16, tag=f"xq_{F}", name="xq", bufs=2)
    nc.scalar.activation(xq, tmp, ACT.Copy, bias=-MAGIC)
    if pe_pool is not None:
        for k in range(F // 128):
            ps_t = pe_pool.tile([128, 128], BF16, tag="qpe", name="qpe")
            nc.tensor.transpose(ps_t, xq[:, k * 128:(k + 1) * 128],
                                g.identb)
            if dma_eng is nc.vector:
                nc.vector.tensor_copy(dst3[:, k, :], ps_t)
            else:
                nc.scalar.copy(dst3[:, k, :], ps_t)
    else:
        dma_eng.dma_start(dst3, xq, transpose=True)
    return xq


def _fused_ln_chain(g, S1, S2, mx, mn, nj, uid):
    """g=1,b=0 fast path: rmsnorm(LN(a)) == (a - m) * R with m = S1/D,
    var = S2/D - m^2, R = rsqrt(var*(1+1e-6) + 1e-11).
    absmax = max(mx - m, m - mn) * R.  Quantized int = (a*cq - dq) - MAGIC
    with cq = R*127/clip(absmax,1e-5), dq = m*cq - MAGIC.
    Returns (al, cq, dq); al is the dequant alpha."""
    nc, qpool = g.nc, g.qpool

    def t(nm):
        return qpool.tile([128, nj], F32, tag=f"{nm}_{uid}",
                          name=f"{nm}_{uid}")
    m = t("lm")
    nc.vector.tensor_scalar_mul(m, S1, 1.0 / D)
    t1 = t("lt")
    nc.vector.tensor_scalar_mul(t1, S2, 1.0 / D)
    msq = t("lq")
    nc.vector.tensor_mul(msq, m, m)
    var = t("lv")
    nc.vector.tensor_sub(var, t1, msq)
    dd = t("ld")
    nc.vector.tensor_scalar(dd, var, 1.0 + 1e-6, 1e-11, OP.mult, OP.add)
    sd = t("ls")
    nc.scalar.activation(sd, dd, ACT.Sqrt)
    R = t("lr")
    nc.vector.reciprocal(R, sd)
    t3 = t("l3")
    nc.vector.tensor_sub(t3, mx, m)
    t4 = t("l4")
    nc.vector.tensor_sub(t4, m, mn)
    am = t("la")
    nc.vector.tensor_tensor(am, t3, t4, op=OP.max)
    amn = t("ln")
    nc.vector.tensor_mul(amn, am, R)
    nc.vector.tensor_scalar_max(amn, amn, 1e-5)
    al = t("al")
    nc.vector.tensor_scalar_mul(al, amn, 1.0 / 127.0)
    ra = t("li")
    nc.vector.reciprocal(ra, amn)
    cq = t("lc")
    nc.vector.tensor_mul(cq, ra, R)
    nc.vector.tensor_scalar_mul(cq, cq, 127.0)
    dq = t("lz")
    nc.vector.tensor_mul(dq, m, cq)
    nc.vector.tensor_scalar(dq, dq, MAGIC, None, OP.subtract)
    return al, cq, dq


def _fused_quant_tile(g, X, cq_col, dq_col, dst3, wk, dma_eng):
    """Quantize one fused-LN tile: (X*cq - dq) - MAGIC, then transpose."""
    nc = g.nc
    tmp = wk.tile([128, D], F32, tag="qt_768", name="qt", bufs=1)
    nc.vector.tensor_scalar(tmp, X, cq_col, dq_col, OP.mult, OP.subtract)
    xq = wk.tile([128, D], BF16, tag="xq_768", name="xq", bufs=2)
    nc.scalar.activation(xq, tmp, ACT.Copy, bias=-MAGIC)
    dma_eng.dma_start(dst3, xq, transpose=True)
    return xq


def _layernorm(g, a_tiles, g_bc, b_bc, out_tiles, uid):
    """General-g/b LayerNorm (fallback path)."""
    nc, qpool = g.nc, g.qpool
    nj = len(a_tiles)
    s1 = qpool.tile([128, nj], F32, tag=f"ls1_{uid}", name=f"ls1_{uid}")
    s2 = qpool.tile([128, nj], F32, tag=f"ls2_{uid}", name=f"ls2_{uid}")
    for j, A in enumerate(a_tiles):
        sq = g.sq_scratch(D)
        nc.scalar.activation(sq, A, ACT.Square, accum_out=s2[:, j:j + 1])
        nc.vector.tensor_reduce(s1[:, j:j + 1], A, axis=AX.X, op=OP.add)
    m = qpool.tile([128, nj], F32, tag=f"lmu_{uid}", name=f"lmu_{uid}")
    nc.vector.tensor_scalar_mul(m, s1, 1.0 / D)
    t1 = qpool.tile([128, nj], F32, tag=f"lt1_{uid}", name=f"lt1_{uid}")
    nc.vector.tensor_scalar_mul(t1, s2, 1.0 / D)
    msq = qpool.tile([128, nj], F32, tag=f"lms_{uid}", name=f"lms_{uid}")
    nc.vector.tensor_mul(msq, m, m)
    var = qpool.tile([128, nj], F32, tag=f"lva_{uid}", name=f"lva_{uid}")
    nc.vector.tensor_sub(var, t1, msq)
    sd = qpool.tile([128, nj], F32, tag=f"lsd_{uid}", name=f"lsd_{uid}")
    nc.scalar.activation(sd, var, ACT.Sqrt, bias=g.eps5)
    rs = qpool.tile([128, nj], F32, tag=f"lrs_{uid}", name=f"lrs_{uid}")
    nc.vector.reciprocal(rs, sd)
    for j, A in enumerate(a_tiles):
        X = out_tiles[j]
        nc.vector.tensor_scalar(X, A, m[:, j:j + 1], rs[:, j:j + 1],
                                OP.subtract, OP.mult)
        nc.vector.tensor_mul(X, X, g_bc)
        nc.vector.tensor_add(X, X, b_bc)


def _make_abc(g, al_mat, nj, Ttot, pool, uid):
    """Row-broadcast of per-token alpha: [128, nj] -> [128, Ttot], done
    entirely on the PE (transpose, then a rank-1 ones matmul per 128-token
    block) so it never queues behind gpsimd weight-prefetch DMAs."""
    nc = g.nc
    abc = pool.tile([128, Ttot], F32, tag=f"abc_{uid}", name=f"abc_{uid}")
    with g.tc.tile_pool(name=f"psabc_{uid}", bufs=1, space="PSUM") as pp:
        pst = pp.tile([nj, 128], F32, tag="ps_abc", name="pst")
        nc.tensor.transpose(pst, al_mat, g.ident)
        at = g.stat.tile([nj, 128], F32, tag="at", name="at", bufs=1)
        nc.scalar.copy(at, pst)
        arow = g.stat.tile([1, Ttot], F32, tag="arow", name="arow", bufs=1)
        for j in range(nj):
            nc.sync.dma_start(arow[0:1, j * 128:(j + 1) * 128],
                              at[j:j + 1, :])
        psb = pp.tile([128, Ttot], F32, tag="ps_abc2", name="psb")
        nc.tensor.matmul(psb, g.ones1, arow[0:1, :], start=True, stop=True)
        nc.vector.tensor_copy(abc, psb)
    return abc


# ---------------------------------------------------------------------------
# attention
# ---------------------------------------------------------------------------

def _attn_norm(g, h, o_sb, a_out, s1cols, psum_t, post_j=None):
    """Per-head transpose + softmax normalize; accum_out gives the
    per-token feature sum of the normalized head chunk for free.
    post_j fires after tile j's chunk is written (used by the final head
    to pipeline the downstream LayerNorm stats into the attention tail)."""
    nc = g.nc
    for j in range(NT):
        ps_t = psum_t.tile([128, 65], F32, tag="pst", name="ps_t")
        nc.tensor.transpose(ps_t, o_sb[:, j * 128:(j + 1) * 128],
                            g.ident[0:65, 0:65])
        rec = g.stat.tile([128, 1], F32, tag="rec", name="rec")
        nc.vector.reciprocal(rec, ps_t[:, 64:65])
        acc = s1cols[j][:, h:h + 1] if s1cols is not None else None
        nc.vector.tensor_scalar(a_out[j][:, h * 64:(h + 1) * 64],
                                ps_t[:, 0:64], rec, 0.0, OP.mult, OP.add,
                                accum_out=acc)
        if post_j is not None:
            post_j(j)


def _attention(g, batches, k_views, v_views, q_lo, q_hi, a_out, s1cols,
               psum_s, psum_o, psum_t, awork, acc_pool, tail_cb=None):
    """Batched paired GQA attention.  batches: list of lists of s-tile
    indices in availability order.  The first batch seeds per-head SBUF
    accumulators, middle batches add into them, the last merges and emits
    transposes + normalize.  Single-batch callers skip the accumulators.

    k_views[kp][s]: [128, 128] bf16 (k-heads 2kp/2kp+1 row-tiled);
    v_views[s]: [128, HK, HEAD+1] bf16 (ones column -> denominator)."""
    nc = g.nc
    nb = len(batches)
    accA = accB = None
    if nb > 1:
        accA = acc_pool.tile([65, HQ // 2, 512], BF16, name="accA")
        accB = acc_pool.tile([65, HQ // 2, 512], BF16, name="accB")
    for b, batch in enumerate(batches):
        first, last = b == 0, b == nb - 1
        for qp in range(HQ // 2):
            hA, hB = QPERM[2 * qp], QPERM[2 * qp + 1]
            khA, khB = hA // 2, hB // 2
            kp = khA // 2
            ps_oA = psum_o.tile([65, 512], F32, tag="pvA", name="pvA")
            ps_oB = psum_o.tile([65, 512], F32, tag="pvB", name="pvB")
            for i, s in enumerate(batch):
                ps_pair = psum_s.tile([128, 1024], F32, tag="pss",
                                      name="pss")
                ps_A, ps_B = ps_pair[:, 0:512], ps_pair[:, 512:1024]
                nc.tensor.matmul(ps_A, k_views[kp][s][0:64, :], q_lo[qp],
                                 start=True, stop=True)
                nc.tensor.matmul(ps_B, k_views[kp][s][64:128, :], q_hi[qp],
                                 start=True, stop=True)
                pT = awork.tile([128, 1024], BF16, tag="pT", name="pT",
                                bufs=2)
                nc.scalar.activation(pT, ps_pair, ACT.Exp)
                nc.tensor.matmul(ps_oA, v_views[s][:, khA, :],
                                 pT[:, 0:512], start=(i == 0),
                                 stop=(i == len(batch) - 1))
                nc.tensor.matmul(ps_oB, v_views[s][:, khB, :],
                                 pT[:, 512:1024], start=(i == 0),
                                 stop=(i == len(batch) - 1))
            last_qp = qp == HQ // 2 - 1
            if nb == 1:
                for h, ps_o in ((hA, ps_oA), (hB, ps_oB)):
                    o_sb = awork.tile([65, 512], F32, tag="osb",
                                      name="osb", bufs=2)
                    nc.vector.tensor_copy(o_sb, ps_o)
                    _attn_norm(g, h, o_sb, a_out, s1cols, psum_t,
                               post_j=(tail_cb if last_qp and h == hB
                                       else None))
            elif first:
                nc.vector.tensor_copy(accA[:, qp, :], ps_oA)
                nc.vector.tensor_copy(accB[:, qp, :], ps_oB)
            elif not last:
                nc.vector.tensor_add(accA[:, qp, :], accA[:, qp, :], ps_oA)
                nc.vector.tensor_add(accB[:, qp, :], accB[:, qp, :], ps_oB)
            else:
                for h, ps_o, acc in ((hA, ps_oA, accA), (hB, ps_oB, accB)):
                    o_sb = awork.tile([65, 512], F32, tag="osb",
                                      name="osb", bufs=2)
                    nc.vector.tensor_add(o_sb, acc[:, qp, :], ps_o)
                    _attn_norm(g, h, o_sb, a_out, s1cols, psum_t,
                               post_j=(tail_cb if last_qp and h == hB
                                       else None))


# ---------------------------------------------------------------------------
# program builder
# ---------------------------------------------------------------------------

def build_program(groups=None, fused_ln=True):
    if groups is None:
        groups = GROUPS
    gsz = len(groups[0])
    nc = bacc.Bacc()

    x_in = nc.declare_dram_parameter("x_sh", [T, D], F32, isOutput=False)
    y_in = nc.declare_dram_parameter("y_b", [S, D], F32, isOutput=False)
    wt_in = {}
    for name, (O, I) in WSPECS.items():
        wt_in[name] = nc.declare_dram_parameter(f"{name}_q", [I, O], BF16,
                                                isOutput=False)
    combo_in = nc.declare_dram_parameter("combo", [1, COMBO_W], F32,
                                         isOutput=False)
    out_sh = nc.declare_dram_parameter("out_sh", [T, D], F32, isOutput=True)

    g = Ctx()
    g.nc = nc

    with tile.TileContext(nc) as tc, ExitStack() as ctx:
        g.tc = tc
        g.const = ctx.enter_context(tc.tile_pool(name="const", bufs=1))
        g.stat = ctx.enter_context(tc.tile_pool(name="stat", bufs=4))
        g.work = ctx.enter_context(tc.tile_pool(name="work", bufs=2))
        g.qpool = ctx.enter_context(tc.tile_pool(name="qpool", bufs=1))
        sqpool = ctx.enter_context(tc.tile_pool(name="sqpool", bufs=1))
        dram = ctx.enter_context(tc.tile_pool(name="dram", bufs=1,
                                              space="DRAM"))

        def sq_scratch(F):
            return sqpool.tile([128, F], BF16, tag=f"sq_{F}", name="sq")
        g.sq_scratch = sq_scratch

        # four quarter-exchanges, one per local 128-token tile
        cc_in = [dram.tile([QSZ], BF16, name=f"cc_in{i}") for i in range(NT)]
        cc_out = [dram.tile([gsz, QSZ], BF16, name=f"cc_out{i}")
                  for i in range(NT)]

        g.eps6 = g.const.tile([128, 1], F32, name="eps6")
        nc.vector.memset(g.eps6, 1e-6)
        g.eps5 = g.const.tile([128, 1], F32, name="eps5")
        nc.vector.memset(g.eps5, 1e-5)
        g.ident = g.const.tile([128, 128], F32, name="ident")
        make_identity(nc, g.ident)
        identb = g.const.tile([128, 128], BF16, name="identb")
        nc.vector.tensor_copy(identb, g.ident)
        g.identb = identb
        g.ones1 = g.const.tile([1, 128], F32, name="ones1")
        nc.vector.memset(g.ones1, 1.0)

        # one DMA + partition broadcasts for the scales (+ LN params only
        # in the general-g/b fallback path)
        cbw = NSLOT if fused_ln else COMBO_W
        cb = g.const.tile([128, cbw], F32, name="cb")
        with tc.tile_pool(name="crowp", bufs=1) as crowp:
            crow = crowp.tile([1, COMBO_W], F32, name="crow")
            nc.scalar.dma_start(crow, combo_in[:, :])
            nc.gpsimd.partition_broadcast(cb[:, 0:NSLOT], crow[0:1, 0:NSLOT])
            if not fused_ln:
                for i in range(4):
                    sl = slice(NSLOT + i * D, NSLOT + (i + 1) * D)
                    nc.gpsimd.partition_broadcast(cb[:, sl], crow[0:1, sl])
        msc = {name: cb[:, i:i + 1] for i, name in enumerate(SCALE_SLOTS)}
        ln_bc = ({} if fused_ln else
                 {name: cb[:, NSLOT + i * D:NSLOT + (i + 1) * D]
                  for i, name in enumerate(('sa_g', 'sa_b',
                                            'ca_g', 'ca_b'))})

        g.ka_pool = None

        def keepalive(ap, n):
            """Dummy matmuls reading `ap` (bf16, <=512 cols) to hold the PE
            p-state up through otherwise PE-idle stretches."""
            if g.ka_pool is None:
                return
            for _ in range(n):
                ps = g.ka_pool.tile([128, 512], F32, tag="ka", name="ka")
                nc.tensor.matmul(ps, identb, ap, start=True, stop=True)
        g.keepalive = keepalive

        def load_weight(pool, name, eng):
            O, I = WSPECS[name]
            rows = I // 128
            wt = pool.tile([128, rows, O], BF16, tag=f"w_{name}",
                           name=f"w_{name}")
            for r in range(rows):
                eng.dma_start(wt[:, r, :],
                              wt_in[name][r * 128:(r + 1) * 128, :])
            return wt

        def proj_fm(wsb, xqT_all, mscale, abc, O, Ttot, pool, tag, ps_pool):
            """feature-major projection: O//128 tiles [128, Ttot] bf16."""
            nk = xqT_all.shape[1]
            outs = []
            for mt in range(O // 128):
                ps = ps_pool.tile([128, Ttot], F32, tag="ps", name="ps_pf")
                for k in range(nk):
                    nc.tensor.matmul(ps, wsb[:, k, mt * 128:(mt + 1) * 128],
                                     xqT_all[:, k, :], start=(k == 0),
                                     stop=(k == nk - 1))
                o = pool.tile([128, Ttot], BF16, tag=f"{tag}{mt}",
                              name=f"{tag}{mt}")
                nc.vector.scalar_tensor_tensor(o, ps, mscale, abc,
                                               OP.mult, OP.mult)
                outs.append(o)
            return outs

        def proj_tok_resid(xq_j, wsb, al_mat, mscale, resid_tiles,
                           out_tiles, ps_pool, nk=KT, pre=None, post=None):
            """token-major projection + dequant + residual add.  The pre
            hook (just-in-time quantize) runs SKEWED two tiles ahead of
            the matmuls, so the in-order PE queue never stalls on a
            quant chain; post(j) emits the produced tile's stats."""
            def mm(j):
                xqj = xq_j(j)
                ao = g.stat.tile([128, 1], F32, tag="ao", name="ao")
                nc.vector.tensor_mul(ao, al_mat[:, j:j + 1], mscale)
                for c in range(2):
                    ps = ps_pool.tile([128, 384], F32, tag="ps",
                                      name="ps_pt")
                    for k in range(nk):
                        nc.tensor.matmul(
                            ps, xqj[:, k, :],
                            wsb[:, k, c * 384:(c + 1) * 384],
                            start=(k == 0), stop=(k == nk - 1))
                    nc.vector.scalar_tensor_tensor(
                        out_tiles[j][:, c * 384:(c + 1) * 384], ps, ao,
                        resid_tiles[j][:, c * 384:(c + 1) * 384],
                        OP.mult, OP.add)
                if post is not None:
                    post(j)
            if pre is None:
                for j in range(NT):
                    mm(j)
            else:
                for j in range(NT + 2):
                    if j < NT:
                        pre(j)
                    if j >= 2:
                        mm(j - 2)

        # ---- LN1 (+fused rms) + quant + wo projection + residual ----
        def make_ln_stats(uid):
            """Stats tiles + a tail callback that fills them per tile as
            the final attention head lands (fused path only)."""
            S2m = g.qpool.tile([128, NT], F32, tag=f"wS2_{uid}",
                               name=f"wS2_{uid}")
            if not fused_ln:
                return (S2m, None, None, None), None, None

            S1m = g.qpool.tile([128, NT], F32, tag=f"wS1_{uid}",
                               name=f"wS1_{uid}")
            mx = g.qpool.tile([128, NT], F32, tag=f"wmx_{uid}",
                              name=f"wmx_{uid}")
            mn = g.qpool.tile([128, NT], F32, tag=f"wmn_{uid}",
                              name=f"wmn_{uid}")
            holder = {}

            def tail_cb(j):
                a_tok, s1cols = holder['a_tok'], holder['s1cols']
                sq = g.sq_scratch(D)
                nc.scalar.activation(sq, a_tok[j], ACT.Square,
                                     accum_out=S2m[:, j:j + 1])
                nc.vector.tensor_reduce(S1m[:, j:j + 1], s1cols[j],
                                        axis=AX.X, op=OP.add)
                nc.vector.tensor_reduce(mx[:, j:j + 1], a_tok[j],
                                        axis=AX.X, op=OP.max)
                nc.vector.tensor_reduce(mn[:, j:j + 1], a_tok[j],
                                        axis=AX.X, op=OP.min)
            return (S2m, S1m, mx, mn), tail_cb, holder

        def wo_block(a_tok, s1cols, gname, bname, woname, resid, out_tiles,
                     aqT, post, uid, stats):
            es_ka = ExitStack()
            g.ka_pool = es_ka.enter_context(
                tc.tile_pool(name=f"ka_{uid}", bufs=1, space="PSUM"))
            S2m, S1m, mx, mn = stats
            if fused_ln:
                al, cq, dq = _fused_ln_chain(g, S1m, S2m, mx, mn, NT, uid)

                def pre(j):
                    _fused_quant_tile(
                        g, a_tok[j], cq[:, j:j + 1], dq[:, j:j + 1],
                        aqT[:, :, j * 128:(j + 1) * 128], g.work, nc.sync)
            else:
                ln_t = a_tok
                _layernorm(g, a_tok, ln_bc[gname], ln_bc[bname], ln_t, uid)
                amax = g.qpool.tile([128, NT], F32, tag=f"wam_{uid}",
                                    name=f"wam_{uid}")
                for j in range(NT):
                    _rms_stats_tile(g, ln_t[j], D, S2m, amax, j)
                al, srnd = _rms_chain(g, S2m, amax, D, NT, uid)

                def pre(j):
                    _quant_tile(g, ln_t[j], D, srnd[:, j:j + 1],
                                aqT[:, :, j * 128:(j + 1) * 128],
                                g.work, nc.sync)
            with tc.tile_pool(name=f"pswo_{uid}", bufs=3,
                              space="PSUM") as pswo:
                proj_tok_resid(
                    lambda j: aqT[:, :, j * 128:(j + 1) * 128],
                    g.w[woname], al, msc[woname], resid, out_tiles,
                    pswo, pre=pre, post=post)
            g.ka_pool = None
            es_ka.close()


        # ------------------------------------------------------------------
        # scoped pools
        # ------------------------------------------------------------------
        es_wsa = ExitStack()
        es_wca = ExitStack()
        es_x = ExitStack()
        es_x2 = ExitStack()
        es_sa = ExitStack()
        es_cond = ExitStack()
        es_ffnw = ExitStack()
        es_saq = ExitStack()

        resid3 = ctx.enter_context(tc.tile_pool(name="resid3", bufs=1))
        x3_all = resid3.tile([128, NT, D], F32, name="x3_all")
        x3 = [x3_all[:, j, :] for j in range(NT)]
        x2pool = es_x2.enter_context(tc.tile_pool(name="x2pool", bufs=1,
                                                  side="right"))
        x2_all = x2pool.tile([128, NT, D], F32, name="x2_all")
        x2 = [x2_all[:, j, :] for j in range(NT)]

        # x first on the sync queue, then its stats/quant compute ops are
        # emitted BEFORE any weight-row DMA lands on a compute-engine
        # queue: HBM bandwidth is saturated during startup, so a weight
        # DMA ahead of the stats ops would stall them ~20us.
        with_wsa = es_wsa.enter_context(tc.tile_pool(name="w_sa", bufs=1))
        with_wca = es_wca.enter_context(tc.tile_pool(name="w_ca", bufs=1,
                                                     side="right"))
        xpool = es_x.enter_context(tc.tile_pool(name="xpool", bufs=1))
        x_all = xpool.tile([128, NT, D], F32, name="x_all")
        for j in range(NT):
            nc.sync.dma_start(x_all[:, j, :], x_in[j * 128:(j + 1) * 128, :])
        x_tiles = [x_all[:, j, :] for j in range(NT)]

        # K/V/Q weight rows immediately behind x on the sync queue: the
        # bandwidth window while the stats run is otherwise free (DMA
        # transposes of the quant tiles only start ~30us in).
        g.w = {}
        g.w['sa_wk'] = load_weight(with_wsa, 'sa_wk', nc.sync)
        g.w['sa_wv'] = load_weight(with_wsa, 'sa_wv', nc.sync)
        g.w['sa_wq'] = load_weight(with_wsa, 'sa_wq', nc.sync)

        sa_act = es_sa.enter_context(tc.tile_pool(name="sa_act", bufs=1))
        sa_xq = es_saq.enter_context(tc.tile_pool(name="sa_xq", bufs=1))

        # ---- SA input quant ----
        x1qT = sa_xq.tile([128, KT, T], BF16, name="x1qT")
        S2x = g.qpool.tile([128, NT], F32, tag="S2x1", name="S2x1")
        amx1 = g.qpool.tile([128, NT], F32, tag="amx1", name="amx1")
        for j in range(NT):
            _rms_stats_tile(g, x_tiles[j], D, S2x, amx1, j)
        al_x, srnd_x = _rms_chain(g, S2x, amx1, D, NT, "x1")
        first_xq = None
        with tc.tile_pool(name="ps_qpe", bufs=4, space="PSUM") as qpe:
            for j in range(NT):
                xq = _quant_tile(g, x_tiles[j], D, srnd_x[:, j:j + 1],
                                 x1qT[:, :, j * 128:(j + 1) * 128], g.work,
                                 nc.sync, pe_pool=qpe)
                if first_xq is None:
                    first_xq = xq
        abc_x = _make_abc(g, al_x, NT, T, sa_xq, "x1")

        # HAM warm-up: dense burst reading the first quant tile ramps the
        # PE clock while the remaining quant tiles stream.
        with tc.tile_pool(name="ps_warm0", bufs=1, space="PSUM") as psw:
            wps = psw.tile([128, 512], F32, tag="warm0", name="warm0")
            for _ in range(16):
                nc.tensor.matmul(wps, identb, first_xq[:, 0:512],
                                 start=True, stop=True)

        # ---- K, V projections; fire the four quarter-gathers; then Q ----
        with tc.tile_pool(name="ps_proj", bufs=2, space="PSUM") as psp:
            kf = proj_fm(g.w['sa_wk'], x1qT, msc['sa_wk'], abc_x, DKV, T,
                         sa_xq, "kf", psp)
            for j in range(NT):
                for t in range(KP):
                    dst = cc_in[j][t * 128 * 128:(t + 1) * 128 * 128]
                    nc.sync.dma_start(
                        dst.rearrange("(p t) -> p t", p=128),
                        kf[t][:, j * 128:(j + 1) * 128])
            for j in range(NT):
                ps = psp.tile([128, DKV], F32, tag="psv", name="ps_v")
                for k in range(KT):
                    nc.tensor.matmul(ps, x1qT[:, k, j * 128:(j + 1) * 128],
                                     g.w['sa_wv'][:, k, :], start=(k == 0),
                                     stop=(k == KT - 1))
                av = g.stat.tile([128, 1], F32, tag="av", name="av")
                nc.vector.tensor_mul(av, al_x[:, j:j + 1], msc['sa_wv'])
                vtok = g.work.tile([128, DKV], BF16, tag="vtok",
                                   name="vtok")
                nc.vector.tensor_scalar_mul(vtok, ps, av)
                nc.sync.dma_start(
                    cc_in[j][KSLICE:KSLICE + VSLICE].rearrange(
                        "(p f) -> p f", p=128), vtok)
                nc.gpsimd.collective_compute(
                    "AllGather", OP.bypass, replica_groups=groups,
                    ins=[cc_in[j][:].opt()],
                    outs=[cc_out[j][:, :].opt()])

            # deferred weight prefetch: the gpsimd SWDGE queue is blocked
            # by the gather triggers above until the K/V writes land, so
            # these streams start only once the startup crunch is over.
            for k in ('w_cond', 'ca_wk', 'ca_wv'):
                g.w[k] = load_weight(with_wca, k, nc.gpsimd)
            g.w['sa_wo'] = load_weight(with_wsa, 'sa_wo', nc.gpsimd)

            qpairs = proj_fm(g.w['sa_wq'], x1qT, msc['sa_wq'], abc_x, D, T,
                             sa_act, "qp", psp)
            es_saq.close()

            # ---- CA condition-side work (independent of x; overlaps the
            # gathers).  All its DMAs go on the scalar queue so they can
            # never sit behind a gather-dependent wait. ----
            ca_cond = es_cond.enter_context(tc.tile_pool(name="ca_cond",
                                                         bufs=1,
                                                         side="right"))
            with tc.tile_pool(name="ysc", bufs=1) as ysc:
                y_all = ysc.tile([128, ST, D], F32, name="y_all")
                for j in range(ST):
                    nc.scalar.dma_start(y_all[:, j, :],
                                        y_in[j * 128:(j + 1) * 128, :])
                y_tiles = [y_all[:, j, :] for j in range(ST)]
                yqT = ysc.tile([128, KT, S], BF16, name="yqT")
                S2y = g.qpool.tile([128, ST], F32, tag="S2y", name="S2y")
                amy = g.qpool.tile([128, ST], F32, tag="amy", name="amy")
                for j in range(ST):
                    _rms_stats_tile(g, y_tiles[j], D, S2y, amy, j)
                al_y, srnd_y = _rms_chain(g, S2y, amy, D, ST, "y")
                with tc.tile_pool(name="ps_qpy", bufs=2,
                                  space="PSUM") as qpy:
                    for j in range(ST):
                        _quant_tile(g, y_tiles[j], D, srnd_y[:, j:j + 1],
                                    yqT[:, :, j * 128:(j + 1) * 128],
                                    g.work, nc.vector, pe_pool=qpy)
                yc_all = ysc.tile([128, ST, D], F32, name="yc_all")
                yc = [yc_all[:, j, :] for j in range(ST)]
                for j in range(ST):
                    am = g.stat.tile([128, 1], F32, tag="am", name="am")
                    nc.vector.tensor_mul(am, al_y[:, j:j + 1],
                                         msc['w_cond'])
                    for c in range(2):
                        ps = psp.tile([128, 384], F32, tag="psy",
                                      name="ps_yc")
                        for k in range(KT):
                            nc.tensor.matmul(
                                ps, yqT[:, k, j * 128:(j + 1) * 128],
                                g.w['w_cond'][:, k, c * 384:(c + 1) * 384],
                                start=(k == 0), stop=(k == KT - 1))
                        nc.vector.tensor_scalar_mul(
                            yc[j][:, c * 384:(c + 1) * 384], ps, am)

                ycqT = ysc.tile([128, KT, S], BF16, name="ycqT")
                S2c = g.qpool.tile([128, ST], F32, tag="S2c", name="S2c")
                amc = g.qpool.tile([128, ST], F32, tag="amc", name="amc")
                for j in range(ST):
                    _rms_stats_tile(g, yc[j], D, S2c, amc, j)
                al_yc, srnd_yc = _rms_chain(g, S2c, amc, D, ST, "yc")
                with tc.tile_pool(name="ps_qpc", bufs=2,
                                  space="PSUM") as qpc:
                    for j in range(ST):
                        _quant_tile(g, yc[j], D, srnd_yc[:, j:j + 1],
                                    ycqT[:, :, j * 128:(j + 1) * 128],
                                    g.work, nc.vector, pe_pool=qpc)
                abc_yc = _make_abc(g, al_yc, ST, S, ysc, "yc")

                ca_kpairs = proj_fm(g.w['ca_wk'], ycqT, msc['ca_wk'],
                                    abc_yc, DKV, S, ca_cond, "ck", psp)
                v_ca = []
                for j in range(ST):
                    ps = psp.tile([128, DKV], F32, tag="psv", name="ps_vc")
                    for k in range(KT):
                        nc.tensor.matmul(
                            ps, ycqT[:, k, j * 128:(j + 1) * 128],
                            g.w['ca_wv'][:, k, :], start=(k == 0),
                            stop=(k == KT - 1))
                    av = g.stat.tile([128, 1], F32, tag="av", name="avc")
                    nc.vector.tensor_mul(av, al_yc[:, j:j + 1],
                                         msc['ca_wv'])
                    va = ca_cond.tile([128, HK, HEAD + 1], BF16,
                                      tag=f"vc{j}", name=f"vc{j}")
                    nc.vector.tensor_scalar_mul(
                        va[:, :, 0:HEAD],
                        ps.rearrange("p (h e) -> p h e", e=HEAD), av)
                    nc.vector.memset(va[:, :, HEAD:HEAD + 1], 1.0)
                    v_ca.append(va)

        # ---- gathered K/V tiles; s-tile index = quarter j * gsz + slot ----
        # (pool opened only now, after ysc closed, so the cond-side scratch
        # and the gathered K/V never coexist in SBUF)
        sa_kv = es_sa.enter_context(tc.tile_pool(name="sa_kv", bufs=1))
        kt_g = []
        for kp in range(KP):
            kt = sa_kv.tile([128, NT * gsz, 128], BF16, tag=f"kT{kp}",
                            name=f"kT{kp}")
            kt_g.append(kt)
        v_aug = []
        for s in range(NT * gsz):
            va = sa_kv.tile([128, HK, HEAD + 1], BF16, tag=f"va{s}",
                            name=f"va{s}")
            nc.vector.memset(va[:, :, HEAD:HEAD + 1], 1.0)
            v_aug.append(va)
        for j in range(NT):
            for kp in range(KP):
                src = cc_out[j][:, kp * 128 * 128:(kp + 1) * 128 * 128]
                nc.sync.dma_start(
                    kt_g[kp][:, j * gsz:(j + 1) * gsz, :],
                    src.rearrange("r (p t) -> p r t", p=128))
            for r in range(gsz):
                s = j * gsz + r
                src = cc_out[j][r, KSLICE:KSLICE + VSLICE]
                nc.sync.dma_start(
                    v_aug[s][:, :, 0:HEAD],
                    src.rearrange("(p h e) -> p h e", p=128, e=HEAD))
        k_views = [[kt_g[kp][:, s, :] for s in range(NT * gsz)]
                   for kp in range(KP)]

        # a second HAM warm-up right before attention
        with tc.tile_pool(name="ps_warm1", bufs=1, space="PSUM") as psw:
            wps = psw.tile([128, 512], F32, tag="warm1", name="warm1")
            for _ in range(12):
                nc.tensor.matmul(wps, identb, qpairs[0][:, 0:512],
                                 start=True, stop=True)

        q_lo = [qt[0:64, :] for qt in qpairs]
        q_hi = [qt[64:128, :] for qt in qpairs]

        a_pool = es_sa.enter_context(tc.tile_pool(name="a_pool", bufs=1))
        a_all = a_pool.tile([128, NT, D], F32, name="a_all")
        a_tok = [a_all[:, j, :] for j in range(NT)]
        s1c = None
        if fused_ln:
            s1c = [a_pool.tile([128, HQ], F32, tag=f"s1c{j}",
                               name=f"s1c{j}") for j in range(NT)]
        stats_l1, tail_l1, hold_l1 = make_ln_stats("l1")
        if hold_l1 is not None:
            hold_l1['a_tok'], hold_l1['s1cols'] = a_tok, s1c
        batches = [[j * gsz + r for r in range(gsz)] for j in range(NT)]
        with tc.tile_pool(name="awork", bufs=1) as awork, \
             tc.tile_pool(name="ps_s", bufs=2, space="PSUM") as psum_s, \
             tc.tile_pool(name="ps_o", bufs=1, space="PSUM") as psum_o, \
             tc.tile_pool(name="ps_t", bufs=2, space="PSUM") as psum_t:
            _attention(g, batches, k_views, v_aug, q_lo, q_hi, a_tok, s1c,
                       psum_s, psum_o, psum_t, awork, a_pool,
                       tail_cb=tail_l1)

        # x2 stats + per-tile chain + quant pipeline into the wo
        # projection, so the q2 projection can start right after wo(3)
        S2x2 = g.qpool.tile([128, NT], F32, tag="S2x2", name="S2x2")
        amx2 = g.qpool.tile([128, NT], F32, tag="amx2", name="amx2")
        al_x2m = g.qpool.tile([128, NT], F32, tag="alx2m", name="alx2m")
        x2qT = x2pool.tile([128, KT, T], BF16, name="x2qT")

        def post_x2(j):
            _rms_stats_tile(g, x2[j], D, S2x2, amx2, j)
            al_j, srnd_j = _rms_chain(g, S2x2[:, j:j + 1],
                                      amx2[:, j:j + 1], D, 1, f"x2{j}")
            nc.vector.tensor_copy(al_x2m[:, j:j + 1], al_j)
            _quant_tile(g, x2[j], D, srnd_j[:, 0:1],
                        x2qT[:, :, j * 128:(j + 1) * 128], g.work, nc.sync)

        a1qT = es_sa.enter_context(
            tc.tile_pool(name="a1qT", bufs=1)).tile(
                [128, KT, T], BF16, name="a1qT")
        wo_block(a_tok, s1c, 'sa_g', 'sa_b', 'sa_wo', x_tiles, x2, a1qT,
                 post_x2, "l1", stats_l1)
        es_sa.close()
        es_x.close()
        es_wsa.close()

        ffn_w = es_ffnw.enter_context(tc.tile_pool(name="ffn_w", bufs=1))

        # ---- CA ----
        with tc.tile_pool(name="ca_act", bufs=1) as ca_act, \
             tc.tile_pool(name="awork2", bufs=1) as awork:
            abc_x2 = _make_abc(g, al_x2m, NT, T, ca_act, "x2")
            # weight prefetch emitted only now, so the DMA burst overlaps
            # the q2 projection + CA attention instead of the x2 transposes
            g.w['ca_wq'] = load_weight(with_wca, 'ca_wq', nc.sync)
            g.w['ca_wo'] = load_weight(with_wca, 'ca_wo', nc.gpsimd)
            g.w['w1'] = load_weight(ffn_w, 'w1', nc.gpsimd)
            with tc.tile_pool(name="ps_q2", bufs=3, space="PSUM") as psq:
                q2pairs = proj_fm(g.w['ca_wq'], x2qT, msc['ca_wq'], abc_x2,
                                  D, T, ca_act, "q2", psq)

            q2_lo = [qt[0:64, :] for qt in q2pairs]
            q2_hi = [qt[64:128, :] for qt in q2pairs]
            ck_views = [[ca_kpairs[kp][:, s * 128:(s + 1) * 128]
                         for s in range(ST)] for kp in range(KP)]

            # keep the PE warm into the short CA attention phase
            with tc.tile_pool(name="ps_warm2", bufs=1, space="PSUM") as psw:
                wps = psw.tile([128, 512], F32, tag="warm2", name="warm2")
                for _ in range(10):
                    nc.tensor.matmul(wps, identb, q2pairs[0][:, 0:512],
                                     start=True, stop=True)

            a2_all = ca_act.tile([128, NT, D], F32, name="a2_all")
            a2_tok = [a2_all[:, j, :] for j in range(NT)]
            s2c = None
            if fused_ln:
                s2c = [ca_act.tile([128, HQ], F32, tag=f"s2c{j}",
                                   name=f"s2c{j}") for j in range(NT)]
            stats_l2, tail_l2, hold_l2 = make_ln_stats("l2")
            if hold_l2 is not None:
                hold_l2['a_tok'], hold_l2['s1cols'] = a2_tok, s2c
            with tc.tile_pool(name="ps_s2", bufs=2, space="PSUM") as psum_s, \
                 tc.tile_pool(name="ps_o2", bufs=1, space="PSUM") as psum_o, \
                 tc.tile_pool(name="ps_t2", bufs=2, space="PSUM") as psum_t:
                _attention(g, [list(range(ST))], ck_views, v_ca, q2_lo,
                           q2_hi, a2_tok, s2c, psum_s, psum_o, psum_t,
                           awork, ca_act, tail_cb=tail_l2)

            # x3 stats + per-tile chain + quant into the wo2 projection
            S2x3 = g.qpool.tile([128, NT], F32, tag="S2x3", name="S2x3")
            amx3 = g.qpool.tile([128, NT], F32, tag="amx3", name="amx3")
            al_x3m = g.qpool.tile([128, NT], F32, tag="alx3m",
                                  name="alx3m")
            x3qT = resid3.tile([128, KT, T], BF16, name="x3qT")

            def post_x3(j):
                _rms_stats_tile(g, x3[j], D, S2x3, amx3, j)
                al_j, srnd_j = _rms_chain(g, S2x3[:, j:j + 1],
                                          amx3[:, j:j + 1], D, 1, f"x3{j}")
                nc.vector.tensor_copy(al_x3m[:, j:j + 1], al_j)
                _quant_tile(g, x3[j], D, srnd_j[:, 0:1],
                            x3qT[:, :, j * 128:(j + 1) * 128], g.work,
                            nc.sync)

            a2qT = x2qT        # x2qT is dead after the q2 projection
            wo_block(a2_tok, s2c, 'ca_g', 'ca_b', 'ca_wo', x2, x3, a2qT,
                     post_x3, "l2", stats_l2)
        es_cond.close()
        es_wca.close()
        es_x2.close()

        g.w['w2'] = load_weight(ffn_w, 'w2', nc.gpsimd)

        # ---- FFN ----
        with tc.tile_pool(name="ffn_act", bufs=1) as ffn_act, \
             tc.tile_pool(name="ffn_wk", bufs=1) as ffn_wk, \
             tc.tile_pool(name="outp", bufs=2) as outp:
            # per-tile pipeline, SKEWED two tiles: w2(j) is emitted after
            # w1(j+2), so the PE's in-order queue gives tile j's quant
            # chain two w1 blocks (~15us) of latency cover instead of
            # stalling the PE on every tile.
            hq_tiles = [None] * NT
            ah_tiles = [None] * NT

            def w1_block(j):
                a3 = g.stat.tile([128, 1], F32, tag=f"a3_{j}",
                                 name=f"a3_{j}")
                nc.vector.tensor_mul(a3, al_x3m[:, j:j + 1], msc['w1'])
                h_j = ffn_act.tile([128, H4], BF16, tag="h_j",
                                   name="h_j", bufs=3)
                for c in range(6):
                    ps = psw1.tile([128, 512], F32, tag="ps", name="ps_h")
                    for k in range(KT):
                        nc.tensor.matmul(
                            ps, x3qT[:, k, j * 128:(j + 1) * 128],
                            g.w['w1'][:, k, c * 512:(c + 1) * 512],
                            start=(k == 0), stop=(k == KT - 1))
                    nc.scalar.activation(h_j[:, c * 512:(c + 1) * 512],
                                         ps, ACT.Gelu, bias=0.0, scale=a3)
                S2h = g.qpool.tile([128, 1], F32, tag=f"S2h{j}",
                                   name=f"S2h{j}")
                amh = g.qpool.tile([128, 1], F32, tag=f"amh{j}",
                                   name=f"amh{j}")
                _rms_stats_tile(g, h_j, H4, S2h, amh, 0)
                al_h, srnd_h = _rms_chain(g, S2h, amh, H4, 1, f"h{j}")
                hqT = ffn_act.tile([128, KTH, 128], BF16, tag="hqT",
                                   name="hqT", bufs=3)
                _quant_tile(g, h_j, H4, srnd_h[:, 0:1], hqT, ffn_wk,
                            nc.sync)
                ah = g.stat.tile([128, 1], F32, tag=f"ah{j}",
                                 name=f"ah{j}")
                nc.vector.tensor_mul(ah, al_h[:, 0:1], msc['w2'])
                hq_tiles[j] = hqT
                ah_tiles[j] = ah

            def w2_block(j):
                hqT, ah = hq_tiles[j], ah_tiles[j]
                xo = outp.tile([128, D], F32, tag="xo", name="xo")
                for c in range(2):
                    ps = psw2.tile([128, 384], F32, tag="ps", name="ps_w2")
                    for k in range(KTH):
                        nc.tensor.matmul(
                            ps, hqT[:, k, :],
                            g.w['w2'][:, k, c * 384:(c + 1) * 384],
                            start=(k == 0), stop=(k == KTH - 1))
                    nc.vector.scalar_tensor_tensor(
                        xo[:, c * 384:(c + 1) * 384], ps, ah,
                        x3[j][:, c * 384:(c + 1) * 384], OP.mult, OP.add)
                nc.sync.dma_start(out_sh[j * 128:(j + 1) * 128, :], xo)

            with tc.tile_pool(name="ps_w1", bufs=4, space="PSUM") as psw1, \
                 tc.tile_pool(name="ps_w2", bufs=3, space="PSUM") as psw2:
                for j in range(NT + 2):
                    if j < NT:
                        w1_block(j)
                    if j >= 2:
                        w2_block(j - 2)
        es_ffnw.close()

    nc.finalize()
    return nc


def _get_program(key):
    if key not in _PROGRAM_CACHE:
        groups, fused = key
        _PROGRAM_CACHE[key] = build_program(
            GROUPS if groups == "full" else [[0]], fused_ln=fused)
    return _PROGRAM_CACHE[key]


LAST_RESULT = None


def _host_quant(w):
    """Exact ternary weight quant (same math as reference _weight_quant)."""
    w = np.asarray(w, np.float32)
    m = np.float32(np.mean(np.abs(w), dtype=np.float32))
    m = np.float32(max(m, np.float32(1e-5)))
    q = np.clip(np.rint(w / m), -1.0, 1.0)
    return q.astype(np.float32), m


def kernel(**inputs):
    """Full-input entry: shard across 8 cores, run, gather."""
    global LAST_RESULT
    x = np.ascontiguousarray(np.asarray(inputs['x'], dtype=np.float32))
    y = np.ascontiguousarray(np.asarray(inputs['y'], dtype=np.float32))

    fused = all(
        np.allclose(np.asarray(inputs[k], np.float32), v, atol=0.0)
        for k, v in (('sa_g', 1.0), ('sa_b', 0.0),
                     ('ca_g', 1.0), ('ca_b', 0.0)))
    nc = _get_program(("full", fused))

    qrows = np.concatenate([np.arange(h * 64, (h + 1) * 64)
                            for h in QPERM])
    combo = np.zeros((1, COMBO_W), np.float32)
    common = {}
    for i, name in enumerate(SCALE_SLOTS):
        q, m = _host_quant(inputs[name])
        if name in ('sa_wq', 'ca_wq'):
            q = q[qrows, :]
            m = m / np.float32(np.sqrt(np.float32(HEAD)))
        combo[0, i] = m
        common[f"{name}_q"] = np.ascontiguousarray(
            q.T.astype(ml_dtypes.bfloat16))
    for i, name in enumerate(('sa_g', 'sa_b', 'ca_g', 'ca_b')):
        combo[0, NSLOT + i * D:NSLOT + (i + 1) * D] = np.asarray(
            inputs[name], np.float32)
    common['combo'] = combo

    in_maps = []
    for c in range(NCORES):
        b, seg = c // GSZ, c % GSZ
        m = dict(common)
        m['x_sh'] = np.ascontiguousarray(x[b, seg * T:(seg + 1) * T, :])
        m['y_b'] = np.ascontiguousarray(y[b])
        in_maps.append(m)
    res = run_bass_kernel_spmd(nc, in_maps, core_ids=list(range(NCORES)))
    LAST_RESULT = res
    out = np.empty((B, N, D), np.float32)
    for c in range(NCORES):
        b, seg = c // GSZ, c % GSZ
        out[b, seg * T:(seg + 1) * T, :] = res.results[c]['out_sh']
    return out
